# revision 68
# baseline (speedup 1.0000x reference)
"""GTrXL layer (TransformerXL attention + GRU gating) on 8 TRN2 NeuronCores.

Sharding: pure data-parallel over batch (BS=8 -> 1 batch element per core).
No collectives. Per-core Bass/Tile kernel computes the full layer for its
batch element.

Layout convention on-chip: activations are kept TRANSPOSED [feature, token]
(feature on partitions, 128-chunks) so that weight matrices in natural [K, N]
layout serve as the stationary matmul operand and matmul outputs land
transposed again:  outT[n, t] = sum_k W[k, n] * xT[k, t].

Matmul compute in bf16 (f32 accumulate in PSUM); LN/softmax/GRU elementwise
math in f32.

Relative-shift: pos scores P[i, relk] are written per 128-query-chunk to a
DRAM scratch of row stride 1536 whose tail 512 columns are pre-filled with
-1e30; the shifted read  shifted[i, j] = P[i, 511 + j - i]  is a single
strided DMA (row step 1535), and the pad lands exactly on the masked region
j > i + 512, so masking comes for free.
"""

import sys

if '/opt/trn_rl_repo' not in sys.path:
    sys.path.insert(0, '/opt/trn_rl_repo')

import numpy as np
import ml_dtypes

import concourse.bass as bass
import concourse.tile as tile
from concourse import bacc, mybir
from concourse.bass_utils import run_bass_kernel_spmd
from concourse.masks import make_identity

BF16 = mybir.dt.bfloat16
F32 = mybir.dt.float32
FP8 = mybir.dt.float8e4
DR = mybir.MatmulPerfMode.DoubleRow
SW = 32.0      # fp8 weight scale
SA = 16.0      # fp8 activation scale
SWA = SW * SA  # psum scale for fp8 DoubleRow chains

HEAD_NUM, HEAD_DIM = 16, 64
D, HID = 1024, 4096
CUR, PREV, BS = 512, 512, 8
FULL = CUR + PREV
EPS = 1e-5
SCALE = 1.0 / (HEAD_DIM ** 0.5)
P = 128
DC = D // P          # 8 feature chunks
HC = HID // P        # 32 hidden chunks
TCF = FULL // P      # 8 full-token chunks
TCC = CUR // P       # 4 query-token chunks
NEG = -1.0e30

AluOp = mybir.AluOpType
Act = mybir.ActivationFunctionType

# set from the actual inputs in kernel(); enables 2-op DVE relu evacuations
_ZERO_BIAS = [False]


def _dram_in(dram, name, shape, dtype):
    return dram.tile(list(shape), dtype, kind="ExternalInput", name=name,
                     uniquify=False)


def _mm_chain(nc, psum, lhsT_tiles, rhs_tiles):
    n = len(lhsT_tiles)
    for i in range(n):
        nc.tensor.matmul(psum, lhsT=lhsT_tiles[i], rhs=rhs_tiles[i],
                         start=(i == 0), stop=(i == n - 1))


def _build():
    nc = bacc.Bacc("TRN2", target_bir_lowering=False)
    with tile.TileContext(nc) as tc:
        _emit(nc, tc)
    nc.compile()
    return nc


def _emit(nc, tc):
    from contextlib import ExitStack

    with ExitStack() as root:
        dram = root.enter_context(tc.tile_pool(name="io", bufs=1, space="DRAM"))

        # ---------------- DRAM I/O ----------------
        xT8_d = _dram_in(dram, "xT8", (D, FULL), FP8)
        inpT_d = _dram_in(dram, "inpT", (D, CUR), F32)
        inpT8_d = _dram_in(dram, "inpT8", (D, CUR), FP8)
        posT8_d = _dram_in(dram, "posT8", (D, FULL), FP8)
        u_d = _dram_in(dram, "u_t", (P, DC), F32)
        v_d = _dram_in(dram, "v_t", (P, DC), F32)
        ln1g_d = _dram_in(dram, "ln1_g_t", (P, DC), F32)
        ln1b_d = _dram_in(dram, "ln1_b16_t", (P, DC), F32)
        ln2g_d = _dram_in(dram, "ln2_g_t", (P, DC), F32)
        ln2b_d = _dram_in(dram, "ln2_b_t", (P, DC), F32)
        bproj_d = _dram_in(dram, "bproj_t", (P, DC), F32)
        b1_d = _dram_in(dram, "b1_t", (P, HC), F32)
        b2_d = _dram_in(dram, "b2_t", (P, DC), F32)
        nbg1_d = _dram_in(dram, "nbg1_t", (P, DC), F32)
        nbg2_d = _dram_in(dram, "nbg2_t", (P, DC), F32)

        wkv_d = _dram_in(dram, "Wkv8", (D, 2 * D), FP8)
        wq_d = _dram_in(dram, "Wq8", (D, D), FP8)
        wpos_d = _dram_in(dram, "Wpos8", (D, D), FP8)
        wproj_d = _dram_in(dram, "Wproj8", (D, D), FP8)
        gw_d = {}
        for g in (1, 2):
            for m in ("Wr", "Ur", "Wz", "Uz", "Wg", "Ug"):
                gw_d[(g, m)] = _dram_in(dram, f"g{g}_{m}8", (D, D), FP8)
        w1_d = _dram_in(dram, "mlp_W18", (D, HID), FP8)
        w2_d = _dram_in(dram, "mlp_W28", (HID, D), FP8)

        out_d = dram.tile([D, CUR], F32, kind="ExternalOutput", name="out",
                          uniquify=False)

        # one [4ic, P, 1536] scratch per head-parity; fixed ic slots keep the
        # NEG pad region [1024+128*ic, 1536) stable across reuses
        scr = [dram.tile([TCC, P, 1536], BF16, name=f"scr{s}") for s in range(2)]

        # ---------------- constants ----------------
        const = root.enter_context(tc.tile_pool(name="const", bufs=1))
        ident_b = const.tile([P, P], BF16)
        make_identity(nc, ident_b)
        # pair-dim stride must be a multiple of 16B for DoubleRow ldweights
        ones8_t = const.tile([P, DC, 16], FP8)
        nc.vector.memset(ones8_t, 1.0)
        eps_t = const.tile([P, 1], F32)
        nc.vector.memset(eps_t, EPS)

        def cload(name, dref, shape, dtype=F32):
            t = const.tile(list(shape), dtype, name=name)
            nc.sync.dma_start(out=t, in_=dref[:])
            return t

        u_sb = cload("u_sb", u_d, (P, DC))
        v_sb = cload("v_sb", v_d, (P, DC))
        ln1g_sb = cload("ln1g_sb", ln1g_d, (P, DC))
        ln1b_sb = cload("ln1b_sb", ln1b_d, (P, DC))
        ln2g_sb = cload("ln2g_sb", ln2g_d, (P, DC))
        ln2b_sb = cload("ln2b_sb", ln2b_d, (P, DC))
        bproj_sb = cload("bproj_sb", bproj_d, (P, DC))
        b1_sb = cload("b1_sb", b1_d, (P, HC))
        b2_sb = cload("b2_sb", b2_d, (P, DC))
        nbg1_sb = cload("nbg1_sb", nbg1_d, (P, DC))
        nbg2_sb = cload("nbg2_sb", nbg2_d, (P, DC))

        padw = const.tile([P, 512], BF16)
        nc.vector.memset(padw, NEG)
        for s in range(2):
            for ic in range(TCC):
                off = 1024 + ic * P
                nc.scalar.dma_start(out=scr[s][ic, :, off:1536],
                                    in_=padw[:, 0:1536 - off])

        # shared psum pools (4 + 2 + 2 = 8 banks)
        psum = root.enter_context(tc.tile_pool(name="psum", bufs=4, space="PSUM"))
        psum_t = root.enter_context(tc.tile_pool(name="psum_t", bufs=2, space="PSUM"))
        psum_s = root.enter_context(tc.tile_pool(name="psum_s", bufs=2, space="PSUM"))

        def PS():
            return psum.tile([P, 512], F32, name="ps", tag="ps")

        def SM():
            return psum_s.tile([1, 512], F32, name="sm", tag="sm")

        # lifetime-managed activations (two-sided stack allocator:
        # frees must be LIFO per side, so lifetimes are laid out on
        # left/right stacks to nest properly)
        def mk(name, shape, dtype, side):
            t, fr = tc.tile(list(shape), dtype, name=name, side=side)
            return t, fr

        x1T, fr_x1T = mk("x1T", (P, DC, FULL), FP8, "left")

        # ================= Phase 1: LN1 (feature-major) =================
        # Host supplies xT8 = fp8(SA * x.T).  Stats come from fp8 DR ones-
        # matmuls (mean) and squared copies (var); normalize+affine applies
        # gamma/beta as per-partition scalars, writing x1T = SA*LN(x) in fp8.
        with ExitStack() as ph:
            xw = ph.enter_context(tc.tile_pool(name="xw", bufs=1, side="left"))
            lt = ph.enter_context(tc.tile_pool(name="lt", bufs=3, side="left"))
            xT8 = xw.tile([P, DC, FULL], FP8)
            nc.sync.dma_start(out=xT8, in_=xT8_d[:].rearrange("(kc p) t -> p kc t", p=P))
            sq8 = xw.tile([P, DC, FULL], FP8)
            for k in range(DC):
                for th in range(2):
                    sl = (slice(None), k, slice(th * 512, (th + 1) * 512))
                    if (2 * k + th) % 2 == 0:
                        # 8x^2 = Square(SA*x * sqrt(8)/SA) on the ACT LUT
                        nc.scalar.activation(out=sq8[sl], in_=xT8[sl],
                                             func=Act.Square,
                                             scale=float(np.sqrt(8.0) / SA))
                    else:
                        nc.vector.scalar_tensor_tensor(
                            out=sq8[sl], in0=xT8[sl], scalar=8.0 / (SA * SA),
                            in1=xT8[sl], op0=AluOp.mult, op1=AluOp.mult)
            for th in range(2):
                ts = slice(th * 512, (th + 1) * 512)
                s1 = SM()
                for j in range(DC // 2):
                    nc.tensor.matmul(s1, lhsT=ones8_t[:, 2 * j:2 * j + 2, 0:1],
                                     rhs=xT8[:, 2 * j:2 * j + 2, ts],
                                     start=(j == 0), stop=(j == DC // 2 - 1),
                                     perf_mode=DR)
                s2 = SM()
                for j in range(DC // 2):
                    nc.tensor.matmul(s2, lhsT=ones8_t[:, 2 * j:2 * j + 2, 0:1],
                                     rhs=sq8[:, 2 * j:2 * j + 2, ts],
                                     start=(j == 0), stop=(j == DC // 2 - 1),
                                     perf_mode=DR)
                m16 = lt.tile([1, 512], F32, name="m16")
                nc.vector.tensor_scalar_mul(m16, s1, 1.0 / D)      # SA*mean
                ex2 = lt.tile([1, 512], F32, name="ex2")
                nc.vector.tensor_scalar_mul(ex2, s2, 1.0 / (8 * D))  # E[x^2]
                msq = lt.tile([1, 512], F32, name="msq")
                nc.vector.scalar_tensor_tensor(out=msq, in0=m16,
                                               scalar=1.0 / (SA * SA), in1=m16,
                                               op0=AluOp.mult, op1=AluOp.mult)
                var = lt.tile([1, 512], F32, name="var")
                nc.vector.tensor_sub(var, ex2, msq)
                sd = lt.tile([1, 512], F32, name="sd1")
                nc.scalar.activation(out=sd, in_=var, func=Act.Sqrt,
                                     bias=eps_t[0:1, :])
                rstd = lt.tile([1, 512], F32, name="rstd1")
                nc.vector.reciprocal(out=rstd, in_=sd)
                meanB = lt.tile([P, 512], F32, name="meanB1")
                nc.gpsimd.partition_broadcast(meanB, m16)
                rstdB = lt.tile([P, 512], F32, name="rstdB1")
                nc.gpsimd.partition_broadcast(rstdB, rstd)
                for k in range(DC):
                    sl = (slice(None), k, ts)
                    t1 = lt.tile([P, 512], F32, name="t1a")
                    eng = nc.gpsimd if k % 2 == 0 else nc.vector
                    eng.tensor_sub(t1, xT8[sl], meanB)             # SA*(x-m)
                    t2 = lt.tile([P, 512], F32, name="t2a")
                    eng2 = nc.gpsimd if k % 2 == 1 else nc.vector
                    eng2.tensor_mul(t2, t1, rstdB)                 # SA*xhat
                    # x1T = SA*(xhat*g + b), alternating ACT/DVE
                    if k % 2 == 0:
                        nc.scalar.activation(out=x1T[sl], in_=t2,
                                             func=Act.Identity,
                                             scale=ln1g_sb[:, k:k + 1],
                                             bias=ln1b_sb[:, k:k + 1])
                    else:
                        nc.vector.tensor_scalar(out=x1T[sl], in0=t2,
                                                scalar1=ln1g_sb[:, k:k + 1],
                                                scalar2=ln1b_sb[:, k:k + 1],
                                                op0=AluOp.mult, op1=AluOp.add)

        # ================= Phase 2: KT, V, qT, rT =================
        kT, fr_kT = mk("kT", (P, DC, FULL), BF16, "right")
        v8, fr_v = mk("v8", (P, TCF, HEAD_NUM, 65), FP8, "right")
        # ones column (value SA) folds the softmax denominator into the AV mm
        nc.vector.memset(v8[:, :, :, 64:65], SA)
        rT, fr_rT = mk("rT", (P, DC, FULL), BF16, "right")
        quT, fr_quT = mk("quT", (P, DC, CUR), BF16, "right")
        qvT, fr_qvT = mk("qvT", (P, DC, CUR), BF16, "right")

        def dr4(ps, w, rhs, nsl, tsl):
            for j in range(DC // 2):
                nc.tensor.matmul(ps, lhsT=w[:, 2 * j:2 * j + 2, nsl],
                                 rhs=rhs[:, 2 * j:2 * j + 2, tsl],
                                 start=(j == 0), stop=(j == DC // 2 - 1),
                                 perf_mode=DR)

        with ExitStack() as ph:
            wkvp = ph.enter_context(tc.tile_pool(name="wkvp", bufs=1, side="right"))
            wkv = wkvp.tile([P, DC, 2 * D], FP8)
            nc.sync.dma_start(out=wkv, in_=wkv_d[:].rearrange("(kc p) n -> p kc n", p=P))
            for n in range(DC):
                for th in range(2):
                    ps = PS()
                    dr4(ps, wkv, x1T, slice(n * P, (n + 1) * P),
                        slice(th * 512, (th + 1) * 512))
                    # k bias is softmax-invariant; kT = psum/SWA (true scale)
                    nc.scalar.activation(out=kT[:, n, th * 512:(th + 1) * 512],
                                         in_=ps, func=Act.Copy, scale=1.0 / SWA)
            for t in range(TCF):
                for nh in range(2):
                    ps = PS()
                    for j in range(DC // 2):
                        nc.tensor.matmul(ps, lhsT=x1T[:, 2 * j:2 * j + 2, t * P:(t + 1) * P],
                                         rhs=wkv[:, 2 * j:2 * j + 2, D + nh * 512:D + (nh + 1) * 512],
                                         start=(j == 0), stop=(j == DC // 2 - 1),
                                         perf_mode=DR)
                    # v8[tok, head, 0:64] = SA * v; col 64 of each slot is SA
                    nc.scalar.activation(
                        out=v8[:, t, nh * 8:(nh + 1) * 8, 0:64],
                        in_=ps[:, :].rearrange("p (h d) -> p h d", d=64),
                        func=Act.Copy, scale=SA / SWA)
        with ExitStack() as ph:
            wqp = ph.enter_context(tc.tile_pool(name="wqp", bufs=1, side="right"))
            wq = wqp.tile([P, DC, D], FP8)
            nc.sync.dma_start(out=wq, in_=wq_d[:].rearrange("(kc p) n -> p kc n", p=P))
            for n in range(DC):
                ps = PS()
                dr4(ps, wq, x1T, slice(n * P, (n + 1) * P), slice(CUR, FULL))
                # u_sb/v_sb hold u+bq / v+bq (host-folded)
                nc.vector.tensor_scalar(out=quT[:, n, :], in0=ps,
                                        scalar1=1.0 / SWA, scalar2=u_sb[:, n:n + 1],
                                        op0=AluOp.mult, op1=AluOp.add)
                nc.vector.tensor_scalar(out=qvT[:, n, :], in0=ps,
                                        scalar1=1.0 / SWA, scalar2=v_sb[:, n:n + 1],
                                        op0=AluOp.mult, op1=AluOp.add)
        with ExitStack() as ph:
            wpp = ph.enter_context(tc.tile_pool(name="wpp", bufs=1, side="right"))
            wpos = wpp.tile([P, DC, D], FP8)
            nc.sync.dma_start(out=wpos, in_=wpos_d[:].rearrange("(kc p) n -> p kc n", p=P))
            posT_sb = wpp.tile([P, DC, FULL], FP8)
            nc.sync.dma_start(out=posT_sb, in_=posT8_d[:].rearrange("(kc p) f -> p kc f", p=P))
            for n in range(DC):
                for fh in range(2):
                    ps = PS()
                    dr4(ps, wpos, posT_sb, slice(n * P, (n + 1) * P),
                        slice(fh * 512, (fh + 1) * 512))
                    # pos bias is softmax-invariant after rel-shift; drop it
                    nc.scalar.activation(out=rT[:, n, fh * 512:(fh + 1) * 512],
                                         in_=ps, func=Act.Copy, scale=1.0 / SWA)
        fr_x1T()

        # ================= Phase 3: attention =================
        # Scores are computed query-major (trimmed to the causal triangle),
        # summed in bf16, PE-transposed into a psum bank per key chunk, and
        # exp-evacuated by ACT straight into fp8 attnT.  The AV matmul runs
        # fp8 DoubleRow against v8 whose 65th column (=SA) yields the softmax
        # denominator as psum row 64 for free.
        avT, fr_avT = mk("avT", (P, DC, CUR), FP8, "left")
        with ExitStack() as ph:
            aw = ph.enter_context(tc.tile_pool(name="aw", bufs=3, side="left"))
            rw = ph.enter_context(tc.tile_pool(name="rw", bufs=2, side="left"))

            def head_slices(h):
                ch, rb = h // 2, (h % 2) * HEAD_DIM
                return (quT[rb:rb + HEAD_DIM, ch, :], qvT[rb:rb + HEAD_DIM, ch, :],
                        kT[rb:rb + HEAD_DIM, ch, :], rT[rb:rb + HEAD_DIM, ch, :])

            def pos_stage(h):
                """Pos scores -> scratch -> single combined shifted read.
                Write at col r+128*ic so the fixed read offset 511 yields the
                reference shift P[i, 511+j-i] with global i."""
                _, qvh, _, rh = head_slices(h)
                s_t = scr[h % 2]
                for ic in range(TCC):
                    w_ic = (ic + 5) * P
                    r_lo = FULL - w_ic
                    pb = aw.tile([P, FULL], BF16, name="pb", bufs=4)
                    for r0, r1 in ((r_lo, min(r_lo + 512, FULL)),
                                   (min(r_lo + 512, FULL), FULL)):
                        if r1 <= r0:
                            continue
                        pp = PS()
                        nc.tensor.matmul(pp[:, 0:r1 - r0],
                                         lhsT=qvh[:, ic * P:(ic + 1) * P],
                                         rhs=rh[:, r0:r1], start=True, stop=True)
                        if ic < 2:
                            nc.scalar.copy(pb[:, r0:r1], pp[:, 0:r1 - r0])
                        else:
                            nc.vector.tensor_copy(pb[:, r0:r1], pp[:, 0:r1 - r0])
                    eng = nc.gpsimd if ic % 2 == 0 else nc.sync
                    eng.dma_start(out=s_t[ic, :, 384:1024 + ic * P],
                                  in_=pb[:, r_lo:1024])
                shp = aw.tile([P, TCC, FULL], BF16, name="shp", bufs=2)
                shift_ap = bass.AP(tensor=s_t.tensor, offset=s_t.offset + 511,
                                   ap=[[1535, P], [P * 1536, TCC], [1, FULL]])
                nc.gpsimd.dma_start(out=shp, in_=shift_ap)
                return shp

            def rest_stage(h, shp):
                quh, _, kh, _ = head_slices(h)
                attnT = aw.tile([P, TCF, 512], FP8, name="attnT", bufs=2)
                sms = []
                for ic in range(TCC):
                    w_ic = (ic + 5) * P
                    sm = aw.tile([P, FULL], BF16, name="sm", bufs=8)
                    for j0, j1 in ((0, 512), (512, w_ic)):
                        if j1 <= j0:
                            continue
                        cp = PS()
                        if h % 2 == 0:
                            # fold pos into psum via identity matmul; ACT
                            # evacuates (balances DVE)
                            nc.tensor.matmul(cp[:, 0:j1 - j0],
                                             lhsT=quh[:, ic * P:(ic + 1) * P],
                                             rhs=kh[:, j0:j1], start=True,
                                             stop=False)
                            nc.tensor.matmul(cp[:, 0:j1 - j0], lhsT=ident_b,
                                             rhs=shp[:, ic, j0:j1],
                                             start=False, stop=True)
                            nc.scalar.copy(sm[:, j0:j1], cp[:, 0:j1 - j0])
                        else:
                            nc.tensor.matmul(cp[:, 0:j1 - j0],
                                             lhsT=quh[:, ic * P:(ic + 1) * P],
                                             rhs=kh[:, j0:j1], start=True,
                                             stop=True)
                            nc.vector.tensor_add(sm[:, j0:j1], cp[:, 0:j1 - j0],
                                                 shp[:, ic, j0:j1])
                    sms.append(sm)
                for jc in range(TCF):
                    ic_lo = max(0, jc - 4)
                    tp = psum_t.tile([P, 512], BF16, name="ptb", tag="pt")
                    for ic in range(ic_lo, TCC):
                        nc.tensor.matmul(tp[:, ic * P:(ic + 1) * P],
                                         lhsT=sms[ic][:, jc * P:(jc + 1) * P],
                                         rhs=ident_b, is_transpose=True,
                                         start=(ic == ic_lo), stop=(ic == TCC - 1))
                    nc.scalar.activation(out=attnT[:, jc, ic_lo * P:512],
                                         in_=tp[:, ic_lo * P:512],
                                         func=Act.Exp, scale=SCALE)
                # AV + denominator: rows 0..63 = SA*unnorm, row 64 = SA*den
                av = PS()
                mms = []
                for ic in range(TCC):
                    njc = ic + 5
                    qs = slice(ic * P, (ic + 1) * P)
                    for j in range(0, njc - 1, 2):
                        mms.append((qs, j, True))
                    if njc % 2 == 1:
                        mms.append((qs, njc - 1, False))
                for i, (qs, j, is_dr) in enumerate(mms):
                    fl = dict(start=(i == 0), stop=(i == len(mms) - 1))
                    if is_dr:
                        nc.tensor.matmul(av[0:65, qs], lhsT=v8[:, j:j + 2, h, :],
                                         rhs=attnT[:, j:j + 2, qs],
                                         perf_mode=DR, **fl)
                    else:
                        nc.tensor.matmul(av[0:65, qs], lhsT=v8[:, j, h, :],
                                         rhs=attnT[:, j, qs], **fl)
                ch, rb = h // 2, (h % 2) * HEAD_DIM
                recip = rw.tile([1, 512], F32, name="recip")
                nc.vector.reciprocal(out=recip, in_=av[64:65, :])
                recipB = rw.tile([HEAD_DIM, 512], F32, name="recipB")
                nc.gpsimd.partition_broadcast(recipB, recip)
                # avT8 = SA * av_norm  (SA psum scales cancel in the ratio)
                nc.vector.scalar_tensor_tensor(out=avT[rb:rb + HEAD_DIM, ch, :],
                                               in0=av[0:HEAD_DIM, :], scalar=SA,
                                               in1=recipB, op0=AluOp.mult,
                                               op1=AluOp.mult)

            # software pipeline: pos DMA round-trip of head h+1 overlaps the
            # score/transpose/av work of head h
            pend = pos_stage(0)
            for h in range(HEAD_NUM):
                nxt = pos_stage(h + 1) if h + 1 < HEAD_NUM else None
                rest_stage(h, pend)
                pend = nxt
        fr_qvT(); fr_quT(); fr_rT(); fr_v(); fr_kT()

        # ================= Phase 4: proj + GRU1 =================
        a1T, fr_a1T = mk("a1T", (P, DC, CUR), FP8, "right")
        with ExitStack() as ph:
            wpr = ph.enter_context(tc.tile_pool(name="wpr", bufs=1, side="left"))
            wproj = wpr.tile([P, DC, D], FP8)
            nc.sync.dma_start(out=wproj, in_=wproj_d[:].rearrange("(kc p) n -> p kc n", p=P))
            for n in range(DC):
                ps = PS()
                for j in range(DC // 2):
                    nc.tensor.matmul(ps, lhsT=wproj[:, 2 * j:2 * j + 2, n * P:(n + 1) * P],
                                     rhs=avT[:, 2 * j:2 * j + 2, :],
                                     start=(j == 0), stop=(j == DC // 2 - 1),
                                     perf_mode=DR)
                # a1T8 = SA * relu(pre + bproj); bproj_sb holds SA*bproj
                nc.scalar.activation(out=a1T[:, n, :], in_=ps, func=Act.Relu,
                                     bias=bproj_sb[:, n:n + 1], scale=SA / SWA)
        fr_avT()

        o1T_f, fr_o1f = mk("o1T_f", (P, DC, CUR), F32, "left")
        o1T_8, fr_o1b = mk("o1T_8", (P, DC, CUR), FP8, "left")
        inpT_f, fr_inpf = mk("inpT_f", (P, DC, CUR), F32, "left")
        inpT_8, fr_inpb = mk("inpT_8", (P, DC, CUR), FP8, "left")
        nc.sync.dma_start(out=inpT_f, in_=inpT_d[:].rearrange("(kc p) t -> p kc t", p=P))
        nc.sync.dma_start(out=inpT_8, in_=inpT8_d[:].rearrange("(kc p) t -> p kc t", p=P))
        with ExitStack() as ph:
            _gru(nc, tc, ph, PS, gw_d, 1, a1T, inpT_8, inpT_f, nbg1_sb,
                 o1T_f, o1T_8)
        fr_inpb(); fr_inpf(); fr_a1T()

        # ================= Phase 5: LN2 =================
        x2T, fr_x2T = mk("x2T", (P, DC, CUR), FP8, "right")
        with ExitStack() as ph:
            lw = ph.enter_context(tc.tile_pool(name="lw", bufs=2, side="left"))
            sqp = ph.enter_context(tc.tile_pool(name="sqp", bufs=1, side="left"))
            sq = sqp.tile([P, DC, 512], FP8, name="sq")
            for n in range(DC):
                # 4*o1^2 fits fp8 range comfortably
                nc.vector.scalar_tensor_tensor(out=sq[:, n, :], in0=o1T_f[:, n, :],
                                               scalar=4.0, in1=o1T_f[:, n, :],
                                               op0=AluOp.mult, op1=AluOp.mult)
            s1 = SM()
            for j in range(DC // 2):
                nc.tensor.matmul(s1, lhsT=ones8_t[:, 2 * j:2 * j + 2, 0:1],
                                 rhs=o1T_8[:, 2 * j:2 * j + 2, :],
                                 start=(j == 0), stop=(j == DC // 2 - 1),
                                 perf_mode=DR)
            mean = lw.tile([1, 512], F32, name="mean")
            nc.vector.tensor_scalar_mul(mean, s1, 1.0 / (SA * D))
            s2 = SM()
            for j in range(DC // 2):
                nc.tensor.matmul(s2, lhsT=ones8_t[:, 2 * j:2 * j + 2, 0:1],
                                 rhs=sq[:, 2 * j:2 * j + 2, :],
                                 start=(j == 0), stop=(j == DC // 2 - 1),
                                 perf_mode=DR)
            m2m = lw.tile([1, 512], F32, name="m2m")
            nc.vector.tensor_scalar_mul(m2m, s2, 1.0 / (4 * D))
            var = lw.tile([1, 512], F32, name="var")
            nc.vector.scalar_tensor_tensor(out=var, in0=mean, scalar=1.0,
                                           in1=mean, op0=AluOp.mult,
                                           op1=AluOp.mult)
            nc.vector.tensor_sub(var, m2m, var)
            sd = lw.tile([1, 512], F32, name="sd2")
            nc.scalar.activation(out=sd, in_=var, func=Act.Sqrt,
                                 bias=eps_t[0:1, :])
            rstd = lw.tile([1, 512], F32, name="rstd2")
            nc.vector.reciprocal(out=rstd, in_=sd)
            meanB = lw.tile([P, 512], F32, name="meanB")
            nc.gpsimd.partition_broadcast(meanB, mean)
            rstdB = lw.tile([P, 512], F32, name="rstdB")
            nc.gpsimd.partition_broadcast(rstdB, rstd)
            for n in range(DC):
                t1 = lw.tile([P, 512], F32, name="t1")
                nc.vector.tensor_sub(t1, o1T_f[:, n, :], meanB)
                nc.vector.tensor_mul(t1, t1, rstdB)
                nc.vector.tensor_scalar(out=x2T[:, n, :], in0=t1,
                                        scalar1=ln2g_sb[:, n:n + 1],
                                        scalar2=ln2b_sb[:, n:n + 1],
                                        op0=AluOp.mult, op1=AluOp.add)

        # ================= Phase 6: MLP (fp8 DoubleRow) =================
        with ExitStack() as ph6:
            m1w = ph6.enter_context(tc.tile_pool(name="m1w", bufs=1, side="right"))
            m1T = m1w.tile([P, HC, 512], FP8)
            with ExitStack() as ph:
                w1p = ph.enter_context(tc.tile_pool(name="w1p", bufs=4, side="right"))
                w1r = w1_d[:].rearrange("(kc p) n -> p kc n", p=P)
                for n in range(HC):
                    w1t = w1p.tile([P, DC, P], FP8, name="w1t", tag="w1t")
                    [nc.sync, nc.scalar, nc.gpsimd][n % 3].dma_start(
                        out=w1t, in_=w1r[:, :, n * P:(n + 1) * P])
                    ps = PS()
                    for j in range(DC // 2):
                        nc.tensor.matmul(ps, lhsT=w1t[:, 2 * j:2 * j + 2, :],
                                         rhs=x2T[:, 2 * j:2 * j + 2, :],
                                         start=(j == 0), stop=(j == DC // 2 - 1),
                                         perf_mode=DR)
                    # m1T8 = SA*relu(pre + b1); psum = SWA*pre; b1_sb = SA*b1
                    if _ZERO_BIAS[0] and n % 2 == 1:
                        nc.vector.tensor_scalar(out=m1T[:, n, :], in0=ps,
                                                scalar1=SA / SWA, scalar2=0.0,
                                                op0=AluOp.mult, op1=AluOp.max)
                    else:
                        nc.scalar.activation(out=m1T[:, n, :], in_=ps,
                                             func=Act.Relu,
                                             bias=b1_sb[:, n:n + 1],
                                             scale=SA / SWA)
            m2T, fr_m2T = mk("m2T", (P, DC, CUR), FP8, "left")
            w2p = ph6.enter_context(tc.tile_pool(name="w2p", bufs=3, side="left"))
            w2r = w2_d[:].rearrange("(kc p) n -> p kc n", p=P)
            for n in range(DC):
                w2t = w2p.tile([P, HC, P], FP8, name="w2t", tag="w2t")
                [nc.sync, nc.scalar, nc.gpsimd][n % 3].dma_start(
                    out=w2t, in_=w2r[:, :, n * P:(n + 1) * P])
                ps = PS()
                for j in range(HC // 2):
                    nc.tensor.matmul(ps, lhsT=w2t[:, 2 * j:2 * j + 2, :],
                                     rhs=m1T[:, 2 * j:2 * j + 2, :],
                                     start=(j == 0), stop=(j == HC // 2 - 1),
                                     perf_mode=DR)
                if _ZERO_BIAS[0] and n % 2 == 1:
                    nc.vector.tensor_scalar(out=m2T[:, n, :], in0=ps,
                                            scalar1=SA / SWA, scalar2=0.0,
                                            op0=AluOp.mult, op1=AluOp.max)
                else:
                    nc.scalar.activation(out=m2T[:, n, :], in_=ps, func=Act.Relu,
                                         bias=b2_sb[:, n:n + 1], scale=SA / SWA)
        fr_x2T()

        # ================= Phase 7: GRU2 =================
        o2T_f, fr_o2 = mk("o2T_f", (P, DC, CUR), F32, "right")
        with ExitStack() as ph:
            _gru(nc, tc, ph, PS, gw_d, 2, m2T, o1T_8, o1T_f, nbg2_sb,
                 o2T_f, None)
        fr_m2T(); fr_o1b(); fr_o1f()

        # ================= Phase 8: DMA transposed output =================
        # out_d holds o2 feature-major [D, CUR]; host transposes for free.
        # per-chunk DMAs overlap the GRU2 tail instead of one serial write.
        out_r = out_d[:].rearrange("(kc p) t -> p kc t", p=P)
        for n in range(DC):
            [nc.sync, nc.scalar, nc.gpsimd][n % 3].dma_start(
                out=out_r[:, n, :], in_=o2T_f[:, n, :])
        fr_o2()


def _gru(nc, tc, ph, PS, gw_d, g, yT, xT_8, xT_f, nbg_sb, oT_f, oT_8):
    """fp8 DoubleRow GRU gate. yT/xT_8 hold SA*value in fp8; weights SW*W.
    PSUM accumulates SWA*pre_act; Act applies 1/SWA before the nonlinearity."""
    gwp = ph.enter_context(tc.tile_pool(name=f"gw{g}", bufs=4, side="left"))
    gtmp = ph.enter_context(tc.tile_pool(name=f"gt{g}", bufs=2, side="left"))
    gper = ph.enter_context(tc.tile_pool(name=f"gp{g}", bufs=1, side="left"))

    # spread weight-stream transfer time across issue queues
    qeng = [nc.sync, nc.scalar, nc.gpsimd]
    qi = [0]

    def loadw(m):
        w = gwp.tile([P, DC, D], FP8, name=f"gwt_{m}", tag="gwt")
        qeng[qi[0] % 3].dma_start(
            out=w, in_=gw_d[(g, m)][:].rearrange("(kc p) n -> p kc n", p=P))
        qi[0] += 1
        return w

    def dr_chain(ps, w, u, rhs_w, rhs_u):
        for j in range(DC // 2):
            nc.tensor.matmul(ps, lhsT=w[:, 2 * j:2 * j + 2, n * P:(n + 1) * P],
                             rhs=rhs_w[:, 2 * j:2 * j + 2, :],
                             start=(j == 0), stop=False, perf_mode=DR)
        for j in range(DC // 2):
            nc.tensor.matmul(ps, lhsT=u[:, 2 * j:2 * j + 2, n * P:(n + 1) * P],
                             rhs=rhs_u[:, 2 * j:2 * j + 2, :],
                             start=False, stop=(j == DC // 2 - 1), perf_mode=DR)

    wr, ur = loadw("Wr"), loadw("Ur")
    rx = gper.tile([P, DC, 512], FP8, name="rx")
    for n in range(DC):
        ps = PS()
        dr_chain(ps, wr, ur, yT, xT_8)
        rr = gtmp.tile([P, 512], F32, name="rr")
        nc.scalar.activation(out=rr, in_=ps, func=Act.Sigmoid, scale=1.0 / SWA)
        # rx8 = SA * r * x
        nc.vector.scalar_tensor_tensor(out=rx[:, n, :], in0=rr, scalar=SA,
                                       in1=xT_f[:, n, :], op0=AluOp.mult,
                                       op1=AluOp.mult)
    wz, uz = loadw("Wz"), loadw("Uz")
    zt = gper.tile([P, DC, 512], F32, name="zt")
    for n in range(DC):
        ps = PS()
        dr_chain(ps, wz, uz, yT, xT_8)
        nc.scalar.activation(out=zt[:, n, :], in_=ps, func=Act.Sigmoid,
                             bias=nbg_sb[:, n:n + 1], scale=1.0 / SWA)
    wg, ug = loadw("Wg"), loadw("Ug")
    for n in range(DC):
        ps = PS()
        dr_chain(ps, wg, ug, yT, rx)
        ht = gtmp.tile([P, 512], F32, name="ht")
        nc.scalar.activation(out=ht, in_=ps, func=Act.Tanh, scale=1.0 / SWA)
        nc.gpsimd.tensor_sub(ht, ht, xT_f[:, n, :])
        nc.vector.tensor_mul(ht, ht, zt[:, n, :])
        nc.gpsimd.tensor_add(oT_f[:, n, :], ht, xT_f[:, n, :])
        if oT_8 is not None:
            nc.vector.tensor_scalar_mul(oT_8[:, n, :], oT_f[:, n, :], SA)


_NC_CACHE = {}


def _get_nc():
    if "nc" not in _NC_CACHE:
        _NC_CACHE["nc"] = _build()
    return _NC_CACHE["nc"]


def _chunk_t(vec):
    n = vec.shape[0] // P
    return np.ascontiguousarray(vec.reshape(n, P).T.astype(np.float32))


def _prep(inputs):
    f32 = np.float32
    bf = ml_dtypes.bfloat16
    fp8 = ml_dtypes.float8_e4m3
    inp = np.asarray(inputs["inputs"], f32)
    mem = np.asarray(inputs["memory"], f32)
    pos = np.asarray(inputs["pos_embedding"], f32)[:, 0, :]
    sw, sa = np.float32(SW), np.float32(SA)

    bq = np.asarray(inputs["bq"], f32)
    bkvV = np.asarray(inputs["bkv"], f32)[D:2 * D]
    wproj_f = np.asarray(inputs["Wproj"], f32)
    # v-bias shifts normalized av by a constant vector -> folds into bproj
    bproj_eff = np.asarray(inputs["bproj"], f32) + bkvV @ wproj_f
    shared = {
        "posT8": (sa * pos.T).astype(fp8),
        # bq folds into u and v (k/pos biases are softmax-invariant, dropped)
        "u_t": _chunk_t(np.asarray(inputs["u"], f32).reshape(-1) + bq),
        "v_t": _chunk_t(np.asarray(inputs["v"], f32).reshape(-1) + bq),
        "ln1_g_t": _chunk_t(np.asarray(inputs["ln1_g"], f32)),
        "ln1_b16_t": _chunk_t(sa * np.asarray(inputs["ln1_b"], f32)),
        "ln2_g_t": _chunk_t(sa * np.asarray(inputs["ln2_g"], f32)),
        "ln2_b_t": _chunk_t(sa * np.asarray(inputs["ln2_b"], f32)),
        "bproj_t": _chunk_t(sa * bproj_eff),
        "b1_t": _chunk_t(sa * np.asarray(inputs["mlp_b1"], f32)),
        "b2_t": _chunk_t(sa * np.asarray(inputs["mlp_b2"], f32)),
        "nbg1_t": _chunk_t(-np.asarray(inputs["g1_bg"], f32)),
        "nbg2_t": _chunk_t(-np.asarray(inputs["g2_bg"], f32)),
        "Wkv8": (sw * np.asarray(inputs["Wkv"], f32)).astype(fp8),
        "Wq8": (sw * np.asarray(inputs["Wq"], f32)).astype(fp8),
        "Wpos8": (sw * np.asarray(inputs["Wpos"], f32)).astype(fp8),
        "Wproj8": (sw * wproj_f).astype(fp8),
        "mlp_W18": (sw * np.asarray(inputs["mlp_W1"], f32)).astype(fp8),
        "mlp_W28": (sw * np.asarray(inputs["mlp_W2"], f32)).astype(fp8),
    }
    for g in (1, 2):
        for m in ("Wr", "Ur", "Wz", "Uz", "Wg", "Ug"):
            shared[f"g{g}_{m}8"] = (
                sw * np.asarray(inputs[f"g{g}_{m}"], f32)).astype(fp8)

    in_maps = []
    for b in range(BS):
        im = dict(shared)
        x_full = np.concatenate([mem[:, b, :], inp[:, b, :]], axis=0)
        im["xT8"] = (sa * x_full.T).astype(fp8)
        inpT = np.ascontiguousarray(inp[:, b, :].T)
        im["inpT"] = inpT
        im["inpT8"] = (sa * inpT).astype(fp8)
        in_maps.append(im)
    return in_maps


def kernel(**inputs):
    _ZERO_BIAS[0] = all(
        not np.any(np.asarray(inputs[k]))
        for k in ("mlp_b1", "mlp_b2", "bproj"))
    nc = _get_nc()
    in_maps = _prep(inputs)
    res = run_bass_kernel_spmd(nc, in_maps, core_ids=list(range(BS)))
    # device returns feature-major [D, CUR]; transpose back on host
    out = np.stack([res.results[b]["out"].T for b in range(BS)], axis=1)
    return np.ascontiguousarray(out.astype(np.float32))


if __name__ == "__main__":
    _get_nc()
    print("build+compile OK")



# revision 69
# speedup vs baseline: 1.0419x; 1.0419x over previous
"""GTrXL layer (TransformerXL attention + GRU gating) on 8 TRN2 NeuronCores.

Sharding: pure data-parallel over batch (BS=8 -> 1 batch element per core).
No collectives. Per-core Bass/Tile kernel computes the full layer for its
batch element.

Layout convention on-chip: activations are kept TRANSPOSED [feature, token]
(feature on partitions, 128-chunks) so that weight matrices in natural [K, N]
layout serve as the stationary matmul operand and matmul outputs land
transposed again:  outT[n, t] = sum_k W[k, n] * xT[k, t].

Matmul compute in bf16 (f32 accumulate in PSUM); LN/softmax/GRU elementwise
math in f32.

Relative-shift: pos scores P[i, relk] are written per 128-query-chunk to a
DRAM scratch of row stride 1536 whose tail 512 columns are pre-filled with
-1e30; the shifted read  shifted[i, j] = P[i, 511 + j - i]  is a single
strided DMA (row step 1535), and the pad lands exactly on the masked region
j > i + 512, so masking comes for free.
"""

import sys

if '/opt/trn_rl_repo' not in sys.path:
    sys.path.insert(0, '/opt/trn_rl_repo')

import numpy as np
import ml_dtypes

import concourse.bass as bass
import concourse.tile as tile
from concourse import bacc, mybir
from concourse.bass_utils import run_bass_kernel_spmd
from concourse.masks import make_identity

BF16 = mybir.dt.bfloat16
F32 = mybir.dt.float32
FP8 = mybir.dt.float8e4
DR = mybir.MatmulPerfMode.DoubleRow
SW = 32.0      # fp8 weight scale
SA = 16.0      # fp8 activation scale
SWA = SW * SA  # psum scale for fp8 DoubleRow chains

HEAD_NUM, HEAD_DIM = 16, 64
D, HID = 1024, 4096
CUR, PREV, BS = 512, 512, 8
FULL = CUR + PREV
EPS = 1e-5
SCALE = 1.0 / (HEAD_DIM ** 0.5)
P = 128
DC = D // P          # 8 feature chunks
HC = HID // P        # 32 hidden chunks
TCF = FULL // P      # 8 full-token chunks
TCC = CUR // P       # 4 query-token chunks
NEG = -1.0e30

AluOp = mybir.AluOpType
Act = mybir.ActivationFunctionType

# set from the actual inputs in kernel(); enables 2-op DVE relu evacuations
_ZERO_BIAS = [False]


def _dram_in(dram, name, shape, dtype):
    return dram.tile(list(shape), dtype, kind="ExternalInput", name=name,
                     uniquify=False)


def _mm_chain(nc, psum, lhsT_tiles, rhs_tiles):
    n = len(lhsT_tiles)
    for i in range(n):
        nc.tensor.matmul(psum, lhsT=lhsT_tiles[i], rhs=rhs_tiles[i],
                         start=(i == 0), stop=(i == n - 1))


def _build():
    nc = bacc.Bacc("TRN2", target_bir_lowering=False)
    with tile.TileContext(nc) as tc:
        _emit(nc, tc)
    nc.compile()
    return nc


def _emit(nc, tc):
    from contextlib import ExitStack

    with ExitStack() as root:
        dram = root.enter_context(tc.tile_pool(name="io", bufs=1, space="DRAM"))

        # ---------------- DRAM I/O ----------------
        xT8_d = _dram_in(dram, "xT8", (D, FULL), FP8)
        inpT_d = _dram_in(dram, "inpT", (D, CUR), F32)
        inpT8_d = _dram_in(dram, "inpT8", (D, CUR), FP8)
        posT8_d = _dram_in(dram, "posT8", (D, FULL), FP8)
        u_d = _dram_in(dram, "u_t", (P, DC), F32)
        v_d = _dram_in(dram, "v_t", (P, DC), F32)
        ln1g_d = _dram_in(dram, "ln1_g_t", (P, DC), F32)
        ln1b_d = _dram_in(dram, "ln1_b16_t", (P, DC), F32)
        ln2g_d = _dram_in(dram, "ln2_g_t", (P, DC), F32)
        ln2b_d = _dram_in(dram, "ln2_b_t", (P, DC), F32)
        bproj_d = _dram_in(dram, "bproj_t", (P, DC), F32)
        b1_d = _dram_in(dram, "b1_t", (P, HC), F32)
        b2_d = _dram_in(dram, "b2_t", (P, DC), F32)
        nbg1_d = _dram_in(dram, "nbg1_t", (P, DC), F32)
        nbg2_d = _dram_in(dram, "nbg2_t", (P, DC), F32)

        wkv_d = _dram_in(dram, "Wkv8", (D, 2 * D), FP8)
        wq_d = _dram_in(dram, "Wq8", (D, D), FP8)
        wpos_d = _dram_in(dram, "Wpos8", (D, D), FP8)
        wproj_d = _dram_in(dram, "Wproj8", (D, D), FP8)
        gw_d = {}
        for g in (1, 2):
            for m in ("Wr", "Ur", "Wz", "Uz", "Wg", "Ug"):
                gw_d[(g, m)] = _dram_in(dram, f"g{g}_{m}8", (D, D), FP8)
        w1_d = _dram_in(dram, "mlp_W18", (D, HID), FP8)
        w2_d = _dram_in(dram, "mlp_W28", (HID, D), FP8)

        out_d = dram.tile([D, CUR], F32, kind="ExternalOutput", name="out",
                          uniquify=False)

        # one [4ic, P, 1536] scratch per head-parity; fixed ic slots keep the
        # NEG pad region [1024+128*ic, 1536) stable across reuses
        scr = [dram.tile([TCC, P, 1536], BF16, name=f"scr{s}") for s in range(2)]

        # ---------------- constants ----------------
        const = root.enter_context(tc.tile_pool(name="const", bufs=1))
        ident_b = const.tile([P, P], BF16)
        make_identity(nc, ident_b)
        # pair-dim stride must be a multiple of 16B for DoubleRow ldweights
        ones8_t = const.tile([P, DC, 16], FP8)
        nc.vector.memset(ones8_t, 1.0)
        eps_t = const.tile([P, 1], F32)
        nc.vector.memset(eps_t, EPS)

        def cload(name, dref, shape, dtype=F32):
            t = const.tile(list(shape), dtype, name=name)
            nc.sync.dma_start(out=t, in_=dref[:])
            return t

        u_sb = cload("u_sb", u_d, (P, DC))
        v_sb = cload("v_sb", v_d, (P, DC))
        ln1g_sb = cload("ln1g_sb", ln1g_d, (P, DC))
        ln1b_sb = cload("ln1b_sb", ln1b_d, (P, DC))
        ln2g_sb = cload("ln2g_sb", ln2g_d, (P, DC))
        ln2b_sb = cload("ln2b_sb", ln2b_d, (P, DC))
        bproj_sb = cload("bproj_sb", bproj_d, (P, DC))
        b1_sb = cload("b1_sb", b1_d, (P, HC))
        b2_sb = cload("b2_sb", b2_d, (P, DC))
        nbg1_sb = cload("nbg1_sb", nbg1_d, (P, DC))
        nbg2_sb = cload("nbg2_sb", nbg2_d, (P, DC))

        padw = const.tile([P, 512], BF16)
        nc.vector.memset(padw, NEG)
        for s in range(2):
            for ic in range(TCC):
                off = 1024 + ic * P
                nc.scalar.dma_start(out=scr[s][ic, :, off:1536],
                                    in_=padw[:, 0:1536 - off])

        # shared psum pools (4 + 2 + 2 = 8 banks)
        psum = root.enter_context(tc.tile_pool(name="psum", bufs=4, space="PSUM"))
        psum_t = root.enter_context(tc.tile_pool(name="psum_t", bufs=2, space="PSUM"))
        psum_s = root.enter_context(tc.tile_pool(name="psum_s", bufs=2, space="PSUM"))

        def PS():
            return psum.tile([P, 512], F32, name="ps", tag="ps")

        def SM():
            return psum_s.tile([1, 512], F32, name="sm", tag="sm")

        # lifetime-managed activations (two-sided stack allocator:
        # frees must be LIFO per side, so lifetimes are laid out on
        # left/right stacks to nest properly)
        def mk(name, shape, dtype, side):
            t, fr = tc.tile(list(shape), dtype, name=name, side=side)
            return t, fr

        x1T, fr_x1T = mk("x1T", (P, DC, FULL), FP8, "left")

        # ================= Phase 1: LN1 (feature-major) =================
        # Host supplies xT8 = fp8(SA * x.T).  Stats come from fp8 DR ones-
        # matmuls (mean) and squared copies (var); normalize+affine applies
        # gamma/beta as per-partition scalars, writing x1T = SA*LN(x) in fp8.
        with ExitStack() as ph:
            xw = ph.enter_context(tc.tile_pool(name="xw", bufs=1, side="left"))
            lt = ph.enter_context(tc.tile_pool(name="lt", bufs=3, side="left"))
            xT8 = xw.tile([P, DC, FULL], FP8)
            nc.sync.dma_start(out=xT8, in_=xT8_d[:].rearrange("(kc p) t -> p kc t", p=P))
            sq8 = xw.tile([P, DC, FULL], FP8)
            for k in range(DC):
                for th in range(2):
                    sl = (slice(None), k, slice(th * 512, (th + 1) * 512))
                    if (2 * k + th) % 2 == 0:
                        # 8x^2 = Square(SA*x * sqrt(8)/SA) on the ACT LUT
                        nc.scalar.activation(out=sq8[sl], in_=xT8[sl],
                                             func=Act.Square,
                                             scale=float(np.sqrt(8.0) / SA))
                    else:
                        nc.vector.scalar_tensor_tensor(
                            out=sq8[sl], in0=xT8[sl], scalar=8.0 / (SA * SA),
                            in1=xT8[sl], op0=AluOp.mult, op1=AluOp.mult)
            for th in range(2):
                ts = slice(th * 512, (th + 1) * 512)
                s1 = SM()
                for j in range(DC // 2):
                    nc.tensor.matmul(s1, lhsT=ones8_t[:, 2 * j:2 * j + 2, 0:1],
                                     rhs=xT8[:, 2 * j:2 * j + 2, ts],
                                     start=(j == 0), stop=(j == DC // 2 - 1),
                                     perf_mode=DR)
                s2 = SM()
                for j in range(DC // 2):
                    nc.tensor.matmul(s2, lhsT=ones8_t[:, 2 * j:2 * j + 2, 0:1],
                                     rhs=sq8[:, 2 * j:2 * j + 2, ts],
                                     start=(j == 0), stop=(j == DC // 2 - 1),
                                     perf_mode=DR)
                m16 = lt.tile([1, 512], F32, name="m16")
                nc.vector.tensor_scalar_mul(m16, s1, 1.0 / D)      # SA*mean
                ex2 = lt.tile([1, 512], F32, name="ex2")
                nc.vector.tensor_scalar_mul(ex2, s2, 1.0 / (8 * D))  # E[x^2]
                msq = lt.tile([1, 512], F32, name="msq")
                nc.vector.scalar_tensor_tensor(out=msq, in0=m16,
                                               scalar=1.0 / (SA * SA), in1=m16,
                                               op0=AluOp.mult, op1=AluOp.mult)
                var = lt.tile([1, 512], F32, name="var")
                nc.vector.tensor_sub(var, ex2, msq)
                sd = lt.tile([1, 512], F32, name="sd1")
                nc.scalar.activation(out=sd, in_=var, func=Act.Sqrt,
                                     bias=eps_t[0:1, :])
                rstd = lt.tile([1, 512], F32, name="rstd1")
                nc.vector.reciprocal(out=rstd, in_=sd)
                meanB = lt.tile([P, 512], F32, name="meanB1")
                nc.gpsimd.partition_broadcast(meanB, m16)
                rstdB = lt.tile([P, 512], F32, name="rstdB1")
                nc.gpsimd.partition_broadcast(rstdB, rstd)
                for k in range(DC):
                    sl = (slice(None), k, ts)
                    t1 = lt.tile([P, 512], F32, name="t1a")
                    eng = nc.gpsimd if k % 2 == 0 else nc.vector
                    eng.tensor_sub(t1, xT8[sl], meanB)             # SA*(x-m)
                    t2 = lt.tile([P, 512], F32, name="t2a")
                    eng2 = nc.gpsimd if k % 2 == 1 else nc.vector
                    eng2.tensor_mul(t2, t1, rstdB)                 # SA*xhat
                    # x1T = SA*(xhat*g + b), alternating ACT/DVE
                    if k % 2 == 0:
                        nc.scalar.activation(out=x1T[sl], in_=t2,
                                             func=Act.Identity,
                                             scale=ln1g_sb[:, k:k + 1],
                                             bias=ln1b_sb[:, k:k + 1])
                    else:
                        nc.vector.tensor_scalar(out=x1T[sl], in0=t2,
                                                scalar1=ln1g_sb[:, k:k + 1],
                                                scalar2=ln1b_sb[:, k:k + 1],
                                                op0=AluOp.mult, op1=AluOp.add)

        # ================= Phase 2: KT, V, qT, rT =================
        kT, fr_kT = mk("kT", (P, DC, FULL), BF16, "right")
        v8, fr_v = mk("v8", (P, TCF, HEAD_NUM, 65), FP8, "right")
        # ones column (value SA) folds the softmax denominator into the AV mm
        nc.vector.memset(v8[:, :, :, 64:65], SA)
        rT, fr_rT = mk("rT", (P, DC, FULL), BF16, "right")
        quT, fr_quT = mk("quT", (P, DC, CUR), BF16, "right")
        qvT, fr_qvT = mk("qvT", (P, DC, CUR), BF16, "right")

        def dr4(ps, w, rhs, nsl, tsl):
            for j in range(DC // 2):
                nc.tensor.matmul(ps, lhsT=w[:, 2 * j:2 * j + 2, nsl],
                                 rhs=rhs[:, 2 * j:2 * j + 2, tsl],
                                 start=(j == 0), stop=(j == DC // 2 - 1),
                                 perf_mode=DR)

        with ExitStack() as ph:
            wkvp = ph.enter_context(tc.tile_pool(name="wkvp", bufs=1, side="right"))
            wkv = wkvp.tile([P, DC, 2 * D], FP8)
            nc.sync.dma_start(out=wkv, in_=wkv_d[:].rearrange("(kc p) n -> p kc n", p=P))
            for n in range(DC):
                for th in range(2):
                    ps = PS()
                    dr4(ps, wkv, x1T, slice(n * P, (n + 1) * P),
                        slice(th * 512, (th + 1) * 512))
                    # k bias is softmax-invariant; kT = psum/SWA (true scale)
                    nc.scalar.activation(out=kT[:, n, th * 512:(th + 1) * 512],
                                         in_=ps, func=Act.Copy, scale=1.0 / SWA)
            for t in range(TCF):
                for nh in range(2):
                    ps = PS()
                    for j in range(DC // 2):
                        nc.tensor.matmul(ps, lhsT=x1T[:, 2 * j:2 * j + 2, t * P:(t + 1) * P],
                                         rhs=wkv[:, 2 * j:2 * j + 2, D + nh * 512:D + (nh + 1) * 512],
                                         start=(j == 0), stop=(j == DC // 2 - 1),
                                         perf_mode=DR)
                    # v8[tok, head, 0:64] = SA * v; col 64 of each slot is SA
                    nc.scalar.activation(
                        out=v8[:, t, nh * 8:(nh + 1) * 8, 0:64],
                        in_=ps[:, :].rearrange("p (h d) -> p h d", d=64),
                        func=Act.Copy, scale=SA / SWA)
        with ExitStack() as ph:
            wqp = ph.enter_context(tc.tile_pool(name="wqp", bufs=1, side="right"))
            wq = wqp.tile([P, DC, D], FP8)
            nc.sync.dma_start(out=wq, in_=wq_d[:].rearrange("(kc p) n -> p kc n", p=P))
            for n in range(DC):
                ps = PS()
                dr4(ps, wq, x1T, slice(n * P, (n + 1) * P), slice(CUR, FULL))
                # u_sb/v_sb hold u+bq / v+bq (host-folded)
                nc.vector.tensor_scalar(out=quT[:, n, :], in0=ps,
                                        scalar1=1.0 / SWA, scalar2=u_sb[:, n:n + 1],
                                        op0=AluOp.mult, op1=AluOp.add)
                nc.vector.tensor_scalar(out=qvT[:, n, :], in0=ps,
                                        scalar1=1.0 / SWA, scalar2=v_sb[:, n:n + 1],
                                        op0=AluOp.mult, op1=AluOp.add)
        with ExitStack() as ph:
            wpp = ph.enter_context(tc.tile_pool(name="wpp", bufs=1, side="right"))
            wpos = wpp.tile([P, DC, D], FP8)
            nc.sync.dma_start(out=wpos, in_=wpos_d[:].rearrange("(kc p) n -> p kc n", p=P))
            posT_sb = wpp.tile([P, DC, FULL], FP8)
            nc.sync.dma_start(out=posT_sb, in_=posT8_d[:].rearrange("(kc p) f -> p kc f", p=P))
            for n in range(DC):
                for fh in range(2):
                    ps = PS()
                    dr4(ps, wpos, posT_sb, slice(n * P, (n + 1) * P),
                        slice(fh * 512, (fh + 1) * 512))
                    # pos bias is softmax-invariant after rel-shift; drop it
                    nc.scalar.activation(out=rT[:, n, fh * 512:(fh + 1) * 512],
                                         in_=ps, func=Act.Copy, scale=1.0 / SWA)
        fr_x1T()

        # ================= Phase 3: attention =================
        # Scores are computed query-major (trimmed to the causal triangle),
        # summed in bf16, PE-transposed into a psum bank per key chunk, and
        # exp-evacuated by ACT straight into fp8 attnT.  The AV matmul runs
        # fp8 DoubleRow against v8 whose 65th column (=SA) yields the softmax
        # denominator as psum row 64 for free.
        avT, fr_avT = mk("avT", (P, DC, CUR), FP8, "left")
        with ExitStack() as ph:
            aw = ph.enter_context(tc.tile_pool(name="aw", bufs=3, side="left"))
            rw = ph.enter_context(tc.tile_pool(name="rw", bufs=2, side="left"))

            def head_slices(h):
                ch, rb = h // 2, (h % 2) * HEAD_DIM
                return (quT[rb:rb + HEAD_DIM, ch, :], qvT[rb:rb + HEAD_DIM, ch, :],
                        kT[rb:rb + HEAD_DIM, ch, :], rT[rb:rb + HEAD_DIM, ch, :])

            def pos_stage(h):
                """Pos scores -> scratch -> single combined shifted read.
                Write at col r+128*ic so the fixed read offset 511 yields the
                reference shift P[i, 511+j-i] with global i."""
                _, qvh, _, rh = head_slices(h)
                s_t = scr[h % 2]
                for ic in range(TCC):
                    w_ic = (ic + 5) * P
                    r_lo = FULL - w_ic
                    pb = aw.tile([P, FULL], BF16, name="pb", bufs=4)
                    for r0, r1 in ((r_lo, min(r_lo + 512, FULL)),
                                   (min(r_lo + 512, FULL), FULL)):
                        if r1 <= r0:
                            continue
                        pp = PS()
                        nc.tensor.matmul(pp[:, 0:r1 - r0],
                                         lhsT=qvh[:, ic * P:(ic + 1) * P],
                                         rhs=rh[:, r0:r1], start=True, stop=True)
                        nc.vector.tensor_copy(pb[:, r0:r1], pp[:, 0:r1 - r0])
                    eng = nc.gpsimd if ic % 2 == 0 else nc.sync
                    eng.dma_start(out=s_t[ic, :, 384:1024 + ic * P],
                                  in_=pb[:, r_lo:1024])
                shp = aw.tile([P, TCC, FULL], BF16, name="shp", bufs=2)
                shift_ap = bass.AP(tensor=s_t.tensor, offset=s_t.offset + 511,
                                   ap=[[1535, P], [P * 1536, TCC], [1, FULL]])
                nc.gpsimd.dma_start(out=shp, in_=shift_ap)
                return shp

            def rest_stage(h, shp):
                quh, _, kh, _ = head_slices(h)
                attnT = aw.tile([P, TCF, 512], FP8, name="attnT", bufs=2)
                sms = []
                for ic in range(TCC):
                    w_ic = (ic + 5) * P
                    sm = aw.tile([P, FULL], BF16, name="sm", bufs=8)
                    for j0, j1 in ((0, 512), (512, w_ic)):
                        if j1 <= j0:
                            continue
                        cp = PS()
                        if h % 2 == 0:
                            # fold pos into psum via identity matmul; ACT
                            # evacuates (balances DVE)
                            nc.tensor.matmul(cp[:, 0:j1 - j0],
                                             lhsT=quh[:, ic * P:(ic + 1) * P],
                                             rhs=kh[:, j0:j1], start=True,
                                             stop=False)
                            nc.tensor.matmul(cp[:, 0:j1 - j0], lhsT=ident_b,
                                             rhs=shp[:, ic, j0:j1],
                                             start=False, stop=True)
                            nc.scalar.copy(sm[:, j0:j1], cp[:, 0:j1 - j0])
                        else:
                            nc.tensor.matmul(cp[:, 0:j1 - j0],
                                             lhsT=quh[:, ic * P:(ic + 1) * P],
                                             rhs=kh[:, j0:j1], start=True,
                                             stop=True)
                            nc.vector.tensor_add(sm[:, j0:j1], cp[:, 0:j1 - j0],
                                                 shp[:, ic, j0:j1])
                    sms.append(sm)
                for jc in range(TCF):
                    ic_lo = max(0, jc - 4)
                    tp = psum_t.tile([P, 512], BF16, name="ptb", tag="pt")
                    for ic in range(ic_lo, TCC):
                        nc.tensor.matmul(tp[:, ic * P:(ic + 1) * P],
                                         lhsT=sms[ic][:, jc * P:(jc + 1) * P],
                                         rhs=ident_b, is_transpose=True,
                                         start=(ic == ic_lo), stop=(ic == TCC - 1))
                    nc.scalar.activation(out=attnT[:, jc, ic_lo * P:512],
                                         in_=tp[:, ic_lo * P:512],
                                         func=Act.Exp, scale=SCALE)
                # AV + denominator: rows 0..63 = SA*unnorm, row 64 = SA*den
                av = PS()
                mms = []
                for ic in range(TCC):
                    njc = ic + 5
                    qs = slice(ic * P, (ic + 1) * P)
                    for j in range(0, njc - 1, 2):
                        mms.append((qs, j, True))
                    if njc % 2 == 1:
                        mms.append((qs, njc - 1, False))
                for i, (qs, j, is_dr) in enumerate(mms):
                    fl = dict(start=(i == 0), stop=(i == len(mms) - 1))
                    if is_dr:
                        nc.tensor.matmul(av[0:65, qs], lhsT=v8[:, j:j + 2, h, :],
                                         rhs=attnT[:, j:j + 2, qs],
                                         perf_mode=DR, **fl)
                    else:
                        nc.tensor.matmul(av[0:65, qs], lhsT=v8[:, j, h, :],
                                         rhs=attnT[:, j, qs], **fl)
                ch, rb = h // 2, (h % 2) * HEAD_DIM
                recip = rw.tile([1, 512], F32, name="recip")
                nc.vector.reciprocal(out=recip, in_=av[64:65, :])
                recipB = rw.tile([HEAD_DIM, 512], F32, name="recipB")
                nc.gpsimd.partition_broadcast(recipB, recip)
                # avT8 = SA * av_norm  (SA psum scales cancel in the ratio)
                nc.vector.scalar_tensor_tensor(out=avT[rb:rb + HEAD_DIM, ch, :],
                                               in0=av[0:HEAD_DIM, :], scalar=SA,
                                               in1=recipB, op0=AluOp.mult,
                                               op1=AluOp.mult)

            # software pipeline: pos DMA round-trip of head h+1 overlaps the
            # score/transpose/av work of head h
            pend = pos_stage(0)
            for h in range(HEAD_NUM):
                nxt = pos_stage(h + 1) if h + 1 < HEAD_NUM else None
                rest_stage(h, pend)
                pend = nxt
        fr_qvT(); fr_quT(); fr_rT(); fr_v(); fr_kT()

        # ================= Phase 4: proj + GRU1 =================
        a1T, fr_a1T = mk("a1T", (P, DC, CUR), FP8, "right")
        with ExitStack() as ph:
            wpr = ph.enter_context(tc.tile_pool(name="wpr", bufs=1, side="left"))
            wproj = wpr.tile([P, DC, D], FP8)
            nc.sync.dma_start(out=wproj, in_=wproj_d[:].rearrange("(kc p) n -> p kc n", p=P))
            for n in range(DC):
                ps = PS()
                for j in range(DC // 2):
                    nc.tensor.matmul(ps, lhsT=wproj[:, 2 * j:2 * j + 2, n * P:(n + 1) * P],
                                     rhs=avT[:, 2 * j:2 * j + 2, :],
                                     start=(j == 0), stop=(j == DC // 2 - 1),
                                     perf_mode=DR)
                # a1T8 = SA * relu(pre + bproj); bproj_sb holds SA*bproj
                nc.scalar.activation(out=a1T[:, n, :], in_=ps, func=Act.Relu,
                                     bias=bproj_sb[:, n:n + 1], scale=SA / SWA)
        fr_avT()

        o1T_f, fr_o1f = mk("o1T_f", (P, DC, CUR), F32, "left")
        o1T_8, fr_o1b = mk("o1T_8", (P, DC, CUR), FP8, "left")
        inpT_f, fr_inpf = mk("inpT_f", (P, DC, CUR), F32, "left")
        inpT_8, fr_inpb = mk("inpT_8", (P, DC, CUR), FP8, "left")
        nc.sync.dma_start(out=inpT_f, in_=inpT_d[:].rearrange("(kc p) t -> p kc t", p=P))
        nc.sync.dma_start(out=inpT_8, in_=inpT8_d[:].rearrange("(kc p) t -> p kc t", p=P))
        with ExitStack() as ph:
            _gru(nc, tc, ph, PS, gw_d, 1, a1T, inpT_8, inpT_f, nbg1_sb,
                 o1T_f, o1T_8)
        fr_inpb(); fr_inpf(); fr_a1T()

        # ================= Phase 5: LN2 =================
        x2T, fr_x2T = mk("x2T", (P, DC, CUR), FP8, "right")
        with ExitStack() as ph:
            lw = ph.enter_context(tc.tile_pool(name="lw", bufs=2, side="left"))
            sqp = ph.enter_context(tc.tile_pool(name="sqp", bufs=1, side="left"))
            sq = sqp.tile([P, DC, 512], FP8, name="sq")
            for n in range(DC):
                # 4*o1^2 fits fp8 range comfortably
                nc.vector.scalar_tensor_tensor(out=sq[:, n, :], in0=o1T_f[:, n, :],
                                               scalar=4.0, in1=o1T_f[:, n, :],
                                               op0=AluOp.mult, op1=AluOp.mult)
            s1 = SM()
            for j in range(DC // 2):
                nc.tensor.matmul(s1, lhsT=ones8_t[:, 2 * j:2 * j + 2, 0:1],
                                 rhs=o1T_8[:, 2 * j:2 * j + 2, :],
                                 start=(j == 0), stop=(j == DC // 2 - 1),
                                 perf_mode=DR)
            mean = lw.tile([1, 512], F32, name="mean")
            nc.vector.tensor_scalar_mul(mean, s1, 1.0 / (SA * D))
            s2 = SM()
            for j in range(DC // 2):
                nc.tensor.matmul(s2, lhsT=ones8_t[:, 2 * j:2 * j + 2, 0:1],
                                 rhs=sq[:, 2 * j:2 * j + 2, :],
                                 start=(j == 0), stop=(j == DC // 2 - 1),
                                 perf_mode=DR)
            m2m = lw.tile([1, 512], F32, name="m2m")
            nc.vector.tensor_scalar_mul(m2m, s2, 1.0 / (4 * D))
            var = lw.tile([1, 512], F32, name="var")
            nc.vector.scalar_tensor_tensor(out=var, in0=mean, scalar=1.0,
                                           in1=mean, op0=AluOp.mult,
                                           op1=AluOp.mult)
            nc.vector.tensor_sub(var, m2m, var)
            sd = lw.tile([1, 512], F32, name="sd2")
            nc.scalar.activation(out=sd, in_=var, func=Act.Sqrt,
                                 bias=eps_t[0:1, :])
            rstd = lw.tile([1, 512], F32, name="rstd2")
            nc.vector.reciprocal(out=rstd, in_=sd)
            meanB = lw.tile([P, 512], F32, name="meanB")
            nc.gpsimd.partition_broadcast(meanB, mean)
            rstdB = lw.tile([P, 512], F32, name="rstdB")
            nc.gpsimd.partition_broadcast(rstdB, rstd)
            for n in range(DC):
                t1 = lw.tile([P, 512], F32, name="t1")
                nc.vector.tensor_sub(t1, o1T_f[:, n, :], meanB)
                nc.vector.tensor_mul(t1, t1, rstdB)
                nc.vector.tensor_scalar(out=x2T[:, n, :], in0=t1,
                                        scalar1=ln2g_sb[:, n:n + 1],
                                        scalar2=ln2b_sb[:, n:n + 1],
                                        op0=AluOp.mult, op1=AluOp.add)

        # ================= Phase 6: MLP (fp8 DoubleRow) =================
        with ExitStack() as ph6:
            m1w = ph6.enter_context(tc.tile_pool(name="m1w", bufs=1, side="right"))
            m1T = m1w.tile([P, HC, 512], FP8)
            with ExitStack() as ph:
                w1p = ph.enter_context(tc.tile_pool(name="w1p", bufs=4, side="right"))
                w1r = w1_d[:].rearrange("(kc p) n -> p kc n", p=P)
                for n in range(HC):
                    w1t = w1p.tile([P, DC, P], FP8, name="w1t", tag="w1t")
                    [nc.sync, nc.scalar, nc.gpsimd][n % 3].dma_start(
                        out=w1t, in_=w1r[:, :, n * P:(n + 1) * P])
                    ps = PS()
                    for j in range(DC // 2):
                        nc.tensor.matmul(ps, lhsT=w1t[:, 2 * j:2 * j + 2, :],
                                         rhs=x2T[:, 2 * j:2 * j + 2, :],
                                         start=(j == 0), stop=(j == DC // 2 - 1),
                                         perf_mode=DR)
                    # m1T8 = SA*relu(pre + b1); psum = SWA*pre; b1_sb = SA*b1
                    if _ZERO_BIAS[0] and n % 2 == 1:
                        nc.vector.tensor_scalar(out=m1T[:, n, :], in0=ps,
                                                scalar1=SA / SWA, scalar2=0.0,
                                                op0=AluOp.mult, op1=AluOp.max)
                    else:
                        nc.scalar.activation(out=m1T[:, n, :], in_=ps,
                                             func=Act.Relu,
                                             bias=b1_sb[:, n:n + 1],
                                             scale=SA / SWA)
            m2T, fr_m2T = mk("m2T", (P, DC, CUR), FP8, "left")
            w2p = ph6.enter_context(tc.tile_pool(name="w2p", bufs=3, side="left"))
            w2r = w2_d[:].rearrange("(kc p) n -> p kc n", p=P)
            for n in range(DC):
                w2t = w2p.tile([P, HC, P], FP8, name="w2t", tag="w2t")
                [nc.sync, nc.scalar, nc.gpsimd][n % 3].dma_start(
                    out=w2t, in_=w2r[:, :, n * P:(n + 1) * P])
                ps = PS()
                for j in range(HC // 2):
                    nc.tensor.matmul(ps, lhsT=w2t[:, 2 * j:2 * j + 2, :],
                                     rhs=m1T[:, 2 * j:2 * j + 2, :],
                                     start=(j == 0), stop=(j == HC // 2 - 1),
                                     perf_mode=DR)
                if _ZERO_BIAS[0] and n % 2 == 1:
                    nc.vector.tensor_scalar(out=m2T[:, n, :], in0=ps,
                                            scalar1=SA / SWA, scalar2=0.0,
                                            op0=AluOp.mult, op1=AluOp.max)
                else:
                    nc.scalar.activation(out=m2T[:, n, :], in_=ps, func=Act.Relu,
                                         bias=b2_sb[:, n:n + 1], scale=SA / SWA)
        fr_x2T()

        # ================= Phase 7: GRU2 =================
        o2T_f, fr_o2 = mk("o2T_f", (P, DC, CUR), F32, "right")
        with ExitStack() as ph:
            _gru(nc, tc, ph, PS, gw_d, 2, m2T, o1T_8, o1T_f, nbg2_sb,
                 o2T_f, None)
        fr_m2T(); fr_o1b(); fr_o1f()

        # ================= Phase 8: DMA transposed output =================
        # out_d holds o2 feature-major [D, CUR]; host transposes for free.
        # per-chunk DMAs overlap the GRU2 tail instead of one serial write.
        out_r = out_d[:].rearrange("(kc p) t -> p kc t", p=P)
        for n in range(DC):
            [nc.sync, nc.scalar, nc.gpsimd][n % 3].dma_start(
                out=out_r[:, n, :], in_=o2T_f[:, n, :])
        fr_o2()


def _gru(nc, tc, ph, PS, gw_d, g, yT, xT_8, xT_f, nbg_sb, oT_f, oT_8):
    """fp8 DoubleRow GRU gate. yT/xT_8 hold SA*value in fp8; weights SW*W.
    PSUM accumulates SWA*pre_act; Act applies 1/SWA before the nonlinearity."""
    gwp = ph.enter_context(tc.tile_pool(name=f"gw{g}", bufs=4, side="left"))
    gtmp = ph.enter_context(tc.tile_pool(name=f"gt{g}", bufs=2, side="left"))
    gper = ph.enter_context(tc.tile_pool(name=f"gp{g}", bufs=1, side="left"))

    # spread weight-stream transfer time across issue queues
    qeng = [nc.sync, nc.scalar, nc.gpsimd]
    qi = [0]

    def loadw(m):
        w = gwp.tile([P, DC, D], FP8, name=f"gwt_{m}", tag="gwt")
        qeng[qi[0] % 3].dma_start(
            out=w, in_=gw_d[(g, m)][:].rearrange("(kc p) n -> p kc n", p=P))
        qi[0] += 1
        return w

    def dr_chain(ps, w, u, rhs_w, rhs_u):
        for j in range(DC // 2):
            nc.tensor.matmul(ps, lhsT=w[:, 2 * j:2 * j + 2, n * P:(n + 1) * P],
                             rhs=rhs_w[:, 2 * j:2 * j + 2, :],
                             start=(j == 0), stop=False, perf_mode=DR)
        for j in range(DC // 2):
            nc.tensor.matmul(ps, lhsT=u[:, 2 * j:2 * j + 2, n * P:(n + 1) * P],
                             rhs=rhs_u[:, 2 * j:2 * j + 2, :],
                             start=False, stop=(j == DC // 2 - 1), perf_mode=DR)

    wr, ur = loadw("Wr"), loadw("Ur")
    rx = gper.tile([P, DC, 512], FP8, name="rx")
    for n in range(DC):
        ps = PS()
        dr_chain(ps, wr, ur, yT, xT_8)
        rr = gtmp.tile([P, 512], F32, name="rr")
        nc.scalar.activation(out=rr, in_=ps, func=Act.Sigmoid, scale=1.0 / SWA)
        # rx8 = SA * r * x
        nc.vector.scalar_tensor_tensor(out=rx[:, n, :], in0=rr, scalar=SA,
                                       in1=xT_f[:, n, :], op0=AluOp.mult,
                                       op1=AluOp.mult)
    wz, uz = loadw("Wz"), loadw("Uz")
    zt = gper.tile([P, DC, 512], F32, name="zt")
    for n in range(DC):
        ps = PS()
        dr_chain(ps, wz, uz, yT, xT_8)
        nc.scalar.activation(out=zt[:, n, :], in_=ps, func=Act.Sigmoid,
                             bias=nbg_sb[:, n:n + 1], scale=1.0 / SWA)
    wg, ug = loadw("Wg"), loadw("Ug")
    for n in range(DC):
        ps = PS()
        dr_chain(ps, wg, ug, yT, rx)
        ht = gtmp.tile([P, 512], F32, name="ht")
        nc.scalar.activation(out=ht, in_=ps, func=Act.Tanh, scale=1.0 / SWA)
        nc.gpsimd.tensor_sub(ht, ht, xT_f[:, n, :])
        nc.vector.tensor_mul(ht, ht, zt[:, n, :])
        nc.gpsimd.tensor_add(oT_f[:, n, :], ht, xT_f[:, n, :])
        if oT_8 is not None:
            nc.vector.tensor_scalar_mul(oT_8[:, n, :], oT_f[:, n, :], SA)


_NC_CACHE = {}


def _get_nc():
    if "nc" not in _NC_CACHE:
        _NC_CACHE["nc"] = _build()
    return _NC_CACHE["nc"]


def _chunk_t(vec):
    n = vec.shape[0] // P
    return np.ascontiguousarray(vec.reshape(n, P).T.astype(np.float32))


def _prep(inputs):
    f32 = np.float32
    bf = ml_dtypes.bfloat16
    fp8 = ml_dtypes.float8_e4m3
    inp = np.asarray(inputs["inputs"], f32)
    mem = np.asarray(inputs["memory"], f32)
    pos = np.asarray(inputs["pos_embedding"], f32)[:, 0, :]
    sw, sa = np.float32(SW), np.float32(SA)

    bq = np.asarray(inputs["bq"], f32)
    bkvV = np.asarray(inputs["bkv"], f32)[D:2 * D]
    wproj_f = np.asarray(inputs["Wproj"], f32)
    # v-bias shifts normalized av by a constant vector -> folds into bproj
    bproj_eff = np.asarray(inputs["bproj"], f32) + bkvV @ wproj_f
    shared = {
        "posT8": (sa * pos.T).astype(fp8),
        # bq folds into u and v (k/pos biases are softmax-invariant, dropped)
        "u_t": _chunk_t(np.asarray(inputs["u"], f32).reshape(-1) + bq),
        "v_t": _chunk_t(np.asarray(inputs["v"], f32).reshape(-1) + bq),
        "ln1_g_t": _chunk_t(np.asarray(inputs["ln1_g"], f32)),
        "ln1_b16_t": _chunk_t(sa * np.asarray(inputs["ln1_b"], f32)),
        "ln2_g_t": _chunk_t(sa * np.asarray(inputs["ln2_g"], f32)),
        "ln2_b_t": _chunk_t(sa * np.asarray(inputs["ln2_b"], f32)),
        "bproj_t": _chunk_t(sa * bproj_eff),
        "b1_t": _chunk_t(sa * np.asarray(inputs["mlp_b1"], f32)),
        "b2_t": _chunk_t(sa * np.asarray(inputs["mlp_b2"], f32)),
        "nbg1_t": _chunk_t(-np.asarray(inputs["g1_bg"], f32)),
        "nbg2_t": _chunk_t(-np.asarray(inputs["g2_bg"], f32)),
        "Wkv8": (sw * np.asarray(inputs["Wkv"], f32)).astype(fp8),
        "Wq8": (sw * np.asarray(inputs["Wq"], f32)).astype(fp8),
        "Wpos8": (sw * np.asarray(inputs["Wpos"], f32)).astype(fp8),
        "Wproj8": (sw * wproj_f).astype(fp8),
        "mlp_W18": (sw * np.asarray(inputs["mlp_W1"], f32)).astype(fp8),
        "mlp_W28": (sw * np.asarray(inputs["mlp_W2"], f32)).astype(fp8),
    }
    for g in (1, 2):
        for m in ("Wr", "Ur", "Wz", "Uz", "Wg", "Ug"):
            shared[f"g{g}_{m}8"] = (
                sw * np.asarray(inputs[f"g{g}_{m}"], f32)).astype(fp8)

    in_maps = []
    for b in range(BS):
        im = dict(shared)
        x_full = np.concatenate([mem[:, b, :], inp[:, b, :]], axis=0)
        im["xT8"] = (sa * x_full.T).astype(fp8)
        inpT = np.ascontiguousarray(inp[:, b, :].T)
        im["inpT"] = inpT
        im["inpT8"] = (sa * inpT).astype(fp8)
        in_maps.append(im)
    return in_maps


def kernel(**inputs):
    _ZERO_BIAS[0] = all(
        not np.any(np.asarray(inputs[k]))
        for k in ("mlp_b1", "mlp_b2", "bproj"))
    nc = _get_nc()
    in_maps = _prep(inputs)
    res = run_bass_kernel_spmd(nc, in_maps, core_ids=list(range(BS)))
    # device returns feature-major [D, CUR]; transpose back on host
    out = np.stack([res.results[b]["out"].T for b in range(BS)], axis=1)
    return np.ascontiguousarray(out.astype(np.float32))


if __name__ == "__main__":
    _get_nc()
    print("build+compile OK")



# revision 76
# speedup vs baseline: 1.0798x; 1.0364x over previous
"""GTrXL layer (TransformerXL attention + GRU gating) on 8 TRN2 NeuronCores.

Sharding: pure data-parallel over batch (BS=8 -> 1 batch element per core).
No collectives. Per-core Bass/Tile kernel computes the full layer for its
batch element.

Layout convention on-chip: activations are kept TRANSPOSED [feature, token]
(feature on partitions, 128-chunks) so that weight matrices in natural [K, N]
layout serve as the stationary matmul operand and matmul outputs land
transposed again:  outT[n, t] = sum_k W[k, n] * xT[k, t].

Matmul compute in bf16 (f32 accumulate in PSUM); LN/softmax/GRU elementwise
math in f32.

Relative-shift: pos scores P[i, relk] are written per 128-query-chunk to a
DRAM scratch of row stride 1536 whose tail 512 columns are pre-filled with
-1e30; the shifted read  shifted[i, j] = P[i, 511 + j - i]  is a single
strided DMA (row step 1535), and the pad lands exactly on the masked region
j > i + 512, so masking comes for free.
"""

import sys

if '/opt/trn_rl_repo' not in sys.path:
    sys.path.insert(0, '/opt/trn_rl_repo')

import numpy as np
import ml_dtypes

import concourse.bass as bass
import concourse.tile as tile
from concourse import bacc, mybir
from concourse.bass_utils import run_bass_kernel_spmd
from concourse.masks import make_identity

BF16 = mybir.dt.bfloat16
F32 = mybir.dt.float32
FP8 = mybir.dt.float8e4
DR = mybir.MatmulPerfMode.DoubleRow
SW = 32.0      # fp8 weight scale
SA = 16.0      # fp8 activation scale
SWA = SW * SA  # psum scale for fp8 DoubleRow chains

HEAD_NUM, HEAD_DIM = 16, 64
D, HID = 1024, 4096
CUR, PREV, BS = 512, 512, 8
FULL = CUR + PREV
EPS = 1e-5
SCALE = 1.0 / (HEAD_DIM ** 0.5)
P = 128
DC = D // P          # 8 feature chunks
HC = HID // P        # 32 hidden chunks
TCF = FULL // P      # 8 full-token chunks
TCC = CUR // P       # 4 query-token chunks
NEG = -1.0e30

AluOp = mybir.AluOpType
Act = mybir.ActivationFunctionType

# set from the actual inputs in kernel(); enables 2-op DVE relu evacuations
_ZERO_BIAS = [False]


def _dram_in(dram, name, shape, dtype):
    return dram.tile(list(shape), dtype, kind="ExternalInput", name=name,
                     uniquify=False)


def _mm_chain(nc, psum, lhsT_tiles, rhs_tiles):
    n = len(lhsT_tiles)
    for i in range(n):
        nc.tensor.matmul(psum, lhsT=lhsT_tiles[i], rhs=rhs_tiles[i],
                         start=(i == 0), stop=(i == n - 1))


def _build():
    nc = bacc.Bacc("TRN2", target_bir_lowering=False)
    with tile.TileContext(nc) as tc:
        _emit(nc, tc)
    nc.compile()
    return nc


def _emit(nc, tc):
    from contextlib import ExitStack

    with ExitStack() as root:
        dram = root.enter_context(tc.tile_pool(name="io", bufs=1, space="DRAM"))

        # ---------------- DRAM I/O ----------------
        xT8_d = _dram_in(dram, "xT8", (D, FULL), FP8)
        inpT_d = _dram_in(dram, "inpT", (D, CUR), F32)
        inpT8_d = _dram_in(dram, "inpT8", (D, CUR), FP8)
        posT8_d = _dram_in(dram, "posT8", (D, FULL), FP8)
        u_d = _dram_in(dram, "u_t", (P, DC), F32)
        v_d = _dram_in(dram, "v_t", (P, DC), F32)
        ln1g_d = _dram_in(dram, "ln1_g_t", (P, DC), F32)
        ln1b_d = _dram_in(dram, "ln1_b16_t", (P, DC), F32)
        ln2g_d = _dram_in(dram, "ln2_g_t", (P, DC), F32)
        ln2b_d = _dram_in(dram, "ln2_b_t", (P, DC), F32)
        bproj_d = _dram_in(dram, "bproj_t", (P, DC), F32)
        b1_d = _dram_in(dram, "b1_t", (P, HC), F32)
        b2_d = _dram_in(dram, "b2_t", (P, DC), F32)
        nbg1_d = _dram_in(dram, "nbg1_t", (P, DC), F32)
        nbg2_d = _dram_in(dram, "nbg2_t", (P, DC), F32)

        wkv_d = _dram_in(dram, "Wkv8", (D, 2 * D), FP8)
        wq_d = _dram_in(dram, "Wq8", (D, D), FP8)
        wpos_d = _dram_in(dram, "Wpos8", (D, D), FP8)
        wproj_d = _dram_in(dram, "Wproj8", (D, D), FP8)
        gw_d = {}
        for g in (1, 2):
            for m in ("Wr", "Ur", "Wz", "Uz", "Wg", "Ug"):
                gw_d[(g, m)] = _dram_in(dram, f"g{g}_{m}8", (D, D), FP8)
        w1_d = _dram_in(dram, "mlp_W18", (D, HID), FP8)
        w2_d = _dram_in(dram, "mlp_W28", (HID, D), FP8)

        out_d = dram.tile([D, CUR], F32, kind="ExternalOutput", name="out",
                          uniquify=False)

        # one [4ic, P, 1536] scratch per head-parity; fixed ic slots keep the
        # NEG pad region [1024+128*ic, 1536) stable across reuses
        scr = [dram.tile([TCC, P, 1536], BF16, name=f"scr{s}") for s in range(2)]

        # ---------------- constants ----------------
        const = root.enter_context(tc.tile_pool(name="const", bufs=1))
        ident_b = const.tile([P, P], BF16)
        make_identity(nc, ident_b)
        # pair-dim stride must be a multiple of 16B for DoubleRow ldweights
        ones8_t = const.tile([P, DC, 16], FP8)
        nc.vector.memset(ones8_t, 1.0)
        eps_t = const.tile([P, 1], F32)
        nc.vector.memset(eps_t, EPS)

        def cload(name, dref, shape, dtype=F32):
            t = const.tile(list(shape), dtype, name=name)
            nc.sync.dma_start(out=t, in_=dref[:])
            return t

        u_sb = cload("u_sb", u_d, (P, DC))
        v_sb = cload("v_sb", v_d, (P, DC))
        ln1g_sb = cload("ln1g_sb", ln1g_d, (P, DC))
        ln1b_sb = cload("ln1b_sb", ln1b_d, (P, DC))
        ln2g_sb = cload("ln2g_sb", ln2g_d, (P, DC))
        ln2b_sb = cload("ln2b_sb", ln2b_d, (P, DC))
        bproj_sb = cload("bproj_sb", bproj_d, (P, DC))
        b1_sb = cload("b1_sb", b1_d, (P, HC))
        b2_sb = cload("b2_sb", b2_d, (P, DC))
        nbg1_sb = cload("nbg1_sb", nbg1_d, (P, DC))
        nbg2_sb = cload("nbg2_sb", nbg2_d, (P, DC))

        padw = const.tile([P, 512], BF16)
        nc.vector.memset(padw, NEG)
        for s in range(2):
            for ic in range(TCC):
                off = 1024 + ic * P
                nc.scalar.dma_start(out=scr[s][ic, :, off:1536],
                                    in_=padw[:, 0:1536 - off])

        # shared psum pools (4 + 2 + 2 = 8 banks)
        psum = root.enter_context(tc.tile_pool(name="psum", bufs=4, space="PSUM"))
        psum_t = root.enter_context(tc.tile_pool(name="psum_t", bufs=2, space="PSUM"))
        psum_s = root.enter_context(tc.tile_pool(name="psum_s", bufs=2, space="PSUM"))

        def PS():
            return psum.tile([P, 512], F32, name="ps", tag="ps")

        def SM():
            return psum_s.tile([1, 512], F32, name="sm", tag="sm")

        # lifetime-managed activations (two-sided stack allocator:
        # frees must be LIFO per side, so lifetimes are laid out on
        # left/right stacks to nest properly)
        def mk(name, shape, dtype, side):
            t, fr = tc.tile(list(shape), dtype, name=name, side=side)
            return t, fr

        x1T, fr_x1T = mk("x1T", (P, DC, FULL), FP8, "left")

        # ================= Phase 1: LN1 (feature-major) =================
        # Host supplies xT8 = fp8(SA * x.T).  Stats come from fp8 DR ones-
        # matmuls (mean) and squared copies (var); normalize+affine applies
        # gamma/beta as per-partition scalars, writing x1T = SA*LN(x) in fp8.
        with ExitStack() as ph:
            xw = ph.enter_context(tc.tile_pool(name="xw", bufs=1, side="left"))
            lt = ph.enter_context(tc.tile_pool(name="lt", bufs=3, side="left"))
            xT8 = xw.tile([P, DC, FULL], FP8)
            nc.sync.dma_start(out=xT8, in_=xT8_d[:].rearrange("(kc p) t -> p kc t", p=P))
            sq8 = xw.tile([P, DC, FULL], FP8)
            for k in range(DC):
                for th in range(2):
                    sl = (slice(None), k, slice(th * 512, (th + 1) * 512))
                    if (2 * k + th) % 2 == 0:
                        # 8x^2 = Square(SA*x * sqrt(8)/SA) on the ACT LUT
                        nc.scalar.activation(out=sq8[sl], in_=xT8[sl],
                                             func=Act.Square,
                                             scale=float(np.sqrt(8.0) / SA))
                    else:
                        nc.vector.scalar_tensor_tensor(
                            out=sq8[sl], in0=xT8[sl], scalar=8.0 / (SA * SA),
                            in1=xT8[sl], op0=AluOp.mult, op1=AluOp.mult)
            for th in range(2):
                ts = slice(th * 512, (th + 1) * 512)
                s1 = SM()
                for j in range(DC // 2):
                    nc.tensor.matmul(s1, lhsT=ones8_t[:, 2 * j:2 * j + 2, 0:1],
                                     rhs=xT8[:, 2 * j:2 * j + 2, ts],
                                     start=(j == 0), stop=(j == DC // 2 - 1),
                                     perf_mode=DR)
                s2 = SM()
                for j in range(DC // 2):
                    nc.tensor.matmul(s2, lhsT=ones8_t[:, 2 * j:2 * j + 2, 0:1],
                                     rhs=sq8[:, 2 * j:2 * j + 2, ts],
                                     start=(j == 0), stop=(j == DC // 2 - 1),
                                     perf_mode=DR)
                m16 = lt.tile([1, 512], F32, name="m16")
                nc.vector.tensor_scalar_mul(m16, s1, 1.0 / D)      # SA*mean
                ex2 = lt.tile([1, 512], F32, name="ex2")
                nc.vector.tensor_scalar_mul(ex2, s2, 1.0 / (8 * D))  # E[x^2]
                msq = lt.tile([1, 512], F32, name="msq")
                nc.vector.scalar_tensor_tensor(out=msq, in0=m16,
                                               scalar=1.0 / (SA * SA), in1=m16,
                                               op0=AluOp.mult, op1=AluOp.mult)
                var = lt.tile([1, 512], F32, name="var")
                nc.vector.tensor_sub(var, ex2, msq)
                sd = lt.tile([1, 512], F32, name="sd1")
                nc.scalar.activation(out=sd, in_=var, func=Act.Sqrt,
                                     bias=eps_t[0:1, :])
                rstd = lt.tile([1, 512], F32, name="rstd1")
                nc.vector.reciprocal(out=rstd, in_=sd)
                meanB = lt.tile([P, 512], F32, name="meanB1")
                nc.gpsimd.partition_broadcast(meanB, m16)
                rstdB = lt.tile([P, 512], F32, name="rstdB1")
                nc.gpsimd.partition_broadcast(rstdB, rstd)
                for k in range(DC):
                    sl = (slice(None), k, ts)
                    t1 = lt.tile([P, 512], F32, name="t1a")
                    eng = nc.gpsimd if k % 2 == 0 else nc.vector
                    eng.tensor_sub(t1, xT8[sl], meanB)             # SA*(x-m)
                    t2 = lt.tile([P, 512], F32, name="t2a")
                    eng2 = nc.gpsimd if k % 2 == 1 else nc.vector
                    eng2.tensor_mul(t2, t1, rstdB)                 # SA*xhat
                    # x1T = SA*(xhat*g + b), alternating ACT/DVE
                    if k % 2 == 0:
                        nc.scalar.activation(out=x1T[sl], in_=t2,
                                             func=Act.Identity,
                                             scale=ln1g_sb[:, k:k + 1],
                                             bias=ln1b_sb[:, k:k + 1])
                    else:
                        nc.vector.tensor_scalar(out=x1T[sl], in0=t2,
                                                scalar1=ln1g_sb[:, k:k + 1],
                                                scalar2=ln1b_sb[:, k:k + 1],
                                                op0=AluOp.mult, op1=AluOp.add)

        # ================= Phase 2: KT, V, qT, rT =================
        kT, fr_kT = mk("kT", (P, DC, FULL), BF16, "right")
        v8, fr_v = mk("v8", (P, TCF, HEAD_NUM, 65), FP8, "right")
        # ones column (value SA) folds the softmax denominator into the AV mm
        nc.vector.memset(v8[:, :, :, 64:65], SA)
        rT, fr_rT = mk("rT", (P, DC, FULL), BF16, "right")
        quT, fr_quT = mk("quT", (P, DC, CUR), BF16, "right")
        qvT, fr_qvT = mk("qvT", (P, DC, CUR), BF16, "right")

        def dr4(ps, w, rhs, nsl, tsl):
            for j in range(DC // 2):
                nc.tensor.matmul(ps, lhsT=w[:, 2 * j:2 * j + 2, nsl],
                                 rhs=rhs[:, 2 * j:2 * j + 2, tsl],
                                 start=(j == 0), stop=(j == DC // 2 - 1),
                                 perf_mode=DR)

        with ExitStack() as ph:
            wkvp = ph.enter_context(tc.tile_pool(name="wkvp", bufs=1, side="right"))
            wkv = wkvp.tile([P, DC, 2 * D], FP8)
            nc.sync.dma_start(out=wkv, in_=wkv_d[:].rearrange("(kc p) n -> p kc n", p=P))
            for n in range(DC):
                for th in range(2):
                    ps = PS()
                    dr4(ps, wkv, x1T, slice(n * P, (n + 1) * P),
                        slice(th * 512, (th + 1) * 512))
                    # k bias is softmax-invariant; kT = psum/SWA (true scale)
                    nc.scalar.activation(out=kT[:, n, th * 512:(th + 1) * 512],
                                         in_=ps, func=Act.Copy, scale=1.0 / SWA)
            for t in range(TCF):
                for nh in range(2):
                    ps = PS()
                    for j in range(DC // 2):
                        nc.tensor.matmul(ps, lhsT=x1T[:, 2 * j:2 * j + 2, t * P:(t + 1) * P],
                                         rhs=wkv[:, 2 * j:2 * j + 2, D + nh * 512:D + (nh + 1) * 512],
                                         start=(j == 0), stop=(j == DC // 2 - 1),
                                         perf_mode=DR)
                    # v8[tok, head, 0:64] = SA * v; col 64 of each slot is SA
                    nc.scalar.activation(
                        out=v8[:, t, nh * 8:(nh + 1) * 8, 0:64],
                        in_=ps[:, :].rearrange("p (h d) -> p h d", d=64),
                        func=Act.Copy, scale=SA / SWA)
        with ExitStack() as ph:
            wqp = ph.enter_context(tc.tile_pool(name="wqp", bufs=1, side="right"))
            wq = wqp.tile([P, DC, D], FP8)
            nc.sync.dma_start(out=wq, in_=wq_d[:].rearrange("(kc p) n -> p kc n", p=P))
            for n in range(DC):
                ps = PS()
                dr4(ps, wq, x1T, slice(n * P, (n + 1) * P), slice(CUR, FULL))
                # u_sb/v_sb hold u+bq / v+bq (host-folded)
                nc.vector.tensor_scalar(out=quT[:, n, :], in0=ps,
                                        scalar1=1.0 / SWA, scalar2=u_sb[:, n:n + 1],
                                        op0=AluOp.mult, op1=AluOp.add)
                nc.vector.tensor_scalar(out=qvT[:, n, :], in0=ps,
                                        scalar1=1.0 / SWA, scalar2=v_sb[:, n:n + 1],
                                        op0=AluOp.mult, op1=AluOp.add)
        with ExitStack() as ph:
            wpp = ph.enter_context(tc.tile_pool(name="wpp", bufs=1, side="right"))
            wpos = wpp.tile([P, DC, D], FP8)
            nc.sync.dma_start(out=wpos, in_=wpos_d[:].rearrange("(kc p) n -> p kc n", p=P))
            posT_sb = wpp.tile([P, DC, FULL], FP8)
            nc.sync.dma_start(out=posT_sb, in_=posT8_d[:].rearrange("(kc p) f -> p kc f", p=P))
            for n in range(DC):
                for fh in range(2):
                    ps = PS()
                    dr4(ps, wpos, posT_sb, slice(n * P, (n + 1) * P),
                        slice(fh * 512, (fh + 1) * 512))
                    # pos bias is softmax-invariant after rel-shift; drop it
                    nc.scalar.activation(out=rT[:, n, fh * 512:(fh + 1) * 512],
                                         in_=ps, func=Act.Copy, scale=1.0 / SWA)
        fr_x1T()

        # ================= Phase 3: attention =================
        # Scores are computed query-major (trimmed to the causal triangle),
        # summed in bf16, PE-transposed into a psum bank per key chunk, and
        # exp-evacuated by ACT straight into fp8 attnT.  The AV matmul runs
        # fp8 DoubleRow against v8 whose 65th column (=SA) yields the softmax
        # denominator as psum row 64 for free.
        avT, fr_avT = mk("avT", (P, DC, CUR), FP8, "left")
        with ExitStack() as ph:
            aw = ph.enter_context(tc.tile_pool(name="aw", bufs=3, side="left"))
            rw = ph.enter_context(tc.tile_pool(name="rw", bufs=2, side="left"))

            def head_slices(h):
                ch, rb = h // 2, (h % 2) * HEAD_DIM
                return (quT[rb:rb + HEAD_DIM, ch, :], qvT[rb:rb + HEAD_DIM, ch, :],
                        kT[rb:rb + HEAD_DIM, ch, :], rT[rb:rb + HEAD_DIM, ch, :])

            def pos_stage(h):
                """Pos scores -> scratch -> single combined shifted read.
                Write at col r+128*ic so the fixed read offset 511 yields the
                reference shift P[i, 511+j-i] with global i."""
                _, qvh, _, rh = head_slices(h)
                s_t = scr[h % 2]
                for ic in range(TCC):
                    w_ic = (ic + 5) * P
                    r_lo = FULL - w_ic
                    pb = aw.tile([P, FULL], BF16, name="pb", bufs=4)
                    for ri, (r0, r1) in enumerate(
                            ((r_lo, min(r_lo + 512, FULL)),
                             (min(r_lo + 512, FULL), FULL))):
                        if r1 <= r0:
                            continue
                        pp = PS()
                        nc.tensor.matmul(pp[:, 0:r1 - r0],
                                         lhsT=qvh[:, ic * P:(ic + 1) * P],
                                         rhs=rh[:, r0:r1], start=True, stop=True)
                        if (2 * ic + ri) % 2 == 0:
                            nc.vector.tensor_copy(pb[:, r0:r1], pp[:, 0:r1 - r0])
                        else:
                            nc.scalar.copy(pb[:, r0:r1], pp[:, 0:r1 - r0])
                    eng = nc.gpsimd if ic % 2 == 0 else nc.sync
                    eng.dma_start(out=s_t[ic, :, 384:1024 + ic * P],
                                  in_=pb[:, r_lo:1024])
                shp = aw.tile([P, TCC, FULL], BF16, name="shp", bufs=2)
                shift_ap = bass.AP(tensor=s_t.tensor, offset=s_t.offset + 511,
                                   ap=[[1535, P], [P * 1536, TCC], [1, FULL]])
                nc.gpsimd.dma_start(out=shp, in_=shift_ap)
                return shp

            def rest_stage(h, shp):
                quh, _, kh, _ = head_slices(h)
                attnT = aw.tile([P, TCF, 512], FP8, name="attnT", bufs=2)
                sms = []
                for ic in range(TCC):
                    w_ic = (ic + 5) * P
                    sm = aw.tile([P, FULL], BF16, name="sm", bufs=8)
                    for j0, j1 in ((0, 512), (512, w_ic)):
                        if j1 <= j0:
                            continue
                        cp = PS()
                        if h % 2 == 0:
                            # fold pos into psum via identity matmul; ACT
                            # evacuates (balances DVE)
                            nc.tensor.matmul(cp[:, 0:j1 - j0],
                                             lhsT=quh[:, ic * P:(ic + 1) * P],
                                             rhs=kh[:, j0:j1], start=True,
                                             stop=False)
                            nc.tensor.matmul(cp[:, 0:j1 - j0], lhsT=ident_b,
                                             rhs=shp[:, ic, j0:j1],
                                             start=False, stop=True)
                            nc.scalar.copy(sm[:, j0:j1], cp[:, 0:j1 - j0])
                        else:
                            nc.tensor.matmul(cp[:, 0:j1 - j0],
                                             lhsT=quh[:, ic * P:(ic + 1) * P],
                                             rhs=kh[:, j0:j1], start=True,
                                             stop=True)
                            nc.vector.tensor_add(sm[:, j0:j1], cp[:, 0:j1 - j0],
                                                 shp[:, ic, j0:j1])
                    sms.append(sm)
                for jc in range(TCF):
                    ic_lo = max(0, jc - 4)
                    tp = psum_t.tile([P, 512], BF16, name="ptb", tag="pt")
                    for ic in range(ic_lo, TCC):
                        nc.tensor.matmul(tp[:, ic * P:(ic + 1) * P],
                                         lhsT=sms[ic][:, jc * P:(jc + 1) * P],
                                         rhs=ident_b, is_transpose=True,
                                         start=(ic == ic_lo), stop=(ic == TCC - 1))
                    nc.scalar.activation(out=attnT[:, jc, ic_lo * P:512],
                                         in_=tp[:, ic_lo * P:512],
                                         func=Act.Exp, scale=SCALE)
                # AV + denominator: rows 0..63 = SA*unnorm, row 64 = SA*den
                av = PS()
                mms = []
                for ic in range(TCC):
                    njc = ic + 5
                    qs = slice(ic * P, (ic + 1) * P)
                    for j in range(0, njc - 1, 2):
                        mms.append((qs, j, True))
                    if njc % 2 == 1:
                        mms.append((qs, njc - 1, False))
                for i, (qs, j, is_dr) in enumerate(mms):
                    fl = dict(start=(i == 0), stop=(i == len(mms) - 1))
                    if is_dr:
                        nc.tensor.matmul(av[0:65, qs], lhsT=v8[:, j:j + 2, h, :],
                                         rhs=attnT[:, j:j + 2, qs],
                                         perf_mode=DR, **fl)
                    else:
                        nc.tensor.matmul(av[0:65, qs], lhsT=v8[:, j, h, :],
                                         rhs=attnT[:, j, qs], **fl)
                ch, rb = h // 2, (h % 2) * HEAD_DIM
                recip = rw.tile([1, 512], F32, name="recip")
                nc.vector.reciprocal(out=recip, in_=av[64:65, :])
                recipB = rw.tile([HEAD_DIM, 512], F32, name="recipB")
                nc.gpsimd.partition_broadcast(recipB, recip)
                # avT8 = SA * av_norm  (SA psum scales cancel in the ratio)
                nc.vector.scalar_tensor_tensor(out=avT[rb:rb + HEAD_DIM, ch, :],
                                               in0=av[0:HEAD_DIM, :], scalar=SA,
                                               in1=recipB, op0=AluOp.mult,
                                               op1=AluOp.mult)

            # software pipeline: pos DMA round-trip of head h+1 overlaps the
            # score/transpose/av work of head h
            pend = pos_stage(0)
            for h in range(HEAD_NUM):
                nxt = pos_stage(h + 1) if h + 1 < HEAD_NUM else None
                rest_stage(h, pend)
                pend = nxt
        fr_qvT(); fr_quT(); fr_rT(); fr_v(); fr_kT()

        # ================= Phase 4: proj + GRU1 =================
        a1T, fr_a1T = mk("a1T", (P, DC, CUR), FP8, "right")
        with ExitStack() as ph:
            wpr = ph.enter_context(tc.tile_pool(name="wpr", bufs=1, side="left"))
            wproj = wpr.tile([P, DC, D], FP8)
            nc.sync.dma_start(out=wproj, in_=wproj_d[:].rearrange("(kc p) n -> p kc n", p=P))
            for n in range(DC):
                ps = PS()
                for j in range(DC // 2):
                    nc.tensor.matmul(ps, lhsT=wproj[:, 2 * j:2 * j + 2, n * P:(n + 1) * P],
                                     rhs=avT[:, 2 * j:2 * j + 2, :],
                                     start=(j == 0), stop=(j == DC // 2 - 1),
                                     perf_mode=DR)
                # a1T8 = SA * relu(pre + bproj); bproj_sb holds SA*bproj
                nc.scalar.activation(out=a1T[:, n, :], in_=ps, func=Act.Relu,
                                     bias=bproj_sb[:, n:n + 1], scale=SA / SWA)
        fr_avT()

        o1T_f, fr_o1f = mk("o1T_f", (P, DC, CUR), F32, "left")
        o1T_8, fr_o1b = mk("o1T_8", (P, DC, CUR), FP8, "left")
        inpT_f, fr_inpf = mk("inpT_f", (P, DC, CUR), F32, "left")
        inpT_8, fr_inpb = mk("inpT_8", (P, DC, CUR), FP8, "left")
        nc.sync.dma_start(out=inpT_f, in_=inpT_d[:].rearrange("(kc p) t -> p kc t", p=P))
        nc.sync.dma_start(out=inpT_8, in_=inpT8_d[:].rearrange("(kc p) t -> p kc t", p=P))
        with ExitStack() as ph:
            _gru(nc, tc, ph, PS, gw_d, 1, a1T, inpT_8, inpT_f, nbg1_sb,
                 o1T_f, o1T_8)
        fr_inpb(); fr_inpf(); fr_a1T()

        # ================= Phase 5: LN2 =================
        x2T, fr_x2T = mk("x2T", (P, DC, CUR), FP8, "right")
        with ExitStack() as ph:
            lw = ph.enter_context(tc.tile_pool(name="lw", bufs=2, side="left"))
            sqp = ph.enter_context(tc.tile_pool(name="sqp", bufs=1, side="left"))
            sq = sqp.tile([P, DC, 512], FP8, name="sq")
            for n in range(DC):
                # 4*o1^2 fits fp8 range comfortably
                nc.vector.scalar_tensor_tensor(out=sq[:, n, :], in0=o1T_f[:, n, :],
                                               scalar=4.0, in1=o1T_f[:, n, :],
                                               op0=AluOp.mult, op1=AluOp.mult)
            s1 = SM()
            for j in range(DC // 2):
                nc.tensor.matmul(s1, lhsT=ones8_t[:, 2 * j:2 * j + 2, 0:1],
                                 rhs=o1T_8[:, 2 * j:2 * j + 2, :],
                                 start=(j == 0), stop=(j == DC // 2 - 1),
                                 perf_mode=DR)
            mean = lw.tile([1, 512], F32, name="mean")
            nc.vector.tensor_scalar_mul(mean, s1, 1.0 / (SA * D))
            s2 = SM()
            for j in range(DC // 2):
                nc.tensor.matmul(s2, lhsT=ones8_t[:, 2 * j:2 * j + 2, 0:1],
                                 rhs=sq[:, 2 * j:2 * j + 2, :],
                                 start=(j == 0), stop=(j == DC // 2 - 1),
                                 perf_mode=DR)
            m2m = lw.tile([1, 512], F32, name="m2m")
            nc.vector.tensor_scalar_mul(m2m, s2, 1.0 / (4 * D))
            var = lw.tile([1, 512], F32, name="var")
            nc.vector.scalar_tensor_tensor(out=var, in0=mean, scalar=1.0,
                                           in1=mean, op0=AluOp.mult,
                                           op1=AluOp.mult)
            nc.vector.tensor_sub(var, m2m, var)
            sd = lw.tile([1, 512], F32, name="sd2")
            nc.scalar.activation(out=sd, in_=var, func=Act.Sqrt,
                                 bias=eps_t[0:1, :])
            rstd = lw.tile([1, 512], F32, name="rstd2")
            nc.vector.reciprocal(out=rstd, in_=sd)
            meanB = lw.tile([P, 512], F32, name="meanB")
            nc.gpsimd.partition_broadcast(meanB, mean)
            rstdB = lw.tile([P, 512], F32, name="rstdB")
            nc.gpsimd.partition_broadcast(rstdB, rstd)
            for n in range(DC):
                t1 = lw.tile([P, 512], F32, name="t1")
                nc.vector.tensor_sub(t1, o1T_f[:, n, :], meanB)
                nc.vector.tensor_mul(t1, t1, rstdB)
                nc.vector.tensor_scalar(out=x2T[:, n, :], in0=t1,
                                        scalar1=ln2g_sb[:, n:n + 1],
                                        scalar2=ln2b_sb[:, n:n + 1],
                                        op0=AluOp.mult, op1=AluOp.add)

        # ================= Phase 6: MLP (fp8 DoubleRow) =================
        with ExitStack() as ph6:
            m1w = ph6.enter_context(tc.tile_pool(name="m1w", bufs=1, side="right"))
            m1T = m1w.tile([P, HC, 512], FP8)
            with ExitStack() as ph:
                w1p = ph.enter_context(tc.tile_pool(name="w1p", bufs=4, side="right"))
                w1r = w1_d[:].rearrange("(kc p) n -> p kc n", p=P)
                for n in range(HC):
                    w1t = w1p.tile([P, DC, P], FP8, name="w1t", tag="w1t")
                    [nc.sync, nc.scalar, nc.gpsimd][n % 3].dma_start(
                        out=w1t, in_=w1r[:, :, n * P:(n + 1) * P])
                    ps = PS()
                    for j in range(DC // 2):
                        nc.tensor.matmul(ps, lhsT=w1t[:, 2 * j:2 * j + 2, :],
                                         rhs=x2T[:, 2 * j:2 * j + 2, :],
                                         start=(j == 0), stop=(j == DC // 2 - 1),
                                         perf_mode=DR)
                    # m1T8 = SA*relu(pre + b1); psum = SWA*pre; b1_sb = SA*b1
                    if _ZERO_BIAS[0] and n % 2 == 1:
                        nc.vector.tensor_scalar(out=m1T[:, n, :], in0=ps,
                                                scalar1=SA / SWA, scalar2=0.0,
                                                op0=AluOp.mult, op1=AluOp.max)
                    else:
                        nc.scalar.activation(out=m1T[:, n, :], in_=ps,
                                             func=Act.Relu,
                                             bias=b1_sb[:, n:n + 1],
                                             scale=SA / SWA)
            m2T, fr_m2T = mk("m2T", (P, DC, CUR), FP8, "left")
            w2p = ph6.enter_context(tc.tile_pool(name="w2p", bufs=3, side="left"))
            w2r = w2_d[:].rearrange("(kc p) n -> p kc n", p=P)
            for n in range(DC):
                w2t = w2p.tile([P, HC, P], FP8, name="w2t", tag="w2t")
                [nc.sync, nc.scalar, nc.gpsimd][n % 3].dma_start(
                    out=w2t, in_=w2r[:, :, n * P:(n + 1) * P])
                ps = PS()
                for j in range(HC // 2):
                    nc.tensor.matmul(ps, lhsT=w2t[:, 2 * j:2 * j + 2, :],
                                     rhs=m1T[:, 2 * j:2 * j + 2, :],
                                     start=(j == 0), stop=(j == HC // 2 - 1),
                                     perf_mode=DR)
                if _ZERO_BIAS[0] and n % 2 == 1:
                    nc.vector.tensor_scalar(out=m2T[:, n, :], in0=ps,
                                            scalar1=SA / SWA, scalar2=0.0,
                                            op0=AluOp.mult, op1=AluOp.max)
                else:
                    nc.scalar.activation(out=m2T[:, n, :], in_=ps, func=Act.Relu,
                                         bias=b2_sb[:, n:n + 1], scale=SA / SWA)
        fr_x2T()

        # ================= Phase 7: GRU2 =================
        o2T_f, fr_o2 = mk("o2T_f", (P, DC, CUR), F32, "right")
        with ExitStack() as ph:
            _gru(nc, tc, ph, PS, gw_d, 2, m2T, o1T_8, o1T_f, nbg2_sb,
                 o2T_f, None)
        fr_m2T(); fr_o1b(); fr_o1f()

        # ================= Phase 8: DMA transposed output =================
        # out_d holds o2 feature-major [D, CUR]; host transposes for free.
        # per-chunk DMAs overlap the GRU2 tail instead of one serial write.
        out_r = out_d[:].rearrange("(kc p) t -> p kc t", p=P)
        for n in range(DC):
            [nc.sync, nc.scalar, nc.gpsimd][n % 3].dma_start(
                out=out_r[:, n, :], in_=o2T_f[:, n, :])
        fr_o2()


def _gru(nc, tc, ph, PS, gw_d, g, yT, xT_8, xT_f, nbg_sb, oT_f, oT_8):
    """fp8 DoubleRow GRU gate. yT/xT_8 hold SA*value in fp8; weights SW*W.
    PSUM accumulates SWA*pre_act; Act applies 1/SWA before the nonlinearity."""
    gwp = ph.enter_context(tc.tile_pool(name=f"gw{g}", bufs=4, side="left"))
    gtmp = ph.enter_context(tc.tile_pool(name=f"gt{g}", bufs=2, side="left"))
    gper = ph.enter_context(tc.tile_pool(name=f"gp{g}", bufs=1, side="left"))

    # spread weight-stream transfer time across issue queues
    qeng = [nc.sync, nc.scalar, nc.gpsimd]
    qi = [0]

    def loadw(m):
        w = gwp.tile([P, DC, D], FP8, name=f"gwt_{m}", tag="gwt")
        qeng[qi[0] % 3].dma_start(
            out=w, in_=gw_d[(g, m)][:].rearrange("(kc p) n -> p kc n", p=P))
        qi[0] += 1
        return w

    def dr_chain(ps, w, u, rhs_w, rhs_u):
        for j in range(DC // 2):
            nc.tensor.matmul(ps, lhsT=w[:, 2 * j:2 * j + 2, n * P:(n + 1) * P],
                             rhs=rhs_w[:, 2 * j:2 * j + 2, :],
                             start=(j == 0), stop=False, perf_mode=DR)
        for j in range(DC // 2):
            nc.tensor.matmul(ps, lhsT=u[:, 2 * j:2 * j + 2, n * P:(n + 1) * P],
                             rhs=rhs_u[:, 2 * j:2 * j + 2, :],
                             start=False, stop=(j == DC // 2 - 1), perf_mode=DR)

    wr, ur = loadw("Wr"), loadw("Ur")
    rx = gper.tile([P, DC, 512], FP8, name="rx")
    for n in range(DC):
        ps = PS()
        dr_chain(ps, wr, ur, yT, xT_8)
        rr = gtmp.tile([P, 512], F32, name="rr")
        nc.scalar.activation(out=rr, in_=ps, func=Act.Sigmoid, scale=1.0 / SWA)
        # rx8 = SA * r * x
        nc.vector.scalar_tensor_tensor(out=rx[:, n, :], in0=rr, scalar=SA,
                                       in1=xT_f[:, n, :], op0=AluOp.mult,
                                       op1=AluOp.mult)
    wz, uz = loadw("Wz"), loadw("Uz")
    zt = gper.tile([P, DC, 512], F32, name="zt")
    for n in range(DC):
        ps = PS()
        dr_chain(ps, wz, uz, yT, xT_8)
        nc.scalar.activation(out=zt[:, n, :], in_=ps, func=Act.Sigmoid,
                             bias=nbg_sb[:, n:n + 1], scale=1.0 / SWA)
    wg, ug = loadw("Wg"), loadw("Ug")
    for n in range(DC):
        ps = PS()
        dr_chain(ps, wg, ug, yT, rx)
        ht = gtmp.tile([P, 512], F32, name="ht")
        nc.scalar.activation(out=ht, in_=ps, func=Act.Tanh, scale=1.0 / SWA)
        nc.gpsimd.tensor_sub(ht, ht, xT_f[:, n, :])
        nc.vector.tensor_mul(ht, ht, zt[:, n, :])
        nc.gpsimd.tensor_add(oT_f[:, n, :], ht, xT_f[:, n, :])
        if oT_8 is not None:
            nc.vector.tensor_scalar_mul(oT_8[:, n, :], oT_f[:, n, :], SA)


_NC_CACHE = {}


def _get_nc():
    if "nc" not in _NC_CACHE:
        _NC_CACHE["nc"] = _build()
    return _NC_CACHE["nc"]


def _chunk_t(vec):
    n = vec.shape[0] // P
    return np.ascontiguousarray(vec.reshape(n, P).T.astype(np.float32))


def _prep(inputs):
    f32 = np.float32
    bf = ml_dtypes.bfloat16
    fp8 = ml_dtypes.float8_e4m3
    inp = np.asarray(inputs["inputs"], f32)
    mem = np.asarray(inputs["memory"], f32)
    pos = np.asarray(inputs["pos_embedding"], f32)[:, 0, :]
    sw, sa = np.float32(SW), np.float32(SA)

    bq = np.asarray(inputs["bq"], f32)
    bkvV = np.asarray(inputs["bkv"], f32)[D:2 * D]
    wproj_f = np.asarray(inputs["Wproj"], f32)
    # v-bias shifts normalized av by a constant vector -> folds into bproj
    bproj_eff = np.asarray(inputs["bproj"], f32) + bkvV @ wproj_f
    shared = {
        "posT8": (sa * pos.T).astype(fp8),
        # bq folds into u and v (k/pos biases are softmax-invariant, dropped)
        "u_t": _chunk_t(np.asarray(inputs["u"], f32).reshape(-1) + bq),
        "v_t": _chunk_t(np.asarray(inputs["v"], f32).reshape(-1) + bq),
        "ln1_g_t": _chunk_t(np.asarray(inputs["ln1_g"], f32)),
        "ln1_b16_t": _chunk_t(sa * np.asarray(inputs["ln1_b"], f32)),
        "ln2_g_t": _chunk_t(sa * np.asarray(inputs["ln2_g"], f32)),
        "ln2_b_t": _chunk_t(sa * np.asarray(inputs["ln2_b"], f32)),
        "bproj_t": _chunk_t(sa * bproj_eff),
        "b1_t": _chunk_t(sa * np.asarray(inputs["mlp_b1"], f32)),
        "b2_t": _chunk_t(sa * np.asarray(inputs["mlp_b2"], f32)),
        "nbg1_t": _chunk_t(-np.asarray(inputs["g1_bg"], f32)),
        "nbg2_t": _chunk_t(-np.asarray(inputs["g2_bg"], f32)),
        "Wkv8": (sw * np.asarray(inputs["Wkv"], f32)).astype(fp8),
        "Wq8": (sw * np.asarray(inputs["Wq"], f32)).astype(fp8),
        "Wpos8": (sw * np.asarray(inputs["Wpos"], f32)).astype(fp8),
        "Wproj8": (sw * wproj_f).astype(fp8),
        "mlp_W18": (sw * np.asarray(inputs["mlp_W1"], f32)).astype(fp8),
        "mlp_W28": (sw * np.asarray(inputs["mlp_W2"], f32)).astype(fp8),
    }
    for g in (1, 2):
        for m in ("Wr", "Ur", "Wz", "Uz", "Wg", "Ug"):
            shared[f"g{g}_{m}8"] = (
                sw * np.asarray(inputs[f"g{g}_{m}"], f32)).astype(fp8)

    in_maps = []
    for b in range(BS):
        im = dict(shared)
        x_full = np.concatenate([mem[:, b, :], inp[:, b, :]], axis=0)
        im["xT8"] = (sa * x_full.T).astype(fp8)
        inpT = np.ascontiguousarray(inp[:, b, :].T)
        im["inpT"] = inpT
        im["inpT8"] = (sa * inpT).astype(fp8)
        in_maps.append(im)
    return in_maps


def kernel(**inputs):
    _ZERO_BIAS[0] = all(
        not np.any(np.asarray(inputs[k]))
        for k in ("mlp_b1", "mlp_b2", "bproj"))
    nc = _get_nc()
    in_maps = _prep(inputs)
    res = run_bass_kernel_spmd(nc, in_maps, core_ids=list(range(BS)))
    # device returns feature-major [D, CUR]; transpose back on host
    out = np.stack([res.results[b]["out"].T for b in range(BS)], axis=1)
    return np.ascontiguousarray(out.astype(np.float32))


if __name__ == "__main__":
    _get_nc()
    print("build+compile OK")



# revision 77
# speedup vs baseline: 1.0844x; 1.0042x over previous
"""GTrXL layer (TransformerXL attention + GRU gating) on 8 TRN2 NeuronCores.

Sharding: pure data-parallel over batch (BS=8 -> 1 batch element per core).
No collectives. Per-core Bass/Tile kernel computes the full layer for its
batch element.

Layout convention on-chip: activations are kept TRANSPOSED [feature, token]
(feature on partitions, 128-chunks) so that weight matrices in natural [K, N]
layout serve as the stationary matmul operand and matmul outputs land
transposed again:  outT[n, t] = sum_k W[k, n] * xT[k, t].

Matmul compute in bf16 (f32 accumulate in PSUM); LN/softmax/GRU elementwise
math in f32.

Relative-shift: pos scores P[i, relk] are written per 128-query-chunk to a
DRAM scratch of row stride 1536 whose tail 512 columns are pre-filled with
-1e30; the shifted read  shifted[i, j] = P[i, 511 + j - i]  is a single
strided DMA (row step 1535), and the pad lands exactly on the masked region
j > i + 512, so masking comes for free.
"""

import sys

if '/opt/trn_rl_repo' not in sys.path:
    sys.path.insert(0, '/opt/trn_rl_repo')

import numpy as np
import ml_dtypes

import concourse.bass as bass
import concourse.tile as tile
from concourse import bacc, mybir
from concourse.bass_utils import run_bass_kernel_spmd
from concourse.masks import make_identity

BF16 = mybir.dt.bfloat16
F32 = mybir.dt.float32
FP8 = mybir.dt.float8e4
DR = mybir.MatmulPerfMode.DoubleRow
SW = 32.0      # fp8 weight scale
SA = 16.0      # fp8 activation scale
SWA = SW * SA  # psum scale for fp8 DoubleRow chains

HEAD_NUM, HEAD_DIM = 16, 64
D, HID = 1024, 4096
CUR, PREV, BS = 512, 512, 8
FULL = CUR + PREV
EPS = 1e-5
SCALE = 1.0 / (HEAD_DIM ** 0.5)
P = 128
DC = D // P          # 8 feature chunks
HC = HID // P        # 32 hidden chunks
TCF = FULL // P      # 8 full-token chunks
TCC = CUR // P       # 4 query-token chunks
NEG = -1.0e30

AluOp = mybir.AluOpType
Act = mybir.ActivationFunctionType

# set from the actual inputs in kernel(); enables 2-op DVE relu evacuations
_ZERO_BIAS = [False]


def _dram_in(dram, name, shape, dtype):
    return dram.tile(list(shape), dtype, kind="ExternalInput", name=name,
                     uniquify=False)


def _mm_chain(nc, psum, lhsT_tiles, rhs_tiles):
    n = len(lhsT_tiles)
    for i in range(n):
        nc.tensor.matmul(psum, lhsT=lhsT_tiles[i], rhs=rhs_tiles[i],
                         start=(i == 0), stop=(i == n - 1))


def _build():
    nc = bacc.Bacc("TRN2", target_bir_lowering=False)
    with tile.TileContext(nc) as tc:
        _emit(nc, tc)
    nc.compile()
    return nc


def _emit(nc, tc):
    from contextlib import ExitStack

    with ExitStack() as root:
        dram = root.enter_context(tc.tile_pool(name="io", bufs=1, space="DRAM"))

        # ---------------- DRAM I/O ----------------
        xT8_d = _dram_in(dram, "xT8", (D, FULL), FP8)
        inpT_d = _dram_in(dram, "inpT", (D, CUR), F32)
        inpT8_d = _dram_in(dram, "inpT8", (D, CUR), FP8)
        posT8_d = _dram_in(dram, "posT8", (D, FULL), FP8)
        u_d = _dram_in(dram, "u_t", (P, DC), F32)
        v_d = _dram_in(dram, "v_t", (P, DC), F32)
        ln1g_d = _dram_in(dram, "ln1_g_t", (P, DC), F32)
        ln1b_d = _dram_in(dram, "ln1_b16_t", (P, DC), F32)
        ln2g_d = _dram_in(dram, "ln2_g_t", (P, DC), F32)
        ln2b_d = _dram_in(dram, "ln2_b_t", (P, DC), F32)
        bproj_d = _dram_in(dram, "bproj_t", (P, DC), F32)
        b1_d = _dram_in(dram, "b1_t", (P, HC), F32)
        b2_d = _dram_in(dram, "b2_t", (P, DC), F32)
        nbg1_d = _dram_in(dram, "nbg1_t", (P, DC), F32)
        nbg2_d = _dram_in(dram, "nbg2_t", (P, DC), F32)

        wkv_d = _dram_in(dram, "Wkv8", (D, 2 * D), FP8)
        wq_d = _dram_in(dram, "Wq8", (D, D), FP8)
        wpos_d = _dram_in(dram, "Wpos8", (D, D), FP8)
        wproj_d = _dram_in(dram, "Wproj8", (D, D), FP8)
        gw_d = {}
        for g in (1, 2):
            for m in ("Wr", "Ur", "Wz", "Uz", "Wg", "Ug"):
                gw_d[(g, m)] = _dram_in(dram, f"g{g}_{m}8", (D, D), FP8)
        w1_d = _dram_in(dram, "mlp_W18", (D, HID), FP8)
        w2_d = _dram_in(dram, "mlp_W28", (HID, D), FP8)

        out_d = dram.tile([D, CUR], F32, kind="ExternalOutput", name="out",
                          uniquify=False)

        # one [4ic, P, 1536] scratch per head-parity; fixed ic slots keep the
        # NEG pad region [1024+128*ic, 1536) stable across reuses
        scr = [dram.tile([TCC, P, 1536], BF16, name=f"scr{s}") for s in range(2)]

        # ---------------- constants ----------------
        const = root.enter_context(tc.tile_pool(name="const", bufs=1))
        ident_b = const.tile([P, P], BF16)
        make_identity(nc, ident_b)
        # pair-dim stride must be a multiple of 16B for DoubleRow ldweights
        ones8_t = const.tile([P, DC, 16], FP8)
        nc.vector.memset(ones8_t, 1.0)
        eps_t = const.tile([P, 1], F32)
        nc.vector.memset(eps_t, EPS)

        def cload(name, dref, shape, dtype=F32):
            t = const.tile(list(shape), dtype, name=name)
            nc.sync.dma_start(out=t, in_=dref[:])
            return t

        u_sb = cload("u_sb", u_d, (P, DC))
        v_sb = cload("v_sb", v_d, (P, DC))
        ln1g_sb = cload("ln1g_sb", ln1g_d, (P, DC))
        ln1b_sb = cload("ln1b_sb", ln1b_d, (P, DC))
        ln2g_sb = cload("ln2g_sb", ln2g_d, (P, DC))
        ln2b_sb = cload("ln2b_sb", ln2b_d, (P, DC))
        bproj_sb = cload("bproj_sb", bproj_d, (P, DC))
        b1_sb = cload("b1_sb", b1_d, (P, HC))
        b2_sb = cload("b2_sb", b2_d, (P, DC))
        nbg1_sb = cload("nbg1_sb", nbg1_d, (P, DC))
        nbg2_sb = cload("nbg2_sb", nbg2_d, (P, DC))

        padw = const.tile([P, 512], BF16)
        nc.vector.memset(padw, NEG)
        for s in range(2):
            for ic in range(TCC):
                off = 1024 + ic * P
                nc.scalar.dma_start(out=scr[s][ic, :, off:1536],
                                    in_=padw[:, 0:1536 - off])

        # shared psum pools (4 + 2 + 2 = 8 banks)
        psum = root.enter_context(tc.tile_pool(name="psum", bufs=4, space="PSUM"))
        psum_t = root.enter_context(tc.tile_pool(name="psum_t", bufs=2, space="PSUM"))
        psum_s = root.enter_context(tc.tile_pool(name="psum_s", bufs=2, space="PSUM"))

        def PS():
            return psum.tile([P, 512], F32, name="ps", tag="ps")

        def SM():
            return psum_s.tile([1, 512], F32, name="sm", tag="sm")

        # lifetime-managed activations (two-sided stack allocator:
        # frees must be LIFO per side, so lifetimes are laid out on
        # left/right stacks to nest properly)
        def mk(name, shape, dtype, side):
            t, fr = tc.tile(list(shape), dtype, name=name, side=side)
            return t, fr

        x1T, fr_x1T = mk("x1T", (P, DC, FULL), FP8, "left")

        # ================= Phase 1: LN1 (feature-major) =================
        # Host supplies xT8 = fp8(SA * x.T).  Stats come from fp8 DR ones-
        # matmuls (mean) and squared copies (var); normalize+affine applies
        # gamma/beta as per-partition scalars, writing x1T = SA*LN(x) in fp8.
        with ExitStack() as ph:
            xw = ph.enter_context(tc.tile_pool(name="xw", bufs=1, side="left"))
            lt = ph.enter_context(tc.tile_pool(name="lt", bufs=3, side="left"))
            xT8 = xw.tile([P, DC, FULL], FP8)
            nc.sync.dma_start(out=xT8, in_=xT8_d[:].rearrange("(kc p) t -> p kc t", p=P))
            sq8 = xw.tile([P, DC, FULL], FP8)
            for k in range(DC):
                for th in range(2):
                    sl = (slice(None), k, slice(th * 512, (th + 1) * 512))
                    if (2 * k + th) % 2 == 0:
                        # 8x^2 = Square(SA*x * sqrt(8)/SA) on the ACT LUT
                        nc.scalar.activation(out=sq8[sl], in_=xT8[sl],
                                             func=Act.Square,
                                             scale=float(np.sqrt(8.0) / SA))
                    else:
                        nc.vector.scalar_tensor_tensor(
                            out=sq8[sl], in0=xT8[sl], scalar=8.0 / (SA * SA),
                            in1=xT8[sl], op0=AluOp.mult, op1=AluOp.mult)
            for th in range(2):
                ts = slice(th * 512, (th + 1) * 512)
                s1 = SM()
                for j in range(DC // 2):
                    nc.tensor.matmul(s1, lhsT=ones8_t[:, 2 * j:2 * j + 2, 0:1],
                                     rhs=xT8[:, 2 * j:2 * j + 2, ts],
                                     start=(j == 0), stop=(j == DC // 2 - 1),
                                     perf_mode=DR)
                s2 = SM()
                for j in range(DC // 2):
                    nc.tensor.matmul(s2, lhsT=ones8_t[:, 2 * j:2 * j + 2, 0:1],
                                     rhs=sq8[:, 2 * j:2 * j + 2, ts],
                                     start=(j == 0), stop=(j == DC // 2 - 1),
                                     perf_mode=DR)
                m16 = lt.tile([1, 512], F32, name="m16")
                nc.vector.tensor_scalar_mul(m16, s1, 1.0 / D)      # SA*mean
                ex2 = lt.tile([1, 512], F32, name="ex2")
                nc.vector.tensor_scalar_mul(ex2, s2, 1.0 / (8 * D))  # E[x^2]
                msq = lt.tile([1, 512], F32, name="msq")
                nc.vector.scalar_tensor_tensor(out=msq, in0=m16,
                                               scalar=1.0 / (SA * SA), in1=m16,
                                               op0=AluOp.mult, op1=AluOp.mult)
                var = lt.tile([1, 512], F32, name="var")
                nc.vector.tensor_sub(var, ex2, msq)
                sd = lt.tile([1, 512], F32, name="sd1")
                nc.scalar.activation(out=sd, in_=var, func=Act.Sqrt,
                                     bias=eps_t[0:1, :])
                rstd = lt.tile([1, 512], F32, name="rstd1")
                nc.vector.reciprocal(out=rstd, in_=sd)
                meanB = lt.tile([P, 512], F32, name="meanB1")
                nc.gpsimd.partition_broadcast(meanB, m16)
                rstdB = lt.tile([P, 512], F32, name="rstdB1")
                nc.gpsimd.partition_broadcast(rstdB, rstd)
                for k in range(DC):
                    sl = (slice(None), k, ts)
                    t1 = lt.tile([P, 512], F32, name="t1a")
                    eng = nc.gpsimd if k % 2 == 0 else nc.vector
                    eng.tensor_sub(t1, xT8[sl], meanB)             # SA*(x-m)
                    t2 = lt.tile([P, 512], F32, name="t2a")
                    eng2 = nc.gpsimd if k % 2 == 1 else nc.vector
                    eng2.tensor_mul(t2, t1, rstdB)                 # SA*xhat
                    # x1T = SA*(xhat*g + b), alternating ACT/DVE
                    if k % 2 == 0:
                        nc.scalar.activation(out=x1T[sl], in_=t2,
                                             func=Act.Identity,
                                             scale=ln1g_sb[:, k:k + 1],
                                             bias=ln1b_sb[:, k:k + 1])
                    else:
                        nc.vector.tensor_scalar(out=x1T[sl], in0=t2,
                                                scalar1=ln1g_sb[:, k:k + 1],
                                                scalar2=ln1b_sb[:, k:k + 1],
                                                op0=AluOp.mult, op1=AluOp.add)

        # ================= Phase 2: KT, V, qT, rT =================
        kT, fr_kT = mk("kT", (P, DC, FULL), BF16, "right")
        v8, fr_v = mk("v8", (P, TCF, HEAD_NUM, 65), FP8, "right")
        # ones column (value SA) folds the softmax denominator into the AV mm
        nc.vector.memset(v8[:, :, :, 64:65], SA)
        rT, fr_rT = mk("rT", (P, DC, FULL), BF16, "right")
        quT, fr_quT = mk("quT", (P, DC, CUR), BF16, "right")
        qvT, fr_qvT = mk("qvT", (P, DC, CUR), BF16, "right")

        def dr4(ps, w, rhs, nsl, tsl):
            for j in range(DC // 2):
                nc.tensor.matmul(ps, lhsT=w[:, 2 * j:2 * j + 2, nsl],
                                 rhs=rhs[:, 2 * j:2 * j + 2, tsl],
                                 start=(j == 0), stop=(j == DC // 2 - 1),
                                 perf_mode=DR)

        with ExitStack() as ph:
            wkvp = ph.enter_context(tc.tile_pool(name="wkvp", bufs=1, side="right"))
            wkv = wkvp.tile([P, DC, 2 * D], FP8)
            nc.sync.dma_start(out=wkv, in_=wkv_d[:].rearrange("(kc p) n -> p kc n", p=P))
            for n in range(DC):
                for th in range(2):
                    ps = PS()
                    dr4(ps, wkv, x1T, slice(n * P, (n + 1) * P),
                        slice(th * 512, (th + 1) * 512))
                    # k bias is softmax-invariant; kT = psum/SWA (true scale)
                    nc.scalar.activation(out=kT[:, n, th * 512:(th + 1) * 512],
                                         in_=ps, func=Act.Copy, scale=1.0 / SWA)
            for t in range(TCF):
                for nh in range(2):
                    ps = PS()
                    for j in range(DC // 2):
                        nc.tensor.matmul(ps, lhsT=x1T[:, 2 * j:2 * j + 2, t * P:(t + 1) * P],
                                         rhs=wkv[:, 2 * j:2 * j + 2, D + nh * 512:D + (nh + 1) * 512],
                                         start=(j == 0), stop=(j == DC // 2 - 1),
                                         perf_mode=DR)
                    # v8[tok, head, 0:64] = SA * v; col 64 of each slot is SA
                    nc.scalar.activation(
                        out=v8[:, t, nh * 8:(nh + 1) * 8, 0:64],
                        in_=ps[:, :].rearrange("p (h d) -> p h d", d=64),
                        func=Act.Copy, scale=SA / SWA)
        with ExitStack() as ph:
            wqp = ph.enter_context(tc.tile_pool(name="wqp", bufs=1, side="right"))
            wq = wqp.tile([P, DC, D], FP8)
            nc.sync.dma_start(out=wq, in_=wq_d[:].rearrange("(kc p) n -> p kc n", p=P))
            for n in range(DC):
                ps = PS()
                dr4(ps, wq, x1T, slice(n * P, (n + 1) * P), slice(CUR, FULL))
                # u_sb/v_sb hold u+bq / v+bq (host-folded)
                nc.vector.tensor_scalar(out=quT[:, n, :], in0=ps,
                                        scalar1=1.0 / SWA, scalar2=u_sb[:, n:n + 1],
                                        op0=AluOp.mult, op1=AluOp.add)
                nc.vector.tensor_scalar(out=qvT[:, n, :], in0=ps,
                                        scalar1=1.0 / SWA, scalar2=v_sb[:, n:n + 1],
                                        op0=AluOp.mult, op1=AluOp.add)
        with ExitStack() as ph:
            wpp = ph.enter_context(tc.tile_pool(name="wpp", bufs=1, side="right"))
            wpos = wpp.tile([P, DC, D], FP8)
            nc.sync.dma_start(out=wpos, in_=wpos_d[:].rearrange("(kc p) n -> p kc n", p=P))
            posT_sb = wpp.tile([P, DC, FULL], FP8)
            nc.sync.dma_start(out=posT_sb, in_=posT8_d[:].rearrange("(kc p) f -> p kc f", p=P))
            for n in range(DC):
                for fh in range(2):
                    ps = PS()
                    dr4(ps, wpos, posT_sb, slice(n * P, (n + 1) * P),
                        slice(fh * 512, (fh + 1) * 512))
                    # pos bias is softmax-invariant after rel-shift; drop it
                    nc.scalar.activation(out=rT[:, n, fh * 512:(fh + 1) * 512],
                                         in_=ps, func=Act.Copy, scale=1.0 / SWA)
        fr_x1T()

        # ================= Phase 3: attention =================
        # Scores are computed query-major (trimmed to the causal triangle),
        # summed in bf16, PE-transposed into a psum bank per key chunk, and
        # exp-evacuated by ACT straight into fp8 attnT.  The AV matmul runs
        # fp8 DoubleRow against v8 whose 65th column (=SA) yields the softmax
        # denominator as psum row 64 for free.
        avT, fr_avT = mk("avT", (P, DC, CUR), FP8, "left")
        with ExitStack() as ph:
            aw = ph.enter_context(tc.tile_pool(name="aw", bufs=3, side="left"))
            rw = ph.enter_context(tc.tile_pool(name="rw", bufs=2, side="left"))

            def head_slices(h):
                ch, rb = h // 2, (h % 2) * HEAD_DIM
                return (quT[rb:rb + HEAD_DIM, ch, :], qvT[rb:rb + HEAD_DIM, ch, :],
                        kT[rb:rb + HEAD_DIM, ch, :], rT[rb:rb + HEAD_DIM, ch, :])

            def pos_stage(h):
                """Pos scores -> scratch -> single combined shifted read.
                Write at col r+128*ic so the fixed read offset 511 yields the
                reference shift P[i, 511+j-i] with global i."""
                _, qvh, _, rh = head_slices(h)
                s_t = scr[h % 2]
                for ic in range(TCC):
                    w_ic = (ic + 5) * P
                    r_lo = FULL - w_ic
                    pb = aw.tile([P, FULL], BF16, name="pb", bufs=4)
                    for ri, (r0, r1) in enumerate(
                            ((r_lo, min(r_lo + 512, FULL)),
                             (min(r_lo + 512, FULL), FULL))):
                        if r1 <= r0:
                            continue
                        pp = PS()
                        nc.tensor.matmul(pp[:, 0:r1 - r0],
                                         lhsT=qvh[:, ic * P:(ic + 1) * P],
                                         rhs=rh[:, r0:r1], start=True, stop=True)
                        if (2 * ic + ri) % 2 == 0:
                            nc.vector.tensor_copy(pb[:, r0:r1], pp[:, 0:r1 - r0])
                        else:
                            nc.scalar.copy(pb[:, r0:r1], pp[:, 0:r1 - r0])
                    eng = nc.gpsimd if ic % 2 == 0 else nc.sync
                    eng.dma_start(out=s_t[ic, :, 384:1024 + ic * P],
                                  in_=pb[:, r_lo:1024])
                shp = aw.tile([P, TCC, FULL], BF16, name="shp", bufs=2)
                shift_ap = bass.AP(tensor=s_t.tensor, offset=s_t.offset + 511,
                                   ap=[[1535, P], [P * 1536, TCC], [1, FULL]])
                nc.gpsimd.dma_start(out=shp, in_=shift_ap)
                return shp

            def rest_stage(h, shp):
                quh, _, kh, _ = head_slices(h)
                attnT = aw.tile([P, TCF, 512], FP8, name="attnT", bufs=2)
                sms = []
                for ic in range(TCC):
                    w_ic = (ic + 5) * P
                    sm = aw.tile([P, FULL], BF16, name="sm", bufs=8)
                    for j0, j1 in ((0, 512), (512, w_ic)):
                        if j1 <= j0:
                            continue
                        cp = PS()
                        if h % 4 != 3:
                            # fold pos into psum via identity matmul; ACT
                            # evacuates (balances DVE)
                            nc.tensor.matmul(cp[:, 0:j1 - j0],
                                             lhsT=quh[:, ic * P:(ic + 1) * P],
                                             rhs=kh[:, j0:j1], start=True,
                                             stop=False)
                            nc.tensor.matmul(cp[:, 0:j1 - j0], lhsT=ident_b,
                                             rhs=shp[:, ic, j0:j1],
                                             start=False, stop=True)
                            nc.scalar.copy(sm[:, j0:j1], cp[:, 0:j1 - j0])
                        else:
                            nc.tensor.matmul(cp[:, 0:j1 - j0],
                                             lhsT=quh[:, ic * P:(ic + 1) * P],
                                             rhs=kh[:, j0:j1], start=True,
                                             stop=True)
                            nc.vector.tensor_add(sm[:, j0:j1], cp[:, 0:j1 - j0],
                                                 shp[:, ic, j0:j1])
                    sms.append(sm)
                for jc in range(TCF):
                    ic_lo = max(0, jc - 4)
                    tp = psum_t.tile([P, 512], BF16, name="ptb", tag="pt")
                    for ic in range(ic_lo, TCC):
                        nc.tensor.matmul(tp[:, ic * P:(ic + 1) * P],
                                         lhsT=sms[ic][:, jc * P:(jc + 1) * P],
                                         rhs=ident_b, is_transpose=True,
                                         start=(ic == ic_lo), stop=(ic == TCC - 1))
                    nc.scalar.activation(out=attnT[:, jc, ic_lo * P:512],
                                         in_=tp[:, ic_lo * P:512],
                                         func=Act.Exp, scale=SCALE)
                # AV + denominator: rows 0..63 = SA*unnorm, row 64 = SA*den
                av = PS()
                mms = []
                for ic in range(TCC):
                    njc = ic + 5
                    qs = slice(ic * P, (ic + 1) * P)
                    for j in range(0, njc - 1, 2):
                        mms.append((qs, j, True))
                    if njc % 2 == 1:
                        mms.append((qs, njc - 1, False))
                for i, (qs, j, is_dr) in enumerate(mms):
                    fl = dict(start=(i == 0), stop=(i == len(mms) - 1))
                    if is_dr:
                        nc.tensor.matmul(av[0:65, qs], lhsT=v8[:, j:j + 2, h, :],
                                         rhs=attnT[:, j:j + 2, qs],
                                         perf_mode=DR, **fl)
                    else:
                        nc.tensor.matmul(av[0:65, qs], lhsT=v8[:, j, h, :],
                                         rhs=attnT[:, j, qs], **fl)
                ch, rb = h // 2, (h % 2) * HEAD_DIM
                recip = rw.tile([1, 512], F32, name="recip")
                nc.vector.reciprocal(out=recip, in_=av[64:65, :])
                recipB = rw.tile([HEAD_DIM, 512], F32, name="recipB")
                nc.gpsimd.partition_broadcast(recipB, recip)
                # avT8 = SA * av_norm  (SA psum scales cancel in the ratio)
                nc.vector.scalar_tensor_tensor(out=avT[rb:rb + HEAD_DIM, ch, :],
                                               in0=av[0:HEAD_DIM, :], scalar=SA,
                                               in1=recipB, op0=AluOp.mult,
                                               op1=AluOp.mult)

            # software pipeline: pos DMA round-trip of head h+1 overlaps the
            # score/transpose/av work of head h
            pend = pos_stage(0)
            for h in range(HEAD_NUM):
                nxt = pos_stage(h + 1) if h + 1 < HEAD_NUM else None
                rest_stage(h, pend)
                pend = nxt
        fr_qvT(); fr_quT(); fr_rT(); fr_v(); fr_kT()

        # ================= Phase 4: proj + GRU1 =================
        a1T, fr_a1T = mk("a1T", (P, DC, CUR), FP8, "right")
        with ExitStack() as ph:
            wpr = ph.enter_context(tc.tile_pool(name="wpr", bufs=1, side="left"))
            wproj = wpr.tile([P, DC, D], FP8)
            nc.sync.dma_start(out=wproj, in_=wproj_d[:].rearrange("(kc p) n -> p kc n", p=P))
            for n in range(DC):
                ps = PS()
                for j in range(DC // 2):
                    nc.tensor.matmul(ps, lhsT=wproj[:, 2 * j:2 * j + 2, n * P:(n + 1) * P],
                                     rhs=avT[:, 2 * j:2 * j + 2, :],
                                     start=(j == 0), stop=(j == DC // 2 - 1),
                                     perf_mode=DR)
                # a1T8 = SA * relu(pre + bproj); bproj_sb holds SA*bproj
                nc.scalar.activation(out=a1T[:, n, :], in_=ps, func=Act.Relu,
                                     bias=bproj_sb[:, n:n + 1], scale=SA / SWA)
        fr_avT()

        o1T_f, fr_o1f = mk("o1T_f", (P, DC, CUR), F32, "left")
        o1T_8, fr_o1b = mk("o1T_8", (P, DC, CUR), FP8, "left")
        inpT_f, fr_inpf = mk("inpT_f", (P, DC, CUR), F32, "left")
        inpT_8, fr_inpb = mk("inpT_8", (P, DC, CUR), FP8, "left")
        nc.sync.dma_start(out=inpT_f, in_=inpT_d[:].rearrange("(kc p) t -> p kc t", p=P))
        nc.sync.dma_start(out=inpT_8, in_=inpT8_d[:].rearrange("(kc p) t -> p kc t", p=P))
        with ExitStack() as ph:
            _gru(nc, tc, ph, PS, gw_d, 1, a1T, inpT_8, inpT_f, nbg1_sb,
                 o1T_f, o1T_8)
        fr_inpb(); fr_inpf(); fr_a1T()

        # ================= Phase 5: LN2 =================
        x2T, fr_x2T = mk("x2T", (P, DC, CUR), FP8, "right")
        with ExitStack() as ph:
            lw = ph.enter_context(tc.tile_pool(name="lw", bufs=2, side="left"))
            sqp = ph.enter_context(tc.tile_pool(name="sqp", bufs=1, side="left"))
            sq = sqp.tile([P, DC, 512], FP8, name="sq")
            for n in range(DC):
                # 4*o1^2 fits fp8 range comfortably
                nc.vector.scalar_tensor_tensor(out=sq[:, n, :], in0=o1T_f[:, n, :],
                                               scalar=4.0, in1=o1T_f[:, n, :],
                                               op0=AluOp.mult, op1=AluOp.mult)
            s1 = SM()
            for j in range(DC // 2):
                nc.tensor.matmul(s1, lhsT=ones8_t[:, 2 * j:2 * j + 2, 0:1],
                                 rhs=o1T_8[:, 2 * j:2 * j + 2, :],
                                 start=(j == 0), stop=(j == DC // 2 - 1),
                                 perf_mode=DR)
            mean = lw.tile([1, 512], F32, name="mean")
            nc.vector.tensor_scalar_mul(mean, s1, 1.0 / (SA * D))
            s2 = SM()
            for j in range(DC // 2):
                nc.tensor.matmul(s2, lhsT=ones8_t[:, 2 * j:2 * j + 2, 0:1],
                                 rhs=sq[:, 2 * j:2 * j + 2, :],
                                 start=(j == 0), stop=(j == DC // 2 - 1),
                                 perf_mode=DR)
            m2m = lw.tile([1, 512], F32, name="m2m")
            nc.vector.tensor_scalar_mul(m2m, s2, 1.0 / (4 * D))
            var = lw.tile([1, 512], F32, name="var")
            nc.vector.scalar_tensor_tensor(out=var, in0=mean, scalar=1.0,
                                           in1=mean, op0=AluOp.mult,
                                           op1=AluOp.mult)
            nc.vector.tensor_sub(var, m2m, var)
            sd = lw.tile([1, 512], F32, name="sd2")
            nc.scalar.activation(out=sd, in_=var, func=Act.Sqrt,
                                 bias=eps_t[0:1, :])
            rstd = lw.tile([1, 512], F32, name="rstd2")
            nc.vector.reciprocal(out=rstd, in_=sd)
            meanB = lw.tile([P, 512], F32, name="meanB")
            nc.gpsimd.partition_broadcast(meanB, mean)
            rstdB = lw.tile([P, 512], F32, name="rstdB")
            nc.gpsimd.partition_broadcast(rstdB, rstd)
            for n in range(DC):
                t1 = lw.tile([P, 512], F32, name="t1")
                nc.vector.tensor_sub(t1, o1T_f[:, n, :], meanB)
                nc.vector.tensor_mul(t1, t1, rstdB)
                nc.vector.tensor_scalar(out=x2T[:, n, :], in0=t1,
                                        scalar1=ln2g_sb[:, n:n + 1],
                                        scalar2=ln2b_sb[:, n:n + 1],
                                        op0=AluOp.mult, op1=AluOp.add)

        # ================= Phase 6: MLP (fp8 DoubleRow) =================
        with ExitStack() as ph6:
            m1w = ph6.enter_context(tc.tile_pool(name="m1w", bufs=1, side="right"))
            m1T = m1w.tile([P, HC, 512], FP8)
            with ExitStack() as ph:
                w1p = ph.enter_context(tc.tile_pool(name="w1p", bufs=4, side="right"))
                w1r = w1_d[:].rearrange("(kc p) n -> p kc n", p=P)
                for n in range(HC):
                    w1t = w1p.tile([P, DC, P], FP8, name="w1t", tag="w1t")
                    [nc.sync, nc.scalar, nc.gpsimd][n % 3].dma_start(
                        out=w1t, in_=w1r[:, :, n * P:(n + 1) * P])
                    ps = PS()
                    for j in range(DC // 2):
                        nc.tensor.matmul(ps, lhsT=w1t[:, 2 * j:2 * j + 2, :],
                                         rhs=x2T[:, 2 * j:2 * j + 2, :],
                                         start=(j == 0), stop=(j == DC // 2 - 1),
                                         perf_mode=DR)
                    # m1T8 = SA*relu(pre + b1); psum = SWA*pre; b1_sb = SA*b1
                    if _ZERO_BIAS[0] and n % 2 == 1:
                        nc.vector.tensor_scalar(out=m1T[:, n, :], in0=ps,
                                                scalar1=SA / SWA, scalar2=0.0,
                                                op0=AluOp.mult, op1=AluOp.max)
                    else:
                        nc.scalar.activation(out=m1T[:, n, :], in_=ps,
                                             func=Act.Relu,
                                             bias=b1_sb[:, n:n + 1],
                                             scale=SA / SWA)
            m2T, fr_m2T = mk("m2T", (P, DC, CUR), FP8, "left")
            w2p = ph6.enter_context(tc.tile_pool(name="w2p", bufs=3, side="left"))
            w2r = w2_d[:].rearrange("(kc p) n -> p kc n", p=P)
            for n in range(DC):
                w2t = w2p.tile([P, HC, P], FP8, name="w2t", tag="w2t")
                [nc.sync, nc.scalar, nc.gpsimd][n % 3].dma_start(
                    out=w2t, in_=w2r[:, :, n * P:(n + 1) * P])
                ps = PS()
                for j in range(HC // 2):
                    nc.tensor.matmul(ps, lhsT=w2t[:, 2 * j:2 * j + 2, :],
                                     rhs=m1T[:, 2 * j:2 * j + 2, :],
                                     start=(j == 0), stop=(j == HC // 2 - 1),
                                     perf_mode=DR)
                if _ZERO_BIAS[0] and n % 2 == 1:
                    nc.vector.tensor_scalar(out=m2T[:, n, :], in0=ps,
                                            scalar1=SA / SWA, scalar2=0.0,
                                            op0=AluOp.mult, op1=AluOp.max)
                else:
                    nc.scalar.activation(out=m2T[:, n, :], in_=ps, func=Act.Relu,
                                         bias=b2_sb[:, n:n + 1], scale=SA / SWA)
        fr_x2T()

        # ================= Phase 7: GRU2 =================
        o2T_f, fr_o2 = mk("o2T_f", (P, DC, CUR), F32, "right")
        with ExitStack() as ph:
            _gru(nc, tc, ph, PS, gw_d, 2, m2T, o1T_8, o1T_f, nbg2_sb,
                 o2T_f, None)
        fr_m2T(); fr_o1b(); fr_o1f()

        # ================= Phase 8: DMA transposed output =================
        # out_d holds o2 feature-major [D, CUR]; host transposes for free.
        # per-chunk DMAs overlap the GRU2 tail instead of one serial write.
        out_r = out_d[:].rearrange("(kc p) t -> p kc t", p=P)
        for n in range(DC):
            [nc.sync, nc.scalar, nc.gpsimd][n % 3].dma_start(
                out=out_r[:, n, :], in_=o2T_f[:, n, :])
        fr_o2()


def _gru(nc, tc, ph, PS, gw_d, g, yT, xT_8, xT_f, nbg_sb, oT_f, oT_8):
    """fp8 DoubleRow GRU gate. yT/xT_8 hold SA*value in fp8; weights SW*W.
    PSUM accumulates SWA*pre_act; Act applies 1/SWA before the nonlinearity."""
    gwp = ph.enter_context(tc.tile_pool(name=f"gw{g}", bufs=4, side="left"))
    gtmp = ph.enter_context(tc.tile_pool(name=f"gt{g}", bufs=2, side="left"))
    gper = ph.enter_context(tc.tile_pool(name=f"gp{g}", bufs=1, side="left"))

    # spread weight-stream transfer time across issue queues
    qeng = [nc.sync, nc.scalar, nc.gpsimd]
    qi = [0]

    def loadw(m):
        w = gwp.tile([P, DC, D], FP8, name=f"gwt_{m}", tag="gwt")
        qeng[qi[0] % 3].dma_start(
            out=w, in_=gw_d[(g, m)][:].rearrange("(kc p) n -> p kc n", p=P))
        qi[0] += 1
        return w

    def dr_chain(ps, w, u, rhs_w, rhs_u):
        for j in range(DC // 2):
            nc.tensor.matmul(ps, lhsT=w[:, 2 * j:2 * j + 2, n * P:(n + 1) * P],
                             rhs=rhs_w[:, 2 * j:2 * j + 2, :],
                             start=(j == 0), stop=False, perf_mode=DR)
        for j in range(DC // 2):
            nc.tensor.matmul(ps, lhsT=u[:, 2 * j:2 * j + 2, n * P:(n + 1) * P],
                             rhs=rhs_u[:, 2 * j:2 * j + 2, :],
                             start=False, stop=(j == DC // 2 - 1), perf_mode=DR)

    wr, ur = loadw("Wr"), loadw("Ur")
    rx = gper.tile([P, DC, 512], FP8, name="rx")
    for n in range(DC):
        ps = PS()
        dr_chain(ps, wr, ur, yT, xT_8)
        rr = gtmp.tile([P, 512], F32, name="rr")
        nc.scalar.activation(out=rr, in_=ps, func=Act.Sigmoid, scale=1.0 / SWA)
        # rx8 = SA * r * x
        nc.vector.scalar_tensor_tensor(out=rx[:, n, :], in0=rr, scalar=SA,
                                       in1=xT_f[:, n, :], op0=AluOp.mult,
                                       op1=AluOp.mult)
    wz, uz = loadw("Wz"), loadw("Uz")
    zt = gper.tile([P, DC, 512], F32, name="zt")
    for n in range(DC):
        ps = PS()
        dr_chain(ps, wz, uz, yT, xT_8)
        nc.scalar.activation(out=zt[:, n, :], in_=ps, func=Act.Sigmoid,
                             bias=nbg_sb[:, n:n + 1], scale=1.0 / SWA)
    wg, ug = loadw("Wg"), loadw("Ug")
    for n in range(DC):
        ps = PS()
        dr_chain(ps, wg, ug, yT, rx)
        ht = gtmp.tile([P, 512], F32, name="ht")
        nc.scalar.activation(out=ht, in_=ps, func=Act.Tanh, scale=1.0 / SWA)
        nc.gpsimd.tensor_sub(ht, ht, xT_f[:, n, :])
        eng_m = nc.gpsimd if n % 2 == 0 else nc.vector
        eng_m.tensor_mul(ht, ht, zt[:, n, :])
        nc.gpsimd.tensor_add(oT_f[:, n, :], ht, xT_f[:, n, :])
        if oT_8 is not None:
            nc.vector.tensor_scalar_mul(oT_8[:, n, :], oT_f[:, n, :], SA)


_NC_CACHE = {}


def _get_nc():
    if "nc" not in _NC_CACHE:
        _NC_CACHE["nc"] = _build()
    return _NC_CACHE["nc"]


def _chunk_t(vec):
    n = vec.shape[0] // P
    return np.ascontiguousarray(vec.reshape(n, P).T.astype(np.float32))


def _prep(inputs):
    f32 = np.float32
    bf = ml_dtypes.bfloat16
    fp8 = ml_dtypes.float8_e4m3
    inp = np.asarray(inputs["inputs"], f32)
    mem = np.asarray(inputs["memory"], f32)
    pos = np.asarray(inputs["pos_embedding"], f32)[:, 0, :]
    sw, sa = np.float32(SW), np.float32(SA)

    bq = np.asarray(inputs["bq"], f32)
    bkvV = np.asarray(inputs["bkv"], f32)[D:2 * D]
    wproj_f = np.asarray(inputs["Wproj"], f32)
    # v-bias shifts normalized av by a constant vector -> folds into bproj
    bproj_eff = np.asarray(inputs["bproj"], f32) + bkvV @ wproj_f
    shared = {
        "posT8": (sa * pos.T).astype(fp8),
        # bq folds into u and v (k/pos biases are softmax-invariant, dropped)
        "u_t": _chunk_t(np.asarray(inputs["u"], f32).reshape(-1) + bq),
        "v_t": _chunk_t(np.asarray(inputs["v"], f32).reshape(-1) + bq),
        "ln1_g_t": _chunk_t(np.asarray(inputs["ln1_g"], f32)),
        "ln1_b16_t": _chunk_t(sa * np.asarray(inputs["ln1_b"], f32)),
        "ln2_g_t": _chunk_t(sa * np.asarray(inputs["ln2_g"], f32)),
        "ln2_b_t": _chunk_t(sa * np.asarray(inputs["ln2_b"], f32)),
        "bproj_t": _chunk_t(sa * bproj_eff),
        "b1_t": _chunk_t(sa * np.asarray(inputs["mlp_b1"], f32)),
        "b2_t": _chunk_t(sa * np.asarray(inputs["mlp_b2"], f32)),
        "nbg1_t": _chunk_t(-np.asarray(inputs["g1_bg"], f32)),
        "nbg2_t": _chunk_t(-np.asarray(inputs["g2_bg"], f32)),
        "Wkv8": (sw * np.asarray(inputs["Wkv"], f32)).astype(fp8),
        "Wq8": (sw * np.asarray(inputs["Wq"], f32)).astype(fp8),
        "Wpos8": (sw * np.asarray(inputs["Wpos"], f32)).astype(fp8),
        "Wproj8": (sw * wproj_f).astype(fp8),
        "mlp_W18": (sw * np.asarray(inputs["mlp_W1"], f32)).astype(fp8),
        "mlp_W28": (sw * np.asarray(inputs["mlp_W2"], f32)).astype(fp8),
    }
    for g in (1, 2):
        for m in ("Wr", "Ur", "Wz", "Uz", "Wg", "Ug"):
            shared[f"g{g}_{m}8"] = (
                sw * np.asarray(inputs[f"g{g}_{m}"], f32)).astype(fp8)

    in_maps = []
    for b in range(BS):
        im = dict(shared)
        x_full = np.concatenate([mem[:, b, :], inp[:, b, :]], axis=0)
        im["xT8"] = (sa * x_full.T).astype(fp8)
        inpT = np.ascontiguousarray(inp[:, b, :].T)
        im["inpT"] = inpT
        im["inpT8"] = (sa * inpT).astype(fp8)
        in_maps.append(im)
    return in_maps


def kernel(**inputs):
    _ZERO_BIAS[0] = all(
        not np.any(np.asarray(inputs[k]))
        for k in ("mlp_b1", "mlp_b2", "bproj"))
    nc = _get_nc()
    in_maps = _prep(inputs)
    res = run_bass_kernel_spmd(nc, in_maps, core_ids=list(range(BS)))
    # device returns feature-major [D, CUR]; transpose back on host
    out = np.stack([res.results[b]["out"].T for b in range(BS)], axis=1)
    return np.ascontiguousarray(out.astype(np.float32))


if __name__ == "__main__":
    _get_nc()
    print("build+compile OK")



# revision 78
# speedup vs baseline: 1.0952x; 1.0100x over previous
"""GTrXL layer (TransformerXL attention + GRU gating) on 8 TRN2 NeuronCores.

Sharding: pure data-parallel over batch (BS=8 -> 1 batch element per core).
No collectives. Per-core Bass/Tile kernel computes the full layer for its
batch element.

Layout convention on-chip: activations are kept TRANSPOSED [feature, token]
(feature on partitions, 128-chunks) so that weight matrices in natural [K, N]
layout serve as the stationary matmul operand and matmul outputs land
transposed again:  outT[n, t] = sum_k W[k, n] * xT[k, t].

Matmul compute in bf16 (f32 accumulate in PSUM); LN/softmax/GRU elementwise
math in f32.

Relative-shift: pos scores P[i, relk] are written per 128-query-chunk to a
DRAM scratch of row stride 1536 whose tail 512 columns are pre-filled with
-1e30; the shifted read  shifted[i, j] = P[i, 511 + j - i]  is a single
strided DMA (row step 1535), and the pad lands exactly on the masked region
j > i + 512, so masking comes for free.
"""

import sys

if '/opt/trn_rl_repo' not in sys.path:
    sys.path.insert(0, '/opt/trn_rl_repo')

import numpy as np
import ml_dtypes

import concourse.bass as bass
import concourse.tile as tile
from concourse import bacc, mybir
from concourse.bass_utils import run_bass_kernel_spmd
from concourse.masks import make_identity

BF16 = mybir.dt.bfloat16
F32 = mybir.dt.float32
FP8 = mybir.dt.float8e4
DR = mybir.MatmulPerfMode.DoubleRow
SW = 32.0      # fp8 weight scale
SA = 16.0      # fp8 activation scale
SWA = SW * SA  # psum scale for fp8 DoubleRow chains

HEAD_NUM, HEAD_DIM = 16, 64
D, HID = 1024, 4096
CUR, PREV, BS = 512, 512, 8
FULL = CUR + PREV
EPS = 1e-5
SCALE = 1.0 / (HEAD_DIM ** 0.5)
P = 128
DC = D // P          # 8 feature chunks
HC = HID // P        # 32 hidden chunks
TCF = FULL // P      # 8 full-token chunks
TCC = CUR // P       # 4 query-token chunks
NEG = -1.0e30

AluOp = mybir.AluOpType
Act = mybir.ActivationFunctionType

# set from the actual inputs in kernel(); enables 2-op DVE relu evacuations
_ZERO_BIAS = [False]


def _dram_in(dram, name, shape, dtype):
    return dram.tile(list(shape), dtype, kind="ExternalInput", name=name,
                     uniquify=False)


def _mm_chain(nc, psum, lhsT_tiles, rhs_tiles):
    n = len(lhsT_tiles)
    for i in range(n):
        nc.tensor.matmul(psum, lhsT=lhsT_tiles[i], rhs=rhs_tiles[i],
                         start=(i == 0), stop=(i == n - 1))


def _build():
    nc = bacc.Bacc("TRN2", target_bir_lowering=False)
    with tile.TileContext(nc) as tc:
        _emit(nc, tc)
    nc.compile()
    return nc


def _emit(nc, tc):
    from contextlib import ExitStack

    with ExitStack() as root:
        dram = root.enter_context(tc.tile_pool(name="io", bufs=1, space="DRAM"))

        # ---------------- DRAM I/O ----------------
        xT8_d = _dram_in(dram, "xT8", (D, FULL), FP8)
        inpT_d = _dram_in(dram, "inpT", (D, CUR), F32)
        inpT8_d = _dram_in(dram, "inpT8", (D, CUR), FP8)
        posT8_d = _dram_in(dram, "posT8", (D, FULL), FP8)
        u_d = _dram_in(dram, "u_t", (P, DC), F32)
        v_d = _dram_in(dram, "v_t", (P, DC), F32)
        ln1g_d = _dram_in(dram, "ln1_g_t", (P, DC), F32)
        ln1b_d = _dram_in(dram, "ln1_b16_t", (P, DC), F32)
        ln2g_d = _dram_in(dram, "ln2_g_t", (P, DC), F32)
        ln2b_d = _dram_in(dram, "ln2_b_t", (P, DC), F32)
        bproj_d = _dram_in(dram, "bproj_t", (P, DC), F32)
        b1_d = _dram_in(dram, "b1_t", (P, HC), F32)
        b2_d = _dram_in(dram, "b2_t", (P, DC), F32)
        nbg1_d = _dram_in(dram, "nbg1_t", (P, DC), F32)
        nbg2_d = _dram_in(dram, "nbg2_t", (P, DC), F32)

        wkv_d = _dram_in(dram, "Wkv8", (D, 2 * D), FP8)
        wq_d = _dram_in(dram, "Wq8", (D, D), FP8)
        wpos_d = _dram_in(dram, "Wpos8", (D, D), FP8)
        wproj_d = _dram_in(dram, "Wproj8", (D, D), FP8)
        gw_d = {}
        for g in (1, 2):
            for m in ("Wr", "Ur", "Wz", "Uz", "Wg", "Ug"):
                gw_d[(g, m)] = _dram_in(dram, f"g{g}_{m}8", (D, D), FP8)
        w1_d = _dram_in(dram, "mlp_W18", (D, HID), FP8)
        w2_d = _dram_in(dram, "mlp_W28", (HID, D), FP8)

        out_d = dram.tile([D, CUR], F32, kind="ExternalOutput", name="out",
                          uniquify=False)

        # one [4ic, P, 1536] scratch per head-parity; fixed ic slots keep the
        # NEG pad region [1024+128*ic, 1536) stable across reuses
        scr = [dram.tile([TCC, P, 1536], BF16, name=f"scr{s}") for s in range(2)]

        # ---------------- constants ----------------
        const = root.enter_context(tc.tile_pool(name="const", bufs=1))
        ident_b = const.tile([P, P], BF16)
        make_identity(nc, ident_b)
        # pair-dim stride must be a multiple of 16B for DoubleRow ldweights
        ones8_t = const.tile([P, DC, 16], FP8)
        nc.vector.memset(ones8_t, 1.0)
        eps_t = const.tile([P, 1], F32)
        nc.vector.memset(eps_t, EPS)

        def cload(name, dref, shape, dtype=F32):
            t = const.tile(list(shape), dtype, name=name)
            nc.sync.dma_start(out=t, in_=dref[:])
            return t

        u_sb = cload("u_sb", u_d, (P, DC))
        v_sb = cload("v_sb", v_d, (P, DC))
        ln1g_sb = cload("ln1g_sb", ln1g_d, (P, DC))
        ln1b_sb = cload("ln1b_sb", ln1b_d, (P, DC))
        ln2g_sb = cload("ln2g_sb", ln2g_d, (P, DC))
        ln2b_sb = cload("ln2b_sb", ln2b_d, (P, DC))
        bproj_sb = cload("bproj_sb", bproj_d, (P, DC))
        b1_sb = cload("b1_sb", b1_d, (P, HC))
        b2_sb = cload("b2_sb", b2_d, (P, DC))
        nbg1_sb = cload("nbg1_sb", nbg1_d, (P, DC))
        nbg2_sb = cload("nbg2_sb", nbg2_d, (P, DC))

        padw = const.tile([P, 512], BF16)
        nc.vector.memset(padw, NEG)
        for s in range(2):
            for ic in range(TCC):
                off = 1024 + ic * P
                nc.scalar.dma_start(out=scr[s][ic, :, off:1536],
                                    in_=padw[:, 0:1536 - off])

        # shared psum pools (4 + 2 + 2 = 8 banks)
        psum = root.enter_context(tc.tile_pool(name="psum", bufs=4, space="PSUM"))
        psum_t = root.enter_context(tc.tile_pool(name="psum_t", bufs=2, space="PSUM"))
        psum_s = root.enter_context(tc.tile_pool(name="psum_s", bufs=2, space="PSUM"))

        def PS():
            return psum.tile([P, 512], F32, name="ps", tag="ps")

        def SM():
            return psum_s.tile([1, 512], F32, name="sm", tag="sm")

        # lifetime-managed activations (two-sided stack allocator:
        # frees must be LIFO per side, so lifetimes are laid out on
        # left/right stacks to nest properly)
        def mk(name, shape, dtype, side):
            t, fr = tc.tile(list(shape), dtype, name=name, side=side)
            return t, fr

        x1T, fr_x1T = mk("x1T", (P, DC, FULL), FP8, "left")

        # ================= Phase 1: LN1 (feature-major) =================
        # Host supplies xT8 = fp8(SA * x.T).  Stats come from fp8 DR ones-
        # matmuls (mean) and squared copies (var); normalize+affine applies
        # gamma/beta as per-partition scalars, writing x1T = SA*LN(x) in fp8.
        with ExitStack() as ph:
            xw = ph.enter_context(tc.tile_pool(name="xw", bufs=1, side="left"))
            lt = ph.enter_context(tc.tile_pool(name="lt", bufs=3, side="left"))
            xT8 = xw.tile([P, DC, FULL], FP8)
            nc.sync.dma_start(out=xT8, in_=xT8_d[:].rearrange("(kc p) t -> p kc t", p=P))
            sq8 = xw.tile([P, DC, FULL], FP8)
            for k in range(DC):
                for th in range(2):
                    sl = (slice(None), k, slice(th * 512, (th + 1) * 512))
                    if (2 * k + th) % 2 == 0:
                        # 8x^2 = Square(SA*x * sqrt(8)/SA) on the ACT LUT
                        nc.scalar.activation(out=sq8[sl], in_=xT8[sl],
                                             func=Act.Square,
                                             scale=float(np.sqrt(8.0) / SA))
                    else:
                        nc.vector.scalar_tensor_tensor(
                            out=sq8[sl], in0=xT8[sl], scalar=8.0 / (SA * SA),
                            in1=xT8[sl], op0=AluOp.mult, op1=AluOp.mult)
            for th in range(2):
                ts = slice(th * 512, (th + 1) * 512)
                s1 = SM()
                for j in range(DC // 2):
                    nc.tensor.matmul(s1, lhsT=ones8_t[:, 2 * j:2 * j + 2, 0:1],
                                     rhs=xT8[:, 2 * j:2 * j + 2, ts],
                                     start=(j == 0), stop=(j == DC // 2 - 1),
                                     perf_mode=DR)
                s2 = SM()
                for j in range(DC // 2):
                    nc.tensor.matmul(s2, lhsT=ones8_t[:, 2 * j:2 * j + 2, 0:1],
                                     rhs=sq8[:, 2 * j:2 * j + 2, ts],
                                     start=(j == 0), stop=(j == DC // 2 - 1),
                                     perf_mode=DR)
                m16 = lt.tile([1, 512], F32, name="m16")
                nc.vector.tensor_scalar_mul(m16, s1, 1.0 / D)      # SA*mean
                ex2 = lt.tile([1, 512], F32, name="ex2")
                nc.vector.tensor_scalar_mul(ex2, s2, 1.0 / (8 * D))  # E[x^2]
                msq = lt.tile([1, 512], F32, name="msq")
                nc.vector.scalar_tensor_tensor(out=msq, in0=m16,
                                               scalar=1.0 / (SA * SA), in1=m16,
                                               op0=AluOp.mult, op1=AluOp.mult)
                var = lt.tile([1, 512], F32, name="var")
                nc.vector.tensor_sub(var, ex2, msq)
                sd = lt.tile([1, 512], F32, name="sd1")
                nc.scalar.activation(out=sd, in_=var, func=Act.Sqrt,
                                     bias=eps_t[0:1, :])
                rstd = lt.tile([1, 512], F32, name="rstd1")
                nc.vector.reciprocal(out=rstd, in_=sd)
                meanB = lt.tile([P, 512], F32, name="meanB1")
                nc.gpsimd.partition_broadcast(meanB, m16)
                rstdB = lt.tile([P, 512], F32, name="rstdB1")
                nc.gpsimd.partition_broadcast(rstdB, rstd)
                for k in range(DC):
                    sl = (slice(None), k, ts)
                    t1 = lt.tile([P, 512], F32, name="t1a")
                    eng = nc.gpsimd if k % 2 == 0 else nc.vector
                    eng.tensor_sub(t1, xT8[sl], meanB)             # SA*(x-m)
                    t2 = lt.tile([P, 512], F32, name="t2a")
                    eng2 = nc.gpsimd if k % 2 == 1 else nc.vector
                    eng2.tensor_mul(t2, t1, rstdB)                 # SA*xhat
                    # x1T = SA*(xhat*g + b), alternating ACT/DVE
                    if k % 2 == 0:
                        nc.scalar.activation(out=x1T[sl], in_=t2,
                                             func=Act.Identity,
                                             scale=ln1g_sb[:, k:k + 1],
                                             bias=ln1b_sb[:, k:k + 1])
                    else:
                        nc.vector.tensor_scalar(out=x1T[sl], in0=t2,
                                                scalar1=ln1g_sb[:, k:k + 1],
                                                scalar2=ln1b_sb[:, k:k + 1],
                                                op0=AluOp.mult, op1=AluOp.add)

        # ================= Phase 2: KT, V, qT, rT =================
        kT, fr_kT = mk("kT", (P, DC, FULL), BF16, "right")
        v8, fr_v = mk("v8", (P, TCF, HEAD_NUM, 65), FP8, "right")
        # ones column (value SA) folds the softmax denominator into the AV mm
        nc.vector.memset(v8[:, :, :, 64:65], SA)
        rT, fr_rT = mk("rT", (P, DC, FULL), BF16, "right")
        quT, fr_quT = mk("quT", (P, DC, CUR), BF16, "right")
        qvT, fr_qvT = mk("qvT", (P, DC, CUR), BF16, "right")

        def dr4(ps, w, rhs, nsl, tsl):
            for j in range(DC // 2):
                nc.tensor.matmul(ps, lhsT=w[:, 2 * j:2 * j + 2, nsl],
                                 rhs=rhs[:, 2 * j:2 * j + 2, tsl],
                                 start=(j == 0), stop=(j == DC // 2 - 1),
                                 perf_mode=DR)

        with ExitStack() as ph:
            wkvp = ph.enter_context(tc.tile_pool(name="wkvp", bufs=1, side="right"))
            wkv = wkvp.tile([P, DC, 2 * D], FP8)
            nc.sync.dma_start(out=wkv, in_=wkv_d[:].rearrange("(kc p) n -> p kc n", p=P))
            for n in range(DC):
                for th in range(2):
                    ps = PS()
                    dr4(ps, wkv, x1T, slice(n * P, (n + 1) * P),
                        slice(th * 512, (th + 1) * 512))
                    # k bias is softmax-invariant; kT = psum/SWA (true scale)
                    nc.scalar.activation(out=kT[:, n, th * 512:(th + 1) * 512],
                                         in_=ps, func=Act.Copy, scale=1.0 / SWA)
            for t in range(TCF):
                for nh in range(2):
                    ps = PS()
                    for j in range(DC // 2):
                        nc.tensor.matmul(ps, lhsT=x1T[:, 2 * j:2 * j + 2, t * P:(t + 1) * P],
                                         rhs=wkv[:, 2 * j:2 * j + 2, D + nh * 512:D + (nh + 1) * 512],
                                         start=(j == 0), stop=(j == DC // 2 - 1),
                                         perf_mode=DR)
                    # v8[tok, head, 0:64] = SA * v; col 64 of each slot is SA
                    nc.scalar.activation(
                        out=v8[:, t, nh * 8:(nh + 1) * 8, 0:64],
                        in_=ps[:, :].rearrange("p (h d) -> p h d", d=64),
                        func=Act.Copy, scale=SA / SWA)
        with ExitStack() as ph:
            wqp = ph.enter_context(tc.tile_pool(name="wqp", bufs=1, side="right"))
            wq = wqp.tile([P, DC, D], FP8)
            nc.sync.dma_start(out=wq, in_=wq_d[:].rearrange("(kc p) n -> p kc n", p=P))
            for n in range(DC):
                ps = PS()
                dr4(ps, wq, x1T, slice(n * P, (n + 1) * P), slice(CUR, FULL))
                # u_sb/v_sb hold u+bq / v+bq (host-folded)
                nc.vector.tensor_scalar(out=quT[:, n, :], in0=ps,
                                        scalar1=1.0 / SWA, scalar2=u_sb[:, n:n + 1],
                                        op0=AluOp.mult, op1=AluOp.add)
                nc.vector.tensor_scalar(out=qvT[:, n, :], in0=ps,
                                        scalar1=1.0 / SWA, scalar2=v_sb[:, n:n + 1],
                                        op0=AluOp.mult, op1=AluOp.add)
        with ExitStack() as ph:
            wpp = ph.enter_context(tc.tile_pool(name="wpp", bufs=1, side="right"))
            wpos = wpp.tile([P, DC, D], FP8)
            nc.sync.dma_start(out=wpos, in_=wpos_d[:].rearrange("(kc p) n -> p kc n", p=P))
            posT_sb = wpp.tile([P, DC, FULL], FP8)
            nc.sync.dma_start(out=posT_sb, in_=posT8_d[:].rearrange("(kc p) f -> p kc f", p=P))
            for n in range(DC):
                for fh in range(2):
                    ps = PS()
                    dr4(ps, wpos, posT_sb, slice(n * P, (n + 1) * P),
                        slice(fh * 512, (fh + 1) * 512))
                    # pos bias is softmax-invariant after rel-shift; drop it
                    nc.scalar.activation(out=rT[:, n, fh * 512:(fh + 1) * 512],
                                         in_=ps, func=Act.Copy, scale=1.0 / SWA)
        fr_x1T()

        # ================= Phase 3: attention =================
        # Scores are computed query-major (trimmed to the causal triangle),
        # summed in bf16, PE-transposed into a psum bank per key chunk, and
        # exp-evacuated by ACT straight into fp8 attnT.  The AV matmul runs
        # fp8 DoubleRow against v8 whose 65th column (=SA) yields the softmax
        # denominator as psum row 64 for free.
        avT, fr_avT = mk("avT", (P, DC, CUR), FP8, "left")
        with ExitStack() as ph:
            aw = ph.enter_context(tc.tile_pool(name="aw", bufs=3, side="left"))
            rw = ph.enter_context(tc.tile_pool(name="rw", bufs=2, side="left"))

            def head_slices(h):
                ch, rb = h // 2, (h % 2) * HEAD_DIM
                return (quT[rb:rb + HEAD_DIM, ch, :], qvT[rb:rb + HEAD_DIM, ch, :],
                        kT[rb:rb + HEAD_DIM, ch, :], rT[rb:rb + HEAD_DIM, ch, :])

            def pos_stage(h):
                """Pos scores -> scratch -> single combined shifted read.
                Write at col r+128*ic so the fixed read offset 511 yields the
                reference shift P[i, 511+j-i] with global i."""
                _, qvh, _, rh = head_slices(h)
                s_t = scr[h % 2]
                for ic in range(TCC):
                    w_ic = (ic + 5) * P
                    r_lo = FULL - w_ic
                    pb = aw.tile([P, FULL], BF16, name="pb", bufs=4)
                    for ri, (r0, r1) in enumerate(
                            ((r_lo, min(r_lo + 512, FULL)),
                             (min(r_lo + 512, FULL), FULL))):
                        if r1 <= r0:
                            continue
                        pp = PS()
                        nc.tensor.matmul(pp[:, 0:r1 - r0],
                                         lhsT=qvh[:, ic * P:(ic + 1) * P],
                                         rhs=rh[:, r0:r1], start=True, stop=True)
                        if (2 * ic + ri) % 2 == 0:
                            nc.vector.tensor_copy(pb[:, r0:r1], pp[:, 0:r1 - r0])
                        else:
                            nc.scalar.copy(pb[:, r0:r1], pp[:, 0:r1 - r0])
                    eng = nc.gpsimd if ic % 2 == 0 else nc.sync
                    eng.dma_start(out=s_t[ic, :, 384:1024 + ic * P],
                                  in_=pb[:, r_lo:1024])
                shp = aw.tile([P, TCC, FULL], BF16, name="shp", bufs=2)
                shift_ap = bass.AP(tensor=s_t.tensor, offset=s_t.offset + 511,
                                   ap=[[1535, P], [P * 1536, TCC], [1, FULL]])
                nc.gpsimd.dma_start(out=shp, in_=shift_ap)
                return shp

            def rest_stage(h, shp):
                quh, _, kh, _ = head_slices(h)
                attnT = aw.tile([P, TCF, 512], FP8, name="attnT", bufs=2)
                sms = []
                for ic in range(TCC):
                    w_ic = (ic + 5) * P
                    sm = aw.tile([P, FULL], BF16, name="sm", bufs=8)
                    for j0, j1 in ((0, 512), (512, w_ic)):
                        if j1 <= j0:
                            continue
                        cp = PS()
                        if h % 4 != 3:
                            # fold pos into psum via identity matmul; ACT
                            # evacuates (balances DVE)
                            nc.tensor.matmul(cp[:, 0:j1 - j0],
                                             lhsT=quh[:, ic * P:(ic + 1) * P],
                                             rhs=kh[:, j0:j1], start=True,
                                             stop=False)
                            nc.tensor.matmul(cp[:, 0:j1 - j0], lhsT=ident_b,
                                             rhs=shp[:, ic, j0:j1],
                                             start=False, stop=True)
                            nc.scalar.copy(sm[:, j0:j1], cp[:, 0:j1 - j0])
                        else:
                            nc.tensor.matmul(cp[:, 0:j1 - j0],
                                             lhsT=quh[:, ic * P:(ic + 1) * P],
                                             rhs=kh[:, j0:j1], start=True,
                                             stop=True)
                            nc.vector.tensor_add(sm[:, j0:j1], cp[:, 0:j1 - j0],
                                                 shp[:, ic, j0:j1])
                    sms.append(sm)
                for jc in range(TCF):
                    ic_lo = max(0, jc - 4)
                    tp = psum_t.tile([P, 512], BF16, name="ptb", tag="pt")
                    for ic in range(ic_lo, TCC):
                        nc.tensor.matmul(tp[:, ic * P:(ic + 1) * P],
                                         lhsT=sms[ic][:, jc * P:(jc + 1) * P],
                                         rhs=ident_b, is_transpose=True,
                                         start=(ic == ic_lo), stop=(ic == TCC - 1))
                    nc.scalar.activation(out=attnT[:, jc, ic_lo * P:512],
                                         in_=tp[:, ic_lo * P:512],
                                         func=Act.Exp, scale=SCALE)
                # AV + denominator: rows 0..63 = SA*unnorm, row 64 = SA*den
                av = PS()
                mms = []
                for ic in range(TCC):
                    njc = ic + 5
                    qs = slice(ic * P, (ic + 1) * P)
                    for j in range(0, njc - 1, 2):
                        mms.append((qs, j, True))
                    if njc % 2 == 1:
                        mms.append((qs, njc - 1, False))
                for i, (qs, j, is_dr) in enumerate(mms):
                    fl = dict(start=(i == 0), stop=(i == len(mms) - 1))
                    if is_dr:
                        nc.tensor.matmul(av[0:65, qs], lhsT=v8[:, j:j + 2, h, :],
                                         rhs=attnT[:, j:j + 2, qs],
                                         perf_mode=DR, **fl)
                    else:
                        nc.tensor.matmul(av[0:65, qs], lhsT=v8[:, j, h, :],
                                         rhs=attnT[:, j, qs], **fl)
                ch, rb = h // 2, (h % 2) * HEAD_DIM
                recip = rw.tile([1, 512], F32, name="recip")
                nc.vector.reciprocal(out=recip, in_=av[64:65, :])
                recipB = rw.tile([HEAD_DIM, 512], F32, name="recipB")
                nc.gpsimd.partition_broadcast(recipB, recip)
                # avT8 = SA * av_norm  (SA psum scales cancel in the ratio)
                nc.vector.scalar_tensor_tensor(out=avT[rb:rb + HEAD_DIM, ch, :],
                                               in0=av[0:HEAD_DIM, :], scalar=SA,
                                               in1=recipB, op0=AluOp.mult,
                                               op1=AluOp.mult)

            # software pipeline: pos DMA round-trip of head h+1 overlaps the
            # score/transpose/av work of head h
            pend = pos_stage(0)
            for h in range(HEAD_NUM):
                nxt = pos_stage(h + 1) if h + 1 < HEAD_NUM else None
                rest_stage(h, pend)
                pend = nxt
        fr_qvT(); fr_quT(); fr_rT(); fr_v(); fr_kT()

        # ================= Phase 4: proj + GRU1 =================
        a1T, fr_a1T = mk("a1T", (P, DC, CUR), FP8, "right")
        with ExitStack() as ph:
            wpr = ph.enter_context(tc.tile_pool(name="wpr", bufs=1, side="left"))
            wproj = wpr.tile([P, DC, D], FP8)
            nc.sync.dma_start(out=wproj, in_=wproj_d[:].rearrange("(kc p) n -> p kc n", p=P))
            for n in range(DC):
                ps = PS()
                for j in range(DC // 2):
                    nc.tensor.matmul(ps, lhsT=wproj[:, 2 * j:2 * j + 2, n * P:(n + 1) * P],
                                     rhs=avT[:, 2 * j:2 * j + 2, :],
                                     start=(j == 0), stop=(j == DC // 2 - 1),
                                     perf_mode=DR)
                # a1T8 = SA * relu(pre + bproj); bproj_sb holds SA*bproj
                nc.scalar.activation(out=a1T[:, n, :], in_=ps, func=Act.Relu,
                                     bias=bproj_sb[:, n:n + 1], scale=SA / SWA)
        fr_avT()

        o1T_f, fr_o1f = mk("o1T_f", (P, DC, CUR), F32, "left")
        o1T_8, fr_o1b = mk("o1T_8", (P, DC, CUR), FP8, "left")
        inpT_f, fr_inpf = mk("inpT_f", (P, DC, CUR), F32, "left")
        inpT_8, fr_inpb = mk("inpT_8", (P, DC, CUR), FP8, "left")
        nc.sync.dma_start(out=inpT_f, in_=inpT_d[:].rearrange("(kc p) t -> p kc t", p=P))
        nc.sync.dma_start(out=inpT_8, in_=inpT8_d[:].rearrange("(kc p) t -> p kc t", p=P))
        with ExitStack() as ph:
            _gru(nc, tc, ph, PS, gw_d, 1, a1T, inpT_8, inpT_f, nbg1_sb,
                 o1T_f, o1T_8)
        fr_inpb(); fr_inpf(); fr_a1T()

        # ================= Phase 5: LN2 =================
        x2T, fr_x2T = mk("x2T", (P, DC, CUR), FP8, "right")
        with ExitStack() as ph:
            lw = ph.enter_context(tc.tile_pool(name="lw", bufs=2, side="left"))
            sqp = ph.enter_context(tc.tile_pool(name="sqp", bufs=1, side="left"))
            sq = sqp.tile([P, DC, 512], FP8, name="sq")
            for n in range(DC):
                # 4*o1^2 fits fp8 range comfortably
                nc.vector.scalar_tensor_tensor(out=sq[:, n, :], in0=o1T_f[:, n, :],
                                               scalar=4.0, in1=o1T_f[:, n, :],
                                               op0=AluOp.mult, op1=AluOp.mult)
            s1 = SM()
            for j in range(DC // 2):
                nc.tensor.matmul(s1, lhsT=ones8_t[:, 2 * j:2 * j + 2, 0:1],
                                 rhs=o1T_8[:, 2 * j:2 * j + 2, :],
                                 start=(j == 0), stop=(j == DC // 2 - 1),
                                 perf_mode=DR)
            mean = lw.tile([1, 512], F32, name="mean")
            nc.vector.tensor_scalar_mul(mean, s1, 1.0 / (SA * D))
            s2 = SM()
            for j in range(DC // 2):
                nc.tensor.matmul(s2, lhsT=ones8_t[:, 2 * j:2 * j + 2, 0:1],
                                 rhs=sq[:, 2 * j:2 * j + 2, :],
                                 start=(j == 0), stop=(j == DC // 2 - 1),
                                 perf_mode=DR)
            m2m = lw.tile([1, 512], F32, name="m2m")
            nc.vector.tensor_scalar_mul(m2m, s2, 1.0 / (4 * D))
            var = lw.tile([1, 512], F32, name="var")
            nc.vector.scalar_tensor_tensor(out=var, in0=mean, scalar=1.0,
                                           in1=mean, op0=AluOp.mult,
                                           op1=AluOp.mult)
            nc.vector.tensor_sub(var, m2m, var)
            sd = lw.tile([1, 512], F32, name="sd2")
            nc.scalar.activation(out=sd, in_=var, func=Act.Sqrt,
                                 bias=eps_t[0:1, :])
            rstd = lw.tile([1, 512], F32, name="rstd2")
            nc.vector.reciprocal(out=rstd, in_=sd)
            meanB = lw.tile([P, 512], F32, name="meanB")
            nc.gpsimd.partition_broadcast(meanB, mean)
            rstdB = lw.tile([P, 512], F32, name="rstdB")
            nc.gpsimd.partition_broadcast(rstdB, rstd)
            for n in range(DC):
                t1 = lw.tile([P, 512], F32, name="t1")
                eng_s = nc.gpsimd if n % 2 == 0 else nc.vector
                eng_s.tensor_sub(t1, o1T_f[:, n, :], meanB)
                nc.vector.tensor_mul(t1, t1, rstdB)
                nc.vector.tensor_scalar(out=x2T[:, n, :], in0=t1,
                                        scalar1=ln2g_sb[:, n:n + 1],
                                        scalar2=ln2b_sb[:, n:n + 1],
                                        op0=AluOp.mult, op1=AluOp.add)

        # ================= Phase 6: MLP (fp8 DoubleRow) =================
        with ExitStack() as ph6:
            m1w = ph6.enter_context(tc.tile_pool(name="m1w", bufs=1, side="right"))
            m1T = m1w.tile([P, HC, 512], FP8)
            with ExitStack() as ph:
                w1p = ph.enter_context(tc.tile_pool(name="w1p", bufs=4, side="right"))
                w1r = w1_d[:].rearrange("(kc p) n -> p kc n", p=P)
                for n in range(HC):
                    w1t = w1p.tile([P, DC, P], FP8, name="w1t", tag="w1t")
                    [nc.sync, nc.scalar, nc.gpsimd][n % 3].dma_start(
                        out=w1t, in_=w1r[:, :, n * P:(n + 1) * P])
                    ps = PS()
                    for j in range(DC // 2):
                        nc.tensor.matmul(ps, lhsT=w1t[:, 2 * j:2 * j + 2, :],
                                         rhs=x2T[:, 2 * j:2 * j + 2, :],
                                         start=(j == 0), stop=(j == DC // 2 - 1),
                                         perf_mode=DR)
                    # m1T8 = SA*relu(pre + b1); psum = SWA*pre; b1_sb = SA*b1
                    if _ZERO_BIAS[0] and n % 3 != 0:
                        nc.vector.tensor_scalar(out=m1T[:, n, :], in0=ps,
                                                scalar1=SA / SWA, scalar2=0.0,
                                                op0=AluOp.mult, op1=AluOp.max)
                    else:
                        nc.scalar.activation(out=m1T[:, n, :], in_=ps,
                                             func=Act.Relu,
                                             bias=b1_sb[:, n:n + 1],
                                             scale=SA / SWA)
            m2T, fr_m2T = mk("m2T", (P, DC, CUR), FP8, "left")
            w2p = ph6.enter_context(tc.tile_pool(name="w2p", bufs=3, side="left"))
            w2r = w2_d[:].rearrange("(kc p) n -> p kc n", p=P)
            for n in range(DC):
                w2t = w2p.tile([P, HC, P], FP8, name="w2t", tag="w2t")
                [nc.sync, nc.scalar, nc.gpsimd][n % 3].dma_start(
                    out=w2t, in_=w2r[:, :, n * P:(n + 1) * P])
                ps = PS()
                for j in range(HC // 2):
                    nc.tensor.matmul(ps, lhsT=w2t[:, 2 * j:2 * j + 2, :],
                                     rhs=m1T[:, 2 * j:2 * j + 2, :],
                                     start=(j == 0), stop=(j == HC // 2 - 1),
                                     perf_mode=DR)
                if _ZERO_BIAS[0] and n % 3 != 0:
                    nc.vector.tensor_scalar(out=m2T[:, n, :], in0=ps,
                                            scalar1=SA / SWA, scalar2=0.0,
                                            op0=AluOp.mult, op1=AluOp.max)
                else:
                    nc.scalar.activation(out=m2T[:, n, :], in_=ps, func=Act.Relu,
                                         bias=b2_sb[:, n:n + 1], scale=SA / SWA)
        fr_x2T()

        # ================= Phase 7: GRU2 =================
        o2T_f, fr_o2 = mk("o2T_f", (P, DC, CUR), F32, "right")
        with ExitStack() as ph:
            _gru(nc, tc, ph, PS, gw_d, 2, m2T, o1T_8, o1T_f, nbg2_sb,
                 o2T_f, None)
        fr_m2T(); fr_o1b(); fr_o1f()

        # ================= Phase 8: DMA transposed output =================
        # out_d holds o2 feature-major [D, CUR]; host transposes for free.
        # per-chunk DMAs overlap the GRU2 tail instead of one serial write.
        out_r = out_d[:].rearrange("(kc p) t -> p kc t", p=P)
        for n in range(DC):
            [nc.sync, nc.scalar, nc.gpsimd][n % 3].dma_start(
                out=out_r[:, n, :], in_=o2T_f[:, n, :])
        fr_o2()


def _gru(nc, tc, ph, PS, gw_d, g, yT, xT_8, xT_f, nbg_sb, oT_f, oT_8):
    """fp8 DoubleRow GRU gate. yT/xT_8 hold SA*value in fp8; weights SW*W.
    PSUM accumulates SWA*pre_act; Act applies 1/SWA before the nonlinearity."""
    gwp = ph.enter_context(tc.tile_pool(name=f"gw{g}", bufs=4, side="left"))
    gtmp = ph.enter_context(tc.tile_pool(name=f"gt{g}", bufs=2, side="left"))
    gper = ph.enter_context(tc.tile_pool(name=f"gp{g}", bufs=1, side="left"))

    # spread weight-stream transfer time across issue queues
    qeng = [nc.sync, nc.scalar, nc.gpsimd]
    qi = [0]

    def loadw(m):
        w = gwp.tile([P, DC, D], FP8, name=f"gwt_{m}", tag="gwt")
        qeng[qi[0] % 3].dma_start(
            out=w, in_=gw_d[(g, m)][:].rearrange("(kc p) n -> p kc n", p=P))
        qi[0] += 1
        return w

    def dr_chain(ps, w, u, rhs_w, rhs_u):
        for j in range(DC // 2):
            nc.tensor.matmul(ps, lhsT=w[:, 2 * j:2 * j + 2, n * P:(n + 1) * P],
                             rhs=rhs_w[:, 2 * j:2 * j + 2, :],
                             start=(j == 0), stop=False, perf_mode=DR)
        for j in range(DC // 2):
            nc.tensor.matmul(ps, lhsT=u[:, 2 * j:2 * j + 2, n * P:(n + 1) * P],
                             rhs=rhs_u[:, 2 * j:2 * j + 2, :],
                             start=False, stop=(j == DC // 2 - 1), perf_mode=DR)

    wr, ur = loadw("Wr"), loadw("Ur")
    rx = gper.tile([P, DC, 512], FP8, name="rx")
    for n in range(DC):
        ps = PS()
        dr_chain(ps, wr, ur, yT, xT_8)
        rr = gtmp.tile([P, 512], F32, name="rr")
        nc.scalar.activation(out=rr, in_=ps, func=Act.Sigmoid, scale=1.0 / SWA)
        # rx8 = SA * r * x
        nc.vector.scalar_tensor_tensor(out=rx[:, n, :], in0=rr, scalar=SA,
                                       in1=xT_f[:, n, :], op0=AluOp.mult,
                                       op1=AluOp.mult)
    wz, uz = loadw("Wz"), loadw("Uz")
    zt = gper.tile([P, DC, 512], F32, name="zt")
    for n in range(DC):
        ps = PS()
        dr_chain(ps, wz, uz, yT, xT_8)
        nc.scalar.activation(out=zt[:, n, :], in_=ps, func=Act.Sigmoid,
                             bias=nbg_sb[:, n:n + 1], scale=1.0 / SWA)
    wg, ug = loadw("Wg"), loadw("Ug")
    for n in range(DC):
        ps = PS()
        dr_chain(ps, wg, ug, yT, rx)
        ht = gtmp.tile([P, 512], F32, name="ht")
        nc.scalar.activation(out=ht, in_=ps, func=Act.Tanh, scale=1.0 / SWA)
        nc.gpsimd.tensor_sub(ht, ht, xT_f[:, n, :])
        eng_m = nc.gpsimd if n % 2 == 0 else nc.vector
        eng_m.tensor_mul(ht, ht, zt[:, n, :])
        nc.gpsimd.tensor_add(oT_f[:, n, :], ht, xT_f[:, n, :])
        if oT_8 is not None:
            nc.vector.tensor_scalar_mul(oT_8[:, n, :], oT_f[:, n, :], SA)


_NC_CACHE = {}


def _get_nc():
    if "nc" not in _NC_CACHE:
        _NC_CACHE["nc"] = _build()
    return _NC_CACHE["nc"]


def _chunk_t(vec):
    n = vec.shape[0] // P
    return np.ascontiguousarray(vec.reshape(n, P).T.astype(np.float32))


def _prep(inputs):
    f32 = np.float32
    bf = ml_dtypes.bfloat16
    fp8 = ml_dtypes.float8_e4m3
    inp = np.asarray(inputs["inputs"], f32)
    mem = np.asarray(inputs["memory"], f32)
    pos = np.asarray(inputs["pos_embedding"], f32)[:, 0, :]
    sw, sa = np.float32(SW), np.float32(SA)

    bq = np.asarray(inputs["bq"], f32)
    bkvV = np.asarray(inputs["bkv"], f32)[D:2 * D]
    wproj_f = np.asarray(inputs["Wproj"], f32)
    # v-bias shifts normalized av by a constant vector -> folds into bproj
    bproj_eff = np.asarray(inputs["bproj"], f32) + bkvV @ wproj_f
    shared = {
        "posT8": (sa * pos.T).astype(fp8),
        # bq folds into u and v (k/pos biases are softmax-invariant, dropped)
        "u_t": _chunk_t(np.asarray(inputs["u"], f32).reshape(-1) + bq),
        "v_t": _chunk_t(np.asarray(inputs["v"], f32).reshape(-1) + bq),
        "ln1_g_t": _chunk_t(np.asarray(inputs["ln1_g"], f32)),
        "ln1_b16_t": _chunk_t(sa * np.asarray(inputs["ln1_b"], f32)),
        "ln2_g_t": _chunk_t(sa * np.asarray(inputs["ln2_g"], f32)),
        "ln2_b_t": _chunk_t(sa * np.asarray(inputs["ln2_b"], f32)),
        "bproj_t": _chunk_t(sa * bproj_eff),
        "b1_t": _chunk_t(sa * np.asarray(inputs["mlp_b1"], f32)),
        "b2_t": _chunk_t(sa * np.asarray(inputs["mlp_b2"], f32)),
        "nbg1_t": _chunk_t(-np.asarray(inputs["g1_bg"], f32)),
        "nbg2_t": _chunk_t(-np.asarray(inputs["g2_bg"], f32)),
        "Wkv8": (sw * np.asarray(inputs["Wkv"], f32)).astype(fp8),
        "Wq8": (sw * np.asarray(inputs["Wq"], f32)).astype(fp8),
        "Wpos8": (sw * np.asarray(inputs["Wpos"], f32)).astype(fp8),
        "Wproj8": (sw * wproj_f).astype(fp8),
        "mlp_W18": (sw * np.asarray(inputs["mlp_W1"], f32)).astype(fp8),
        "mlp_W28": (sw * np.asarray(inputs["mlp_W2"], f32)).astype(fp8),
    }
    for g in (1, 2):
        for m in ("Wr", "Ur", "Wz", "Uz", "Wg", "Ug"):
            shared[f"g{g}_{m}8"] = (
                sw * np.asarray(inputs[f"g{g}_{m}"], f32)).astype(fp8)

    in_maps = []
    for b in range(BS):
        im = dict(shared)
        x_full = np.concatenate([mem[:, b, :], inp[:, b, :]], axis=0)
        im["xT8"] = (sa * x_full.T).astype(fp8)
        inpT = np.ascontiguousarray(inp[:, b, :].T)
        im["inpT"] = inpT
        im["inpT8"] = (sa * inpT).astype(fp8)
        in_maps.append(im)
    return in_maps


def kernel(**inputs):
    _ZERO_BIAS[0] = all(
        not np.any(np.asarray(inputs[k]))
        for k in ("mlp_b1", "mlp_b2", "bproj"))
    nc = _get_nc()
    in_maps = _prep(inputs)
    res = run_bass_kernel_spmd(nc, in_maps, core_ids=list(range(BS)))
    # device returns feature-major [D, CUR]; transpose back on host
    out = np.stack([res.results[b]["out"].T for b in range(BS)], axis=1)
    return np.ascontiguousarray(out.astype(np.float32))


if __name__ == "__main__":
    _get_nc()
    print("build+compile OK")

